# revision 19
# baseline (speedup 1.0000x reference)
"""Trainium2 Bass kernel for nn_Block_523986010339 (PVT-style transformer block).

Sharding: data-parallel over batch B=8 -> one batch element per NeuronCore.
Per-core layouts:
  - residual stream token-major fp32 [128p=token%128, 128t=token//128, 64c]
  - matmul operands channel-major bf16 [c, n], n = 128*y + x
  - LN mean folded into matmul weights via an extra "m*g" row; rsqrt scale
    applied token-major with broadcast APs
  - attention: S^T channel-major, exp without max-subtraction (tiny logits),
    denominator via fused ones-column in the V matmul, divided out after proj
  - MLP: fc1 and 3x3 depthwise conv fused into 9 accumulated matmuls over a
    zero-guarded channel-major layout (row pitch 130)
"""

import functools
import json

import numpy as np
import ml_dtypes

import concourse.bass as bass
import concourse.mybir as mybir
import concourse.tile as tile
from concourse.bass_utils import run_bass_kernel_spmd
from concourse.masks import make_identity

F32 = mybir.dt.float32
BF16 = mybir.dt.bfloat16
BF = ml_dtypes.bfloat16

B, N, C, H, W = 8, 16384, 64, 128, 128
SR, HID, NR = 8, 256, 256
P, T = 128, 128
RP = W + 2          # guarded row pitch
PAD = RP + 1        # head/tail pad so all tap offsets stay in-bounds
NG = PAD + RP * (H + 2) + PAD
AX = mybir.AxisListType
OP = mybir.AluOpType
AF = mybir.ActivationFunctionType


def _split_excess_waits(nc, max_waits=1):
    """walrus in this container rejects >1 sync wait per instruction; move
    excess waits onto injected Drain instructions just before the owner."""
    d = json.loads(mybir.module_to_json_string(nc.m))
    n_split = [0]

    def fix(insts):
        out = []
        for inst in insts:
            si = inst.get("sync_info") or {}
            waits = si.get("on_wait") or []
            if len(waits) > max_waits:
                extra = waits[:-max_waits]
                for i in range(0, len(extra), max_waits):
                    n_split[0] += 1
                    out.append({
                        "name": f"WSPLIT-{n_split[0]}",
                        "opcode": "Drain",
                        "engine": inst["engine"],
                        "ins": [],
                        "outs": [],
                        "is_reset_sema": False,
                        "sync_info": {"on_update": [],
                                      "on_wait": extra[i:i + max_waits]},
                    })
                si["on_wait"] = waits[-max_waits:]
                inst["sync_info"] = si
            out.append(inst)
        return out

    for f in d.get("functions", []):
        for bb in f.get("blocks", []):
            bb["instructions"] = fix(bb["instructions"])
    nc.m = mybir.module_from_json_string(json.dumps(d))


def _ln_stats(nc, sc, big, x_tm, epst, nt):
    """Token-major LN stats: returns (g, mg) tiles [128, nt] fp32 given
    x_tm [128, nt, 64] fp32."""
    sq_scr = big.tile([P, nt * C], BF16, tag="scr2", name="sq")
    xsq_view = sq_scr.rearrange("p (t c) -> p t c", c=C)
    nc.scalar.square(out=sq_scr, in_=x_tm.rearrange("p t c -> p (t c)"))
    s1 = sc.tile([P, nt], F32, tag=f"s1_{nt}")
    s2 = sc.tile([P, nt], F32, tag=f"s2_{nt}")
    nc.vector.tensor_reduce(out=s1, in_=x_tm, axis=AX.X, op=OP.add)
    nc.vector.tensor_reduce(out=s2, in_=xsq_view, axis=AX.X, op=OP.add)
    return _ln_finalize(nc, sc, s1, s2, epst, nt)


def _ln_finalize(nc, sc, s1, s2, epst, nt):
    mean = sc.tile([P, nt], F32, tag=f"mean_{nt}")
    var = sc.tile([P, nt], F32, tag=f"var_{nt}")
    nc.vector.tensor_scalar_mul(out=mean, in0=s1, scalar1=1.0 / C)
    nc.vector.tensor_scalar_mul(out=var, in0=s2, scalar1=1.0 / C)
    mm = sc.tile([P, nt], F32, tag=f"mm_{nt}")
    nc.vector.tensor_tensor(out=mm, in0=mean, in1=mean, op=OP.mult)
    nc.vector.tensor_tensor(out=var, in0=var, in1=mm, op=OP.subtract)
    sd = sc.tile([P, nt], F32, tag=f"sd_{nt}")
    nc.scalar.activation(out=sd, in_=var, func=AF.Sqrt, bias=epst, scale=1.0)
    g = sc.tile([P, nt], F32, tag=f"g_{nt}")
    nc.vector.reciprocal(out=g, in_=sd)
    mg = sc.tile([P, nt], F32, tag=f"mg_{nt}")
    nc.vector.tensor_tensor(out=mg, in0=mean, in1=g, op=OP.mult)
    return g, mg


def _build_nc():
    nc = bass.Bass("TRN2")
    x_d = nc.dram_tensor("x", [N, C], F32, kind="ExternalInput")
    out_d = nc.dram_tensor("out", [N, C], F32, kind="ExternalOutput")
    wq_d = nc.dram_tensor("wq", [C, C], BF16, kind="ExternalInput")
    bq_d = nc.dram_tensor("bq", [C, 1], F32, kind="ExternalInput")
    wsr_d = nc.dram_tensor("wsr", [C, 64, C], BF16, kind="ExternalInput")
    bsr_d = nc.dram_tensor("bsr", [C, 1], F32, kind="ExternalInput")
    wkv_d = nc.dram_tensor("wkv", [C, 2 * C], BF16, kind="ExternalInput")
    bkv_d = nc.dram_tensor("bkv", [2 * C, 1], F32, kind="ExternalInput")
    wpj_d = nc.dram_tensor("wpj", [C + 1, C + 1], BF16, kind="ExternalInput")
    wmp_d = nc.dram_tensor("wmp", [128, 6, 128], BF16, kind="ExternalInput")
    wms_d = nc.dram_tensor("wms", [C, 6, 128], BF16, kind="ExternalInput")
    bg_d = nc.dram_tensor("bg", [128, 2], F32, kind="ExternalInput")
    wf2_d = nc.dram_tensor("wf2", [128, 2, C], BF16, kind="ExternalInput")
    bf2_d = nc.dram_tensor("bf2", [C, 1], F32, kind="ExternalInput")

    with tile.TileContext(nc) as tc:
        with (
            tc.tile_pool(name="consts", bufs=1) as consts,
            tc.tile_pool(name="big", bufs=1) as big,
            tc.tile_pool(name="sc", bufs=2) as sc,
            tc.tile_pool(name="ch", bufs=3) as ch,
            tc.tile_pool(name="psA", bufs=6, space="PSUM") as psA,
            tc.tile_pool(name="psT", bufs=2, space="PSUM") as psT,
        ):
            ident = consts.tile([128, 128], BF16)
            make_identity(nc, ident)
            wq = consts.tile([C, C], BF16)
            nc.gpsimd.dma_start(out=wq, in_=wq_d[:, :])
            wsr = consts.tile([C, 64, C], BF16)
            nc.gpsimd.dma_start(out=wsr, in_=wsr_d[:, :, :])
            wkv = consts.tile([C, 2 * C], BF16)
            nc.gpsimd.dma_start(out=wkv, in_=wkv_d[:, :])
            wpj = consts.tile([C + 1, C + 1], BF16)
            nc.gpsimd.dma_start(out=wpj, in_=wpj_d[:, :])
            wmp = consts.tile([128, 6, 128], BF16)
            nc.gpsimd.dma_start(out=wmp, in_=wmp_d[:, :, :])
            wms = consts.tile([C, 6, 128], BF16)
            nc.gpsimd.dma_start(out=wms, in_=wms_d[:, :, :])
            wf2 = consts.tile([128, 2, C], BF16)
            nc.gpsimd.dma_start(out=wf2, in_=wf2_d[:, :, :])
            bq = consts.tile([C, 1], F32)
            nc.gpsimd.dma_start(out=bq, in_=bq_d[:, :])
            bsr = consts.tile([C, 1], F32)
            nc.gpsimd.dma_start(out=bsr, in_=bsr_d[:, :])
            bkv = consts.tile([2 * C, 1], F32)
            nc.gpsimd.dma_start(out=bkv, in_=bkv_d[:, :])
            bg = consts.tile([128, 2], F32)
            nc.gpsimd.dma_start(out=bg, in_=bg_d[:, :])
            bf2 = consts.tile([C, 1], F32)
            nc.gpsimd.dma_start(out=bf2, in_=bf2_d[:, :])
            epst = consts.tile([P, 1], F32)
            nc.vector.memset(epst, 1e-5)

            # ---- load x (token-major), LN1 stats overlapped per slice ----
            x_tm = big.tile([P, T, C], F32, tag="xr")
            x_v = x_d.rearrange("(t p) c -> p t c", p=P)
            sq_scr = big.tile([P, T * C], BF16, tag="scr2", name="sq")
            sqv = sq_scr.rearrange("p (t c) -> p t c", c=C)
            s1 = sc.tile([P, T], F32, tag="s1")
            s2 = sc.tile([P, T], F32, tag="s2")
            for q8 in range(8):
                sl = slice(16 * q8, 16 * (q8 + 1))
                eng = nc.sync if q8 % 2 == 0 else nc.scalar
                eng.dma_start(out=x_tm[:, sl, :], in_=x_v[:, sl, :])
                nc.scalar.square(out=sqv[:, sl, :], in_=x_tm[:, sl, :])
                nc.vector.tensor_reduce(out=s1[:, sl], in_=x_tm[:, sl, :],
                                        axis=AX.X, op=OP.add)
                nc.vector.tensor_reduce(out=s2[:, sl], in_=sqv[:, sl, :],
                                        axis=AX.X, op=OP.add)
            g1, mg1 = _ln_finalize(nc, sc, s1, s2, epst, T)
            # warm up the PE so HAM is at 8/8 when real matmuls start
            for wd in range(14):
                pw = psT.tile([128, 128], F32, tag="tp", name="pw")
                nc.tensor.matmul(out=pw, lhsT=ident, rhs=ident,
                                 start=True, stop=True)
            a1tm = big.tile([P, T, C], BF16, tag="scr2")
            for q8 in range(8):
                sl = slice(16 * q8, 16 * (q8 + 1))
                nc.vector.tensor_tensor(
                    out=a1tm[:, sl, :], in0=x_tm[:, sl, :],
                    in1=g1[:, sl, None].broadcast_to([P, 16, C]), op=OP.mult)
                nc.vector.tensor_tensor(
                    out=a1tm[:, sl, :], in0=a1tm[:, sl, :],
                    in1=mg1[:, sl, None].broadcast_to([P, 16, C]),
                    op=OP.subtract)

            # transpose A1 to channel-major [64, N]: two tiles per transpose
            a1cm = big.tile([C, N], BF16, tag="acm")
            a1cm_v = a1cm.rearrange("c (j a b n) -> c j a b n", a=4, b=2, n=128)
            a1tm_v = a1tm.rearrange("p t c -> p (t c)")
            for j in range(16):
                pt = psT.tile([128, 4, 128], BF16, tag="tp")
                for k in range(4):
                    tt = 8 * j + 2 * k
                    nc.tensor.transpose(out=pt[:, k, :],
                                        in_=a1tm_v[:, 64 * tt:64 * (tt + 2)],
                                        identity=ident)
                nc.scalar.copy(out=a1cm_v[:, j, :, 0, :], in_=pt[0:C, :, :])
                nc.vector.tensor_copy(out=a1cm_v[:, j, :, 1, :],
                                      in_=pt[C:128, :, :])

            # ---- Q^T = wq @ A1 ----
            qt = big.tile([C, N], BF16, tag="qt")
            for i in range(32):
                ps = psA.tile([128, 512], F32, tag="ps", name="ps")[0:C, :]
                nc.tensor.matmul(out=ps, lhsT=wq,
                                 rhs=a1cm[:, 512 * i:512 * (i + 1)],
                                 start=True, stop=True)
                nc.scalar.activation(out=qt[:, 512 * i:512 * (i + 1)], in_=ps,
                                     func=AF.Identity, bias=bq, scale=1.0)

            # ---- spatial reduction conv (8x8 stride 8) ----
            a1sr = a1cm.rearrange("c (Y ky X kx) -> c ky kx Y X", ky=SR, kx=SR, X=16)
            psr = psA.tile([128, 512], F32, tag="ps", name="ps").rearrange("c (a y x) -> c a y x", a=2, y=16)[0:C, 0, :, :]
            for kk in range(64):
                ky, kx = kk // 8, kk % 8
                nc.tensor.matmul(out=psr, lhsT=wsr[:, kk, :],
                                 rhs=a1sr[:, ky, kx, :, :],
                                 start=(kk == 0), stop=(kk == 63))
            xrcm = consts.tile([C, NR], BF16)
            nc.scalar.activation(out=xrcm.rearrange("c (y x) -> c y x", x=16),
                                 in_=psr, func=AF.Identity,
                                 bias=bsr, scale=1.0)

            # ---- LN on reduced tokens (srn), token-major ----
            xr_tm = consts.tile([P, 2, C], F32)
            for hh in range(2):
                pv = psT.tile([128, C], BF16, tag="tp")
                nc.tensor.transpose(out=pv, in_=xrcm[:, 128 * hh:128 * (hh + 1)],
                                    identity=ident[0:C, 0:C])
                nc.vector.tensor_copy(out=xr_tm[:, hh, :], in_=pv)
            g_r, mg_r = _ln_stats(nc, sc, consts, xr_tm, epst, 2)
            ar_tm = consts.tile([P, 2, C], BF16)
            nc.vector.tensor_tensor(
                out=ar_tm, in0=xr_tm,
                in1=g_r[:, :, None].broadcast_to([P, 2, C]), op=OP.mult)
            mgb = sc.tile([P, 2, C], BF16, tag="mgb")
            nc.vector.tensor_tensor(
                out=mgb, in0=mg_r[:, :, None].broadcast_to([P, 2, C]),
                in1=g_r[:, :, None].broadcast_to([P, 2, C]), op=OP.bypass)
            nc.vector.tensor_tensor(out=ar_tm, in0=ar_tm, in1=mgb, op=OP.subtract)
            arcm = consts.tile([C, NR], BF16)
            for hh in range(2):
                pv = psT.tile([C, 128], BF16, tag="tp")
                nc.tensor.transpose(out=pv, in_=ar_tm[:, hh, :], identity=ident)
                nc.vector.tensor_copy(out=arcm[:, 128 * hh:128 * (hh + 1)], in_=pv)

            # ---- KV ----
            pkv = psA.tile([128, 512], F32, tag="ps", name="ps")[:, 0:NR]
            nc.tensor.matmul(out=pkv, lhsT=wkv, rhs=arcm, start=True, stop=True)
            kvcm = consts.tile([2 * C, NR], BF16)
            nc.scalar.activation(out=kvcm, in_=pkv, func=AF.Identity,
                                 bias=bkv, scale=1.0)
            vp = consts.tile([128, 2, C + 1], BF16)
            nc.vector.memset(vp[:, :, C:C + 1], 1.0)
            for hh in range(2):
                pv = psT.tile([128, C], BF16, tag="tp")
                nc.tensor.transpose(out=pv,
                                    in_=kvcm[C:2 * C, 128 * hh:128 * (hh + 1)],
                                    identity=ident[C:2 * C, C:2 * C])
                nc.vector.tensor_copy(out=vp[:, hh, 0:C], in_=pv)

            # ---- attention, streamed in 512-column chunks ----
            y_tm = big.tile([P, T, C], F32, tag="y")
            sq2 = big.tile([P, T * C], BF16, tag="scr2", name="sq2")
            sq2v = sq2.rearrange("p (t c) -> p t c", c=C)
            s1y = sc.tile([P, T], F32, tag="s1y")
            s2y = sc.tile([P, T], F32, tag="s2y")
            for i in range(32):
                ech = ch.tile([128, 2, 512], BF16, tag="e")
                for hh in range(2):
                    pS = psA.tile([128, 512], F32, tag="ps", name="ps")
                    nc.tensor.matmul(out=pS,
                                     lhsT=kvcm[0:C, 128 * hh:128 * (hh + 1)],
                                     rhs=qt[:, 512 * i:512 * (i + 1)],
                                     start=True, stop=True)
                    nc.scalar.activation(out=ech[:, hh, :], in_=pS, func=AF.Exp)
                pO = psA.tile([128, 512], F32, tag="ps", name="ps")[0:C + 1, :]
                for hh in range(2):
                    nc.tensor.matmul(out=pO, lhsT=vp[:, hh, :],
                                     rhs=ech[:, hh, :],
                                     start=(hh == 0), stop=(hh == 1))
                pod = ch.tile([C + 1, 512], BF16, tag="pod")
                nc.vector.tensor_copy(out=pod, in_=pO)
                ptr = psT.tile([128, 4, C + 1], F32, tag="tp")
                for k in range(4):
                    nc.tensor.matmul(out=ptr[:, k, :],
                                     lhsT=pod[:, 128 * k:128 * (k + 1)],
                                     rhs=wpj, start=True, stop=True)
                rt = sc.tile([P, 4, 1], F32, tag="rt")
                nc.vector.reciprocal(out=rt, in_=ptr[:, :, C:C + 1])
                tmp = ch.tile([P, 4, C], F32, tag="tmp")
                nc.vector.tensor_tensor(out=tmp, in0=ptr[:, :, 0:C],
                                        in1=rt.broadcast_to([P, 4, C]),
                                        op=OP.mult)
                nc.vector.tensor_tensor(out=y_tm[:, 4 * i:4 * (i + 1), :],
                                        in0=tmp, in1=x_tm[:, 4 * i:4 * (i + 1), :],
                                        op=OP.add)
                if i % 4 == 3:
                    sl = slice(16 * (i // 4), 16 * (i // 4 + 1))
                    nc.scalar.square(out=sq2v[:, sl, :], in_=y_tm[:, sl, :])
                    nc.vector.tensor_reduce(out=s1y[:, sl], in_=y_tm[:, sl, :],
                                            axis=AX.X, op=OP.add)
                    nc.vector.tensor_reduce(out=s2y[:, sl], in_=sq2v[:, sl, :],
                                            axis=AX.X, op=OP.add)

            # ---- LN2 ----
            g2, mg2 = _ln_finalize(nc, sc, s1y, s2y, epst, T)
            a2tm = big.tile([P, T, C], BF16, tag="scr2")
            for q8 in range(8):
                sl = slice(16 * q8, 16 * (q8 + 1))
                nc.vector.tensor_tensor(
                    out=a2tm[:, sl, :], in0=y_tm[:, sl, :],
                    in1=g2[:, sl, None].broadcast_to([P, 16, C]), op=OP.mult)
                nc.vector.tensor_tensor(
                    out=a2tm[:, sl, :], in0=a2tm[:, sl, :],
                    in1=mg2[:, sl, None].broadcast_to([P, 16, C]),
                    op=OP.subtract)
            # re-warm PE after the LN2 lull
            for wd in range(8):
                pw = psT.tile([128, 128], F32, tag="tp", name="pw")
                nc.tensor.matmul(out=pw, lhsT=ident, rhs=ident,
                                 start=True, stop=True)

            # ---- A2 guarded channel-major, doubled: rows 64:128 shifted by +1 ----
            a2g = big.tile([128, NG], BF16, tag="acm")
            nc.vector.memset(a2g[:, 0:PAD + RP], 0.0)
            nc.vector.memset(a2g[:, NG - PAD - RP:NG], 0.0)
            a2rows = a2g[0:C, PAD + RP:PAD + RP * (H + 1)].rearrange(
                "c (y w) -> c y w", w=RP)
            a2rowsB = a2g[C:128, PAD + RP:PAD + RP * (H + 1)].rearrange(
                "c (y w) -> c y w", w=RP)
            nc.vector.memset(a2rows[:, :, 0:1], 0.0)
            nc.vector.memset(a2rows[:, :, RP - 1:RP], 0.0)
            nc.vector.memset(a2rowsB[:, :, RP - 2:RP], 0.0)
            a2tm_v = a2tm.rearrange("p t c -> p (t c)")
            for j in range(16):
                pt = psT.tile([128, 4, 128], BF16, tag="tp")
                for k in range(4):
                    tt = 8 * j + 2 * k
                    nc.tensor.transpose(out=pt[:, k, :],
                                        in_=a2tm_v[:, 64 * tt:64 * (tt + 2)],
                                        identity=ident)
                ro = a2rows.rearrange("c (j a b) w -> c j a b w", a=4, b=2)
                nc.scalar.copy(out=ro[:, j, :, 0, 1:W + 1], in_=pt[0:C, :, :])
                nc.vector.tensor_copy(out=ro[:, j, :, 1, 1:W + 1],
                                      in_=pt[C:128, :, :])
                nc.vector.tensor_copy(out=a2rowsB[:, 8 * j:8 * (j + 1), 0:W],
                                      in_=a2rows[:, 8 * j:8 * (j + 1), 1:W + 1])

            # ---- MLP: fused fc1 (+) 3x3 depthwise conv, gelu, fc2 ----
            o2cm = big.tile([C, NG], BF16, tag="qt")  # reuses qt slot
            n_mlp = 33
            for j in range(n_mlp):
                cb = PAD + RP + 512 * j
                size = min(512, PAD + RP * (H + 1) - cb)
                gch = []
                for g in range(2):
                    pG = psA.tile([128, 512], F32, tag="ps", name="ps")
                    for dy in (-1, 0, 1):
                        nc.tensor.matmul(
                            out=pG[:, 0:size], lhsT=wmp[:, 2 * (dy + 1) + g, :],
                            rhs=a2g[:, cb + RP * dy - 1:cb + RP * dy - 1 + size],
                            start=(dy == -1), stop=False)
                    for dy in (-1, 0, 1):
                        nc.tensor.matmul(
                            out=pG[:, 0:size], lhsT=wms[:, 2 * (dy + 1) + g, :],
                            rhs=a2g[0:C, cb + RP * dy + 1:cb + RP * dy + 1 + size],
                            start=False, stop=(dy == 1))
                    gc = ch.tile([128, 512], BF16, tag=f"gc{g}")
                    nc.scalar.activation(out=gc[:, 0:size], in_=pG[:, 0:size],
                                         func=AF.Gelu, bias=bg[:, g:g + 1],
                                         scale=1.0)
                    gch.append(gc)
                pF = psA.tile([128, 512], F32, tag="ps", name="ps")
                for g in range(2):
                    nc.tensor.matmul(out=pF[0:C, 0:size], lhsT=wf2[:, g, :],
                                     rhs=gch[g][:, 0:size],
                                     start=(g == 0), stop=(g == 1))
                nc.scalar.activation(out=o2cm[:, cb:cb + size],
                                     in_=pF[0:C, 0:size], func=AF.Identity,
                                     bias=bf2, scale=1.0)

            # ---- MLP epilogue: transpose back, residual, store ----
            y2_tm = big.tile([P, T, C], F32, tag="xr")  # reuses x_tm slot
            out_v = out_d.rearrange("(t p) c -> p t c", p=P)
            for j in range(32):
                pt2 = psT.tile([128, 4, C], BF16, tag="tp")
                for k in range(4):
                    t = 4 * j + k
                    s = PAD + RP * (t + 1) + 1
                    nc.tensor.transpose(out=pt2[:, k, :],
                                        in_=o2cm[:, s:s + W],
                                        identity=ident[0:C, 0:C])
                nc.vector.tensor_tensor(out=y2_tm[:, 4 * j:4 * (j + 1), :],
                                        in0=pt2, in1=y_tm[:, 4 * j:4 * (j + 1), :],
                                        op=OP.add)
                if j % 4 == 3:
                    q8 = j // 4
                    nc.gpsimd.dma_start(out=out_v[:, 16 * q8:16 * (q8 + 1), :],
                                        in_=y2_tm[:, 16 * q8:16 * (q8 + 1), :])

    _split_excess_waits(nc)
    return nc


@functools.cache
def _get_nc():
    return _build_nc()


def _prep_weights(inp):
    f = lambda v: np.asarray(v, np.float32)
    n1w, n1b = f(inp["n1_w"]), f(inp["n1_b"])
    q_w, q_b = f(inp["q_w"]), f(inp["q_b"])
    kv_w, kv_b = f(inp["kv_w"]), f(inp["kv_b"])
    sr_w, sr_b = f(inp["sr_w"]), f(inp["sr_b"])
    srnw, srnb = f(inp["srn_w"]), f(inp["srn_b"])
    pj_w, pj_b = f(inp["proj_w"]), f(inp["proj_b"])
    n2w, n2b = f(inp["n2_w"]), f(inp["n2_b"])
    f1w, f1b = f(inp["fc1_w"]), f(inp["fc1_b"])
    dww, dwb = f(inp["dw_w"]), f(inp["dw_b"])
    f2w, f2b = f(inp["fc2_w"]), f(inp["fc2_b"])

    scale = (C // 1) ** -0.5
    wq_l = (q_w * n1w[None, :]).T * scale
    bq_l = ((q_w @ n1b + q_b) * scale)[:, None]

    wsr_l = np.zeros((64, C, C), np.float32)
    for kk in range(64):
        ky, kx = kk // 8, kk % 8
        wsr_l[kk, :, :] = (sr_w[:, :, ky, kx] * n1w[None, :]).T
    wsr_l = wsr_l.transpose(1, 0, 2)
    bsr_l = (sr_w.sum((2, 3)) @ n1b + sr_b)[:, None]

    wkv_l = (kv_w * srnw[None, :]).T
    bkv_l = (kv_w @ srnb + kv_b)[:, None]

    wpj_l = np.zeros((C + 1, C + 1), np.float32)
    wpj_l[:C, :C] = pj_w.T
    wpj_l[C, :C] = pj_b
    wpj_l[C, C] = 1.0

    k9 = dww[:, 0, :, :].reshape(HID, 9)          # [256, 9]
    wmp_l = np.zeros((6, 128, 128), np.float32)
    wms_l = np.zeros((6, C, 128), np.float32)
    for dy in range(3):
        for g in range(2):
            Ma = (k9[:, dy * 3 + 0][:, None] * f1w * n2w[None, :])[128 * g:128 * (g + 1)]
            Mb = (k9[:, dy * 3 + 1][:, None] * f1w * n2w[None, :])[128 * g:128 * (g + 1)]
            Mc = (k9[:, dy * 3 + 2][:, None] * f1w * n2w[None, :])[128 * g:128 * (g + 1)]
            wmp_l[2 * dy + g, :C, :] = Ma.T
            wmp_l[2 * dy + g, C:, :] = Mb.T
            wms_l[2 * dy + g, :, :] = Mc.T
    wmp_l = wmp_l.transpose(1, 0, 2)
    wms_l = wms_l.transpose(1, 0, 2)
    bg_full = k9.sum(1) * (f1w @ n2b + f1b) + dwb  # [256]
    bg_l = np.ascontiguousarray(bg_full.reshape(2, 128).T)

    wf2_l = np.stack([f2w[:, :128].T, f2w[:, 128:].T], 0).transpose(1, 0, 2)
    bf2_l = f2b[:, None]

    bfc = lambda a: np.ascontiguousarray(a).astype(BF)
    return {
        "wq": bfc(wq_l), "bq": np.ascontiguousarray(bq_l),
        "wsr": bfc(wsr_l), "bsr": np.ascontiguousarray(bsr_l),
        "wkv": bfc(wkv_l), "bkv": np.ascontiguousarray(bkv_l),
        "wpj": bfc(wpj_l),
        "wmp": bfc(wmp_l), "wms": bfc(wms_l),
        "bg": np.ascontiguousarray(bg_l),
        "wf2": bfc(wf2_l), "bf2": np.ascontiguousarray(bf2_l),
    }


def kernel(trace=False, tmpdir=None, **inputs):
    nc = _get_nc()
    x = np.asarray(inputs["x"], np.float32)
    wts = _prep_weights(inputs)
    in_maps = [dict(wts, x=np.ascontiguousarray(x[b])) for b in range(B)]
    res = run_bass_kernel_spmd(nc, in_maps, core_ids=list(range(8)),
                               trace=trace, tmpdir=tmpdir)
    out = np.stack([res.results[b]["out"] for b in range(B)], 0)
    kernel.last_exec_time_ns = res.exec_time_ns
    return out


# revision 21
# speedup vs baseline: 1.1347x; 1.1347x over previous
"""Trainium2 Bass kernel for nn_Block_523986010339 (PVT-style transformer block).

Sharding: data-parallel over batch B=8 -> one batch element per NeuronCore.
Per-core layouts:
  - residual stream token-major fp32 [128p=token%128, 128t=token//128, 64c]
  - matmul operands channel-major bf16 [c, n], n = 128*y + x
  - LN mean folded into matmul weights via an extra "m*g" row; rsqrt scale
    applied token-major with broadcast APs
  - attention: S^T channel-major, exp without max-subtraction (tiny logits),
    denominator via fused ones-column in the V matmul, divided out after proj
  - MLP: fc1 and 3x3 depthwise conv fused into 9 accumulated matmuls over a
    zero-guarded channel-major layout (row pitch 130)
"""

import functools
import json

import numpy as np
import ml_dtypes

import concourse.bass as bass
import concourse.mybir as mybir
import concourse.tile as tile
from concourse.bass_utils import run_bass_kernel_spmd
from concourse.masks import make_identity

F32 = mybir.dt.float32
BF16 = mybir.dt.bfloat16
BF = ml_dtypes.bfloat16

B, N, C, H, W = 8, 16384, 64, 128, 128
SR, HID, NR = 8, 256, 256
P, T = 128, 128
RP = W + 2          # guarded row pitch
PAD = RP + 1        # head/tail pad so all tap offsets stay in-bounds
NG = PAD + RP * (H + 2) + PAD
AX = mybir.AxisListType
OP = mybir.AluOpType
AF = mybir.ActivationFunctionType


def _split_excess_waits(nc, max_waits=1):
    """walrus in this container rejects >1 sync wait per instruction; move
    excess waits onto injected Drain instructions just before the owner."""
    d = json.loads(mybir.module_to_json_string(nc.m))
    n_split = [0]

    def fix(insts):
        out = []
        for inst in insts:
            si = inst.get("sync_info") or {}
            waits = si.get("on_wait") or []
            if len(waits) > max_waits:
                extra = waits[:-max_waits]
                for i in range(0, len(extra), max_waits):
                    n_split[0] += 1
                    out.append({
                        "name": f"WSPLIT-{n_split[0]}",
                        "opcode": "Drain",
                        "engine": inst["engine"],
                        "ins": [],
                        "outs": [],
                        "is_reset_sema": False,
                        "sync_info": {"on_update": [],
                                      "on_wait": extra[i:i + max_waits]},
                    })
                si["on_wait"] = waits[-max_waits:]
                inst["sync_info"] = si
            out.append(inst)
        return out

    for f in d.get("functions", []):
        for bb in f.get("blocks", []):
            bb["instructions"] = fix(bb["instructions"])
    nc.m = mybir.module_from_json_string(json.dumps(d))


def _ln_stats(nc, sc, big, x_tm, epst, nt):
    """Token-major LN stats: returns (g, mg) tiles [128, nt] fp32 given
    x_tm [128, nt, 64] fp32."""
    sq_scr = big.tile([P, nt * C], BF16, tag="scr2", name="sq")
    xsq_view = sq_scr.rearrange("p (t c) -> p t c", c=C)
    nc.scalar.square(out=sq_scr, in_=x_tm.rearrange("p t c -> p (t c)"))
    s1 = sc.tile([P, nt], F32, tag=f"s1_{nt}")
    s2 = sc.tile([P, nt], F32, tag=f"s2_{nt}")
    nc.vector.tensor_reduce(out=s1, in_=x_tm, axis=AX.X, op=OP.add)
    nc.vector.tensor_reduce(out=s2, in_=xsq_view, axis=AX.X, op=OP.add)
    return _ln_finalize(nc, sc, s1, s2, epst, nt)


def _ln_finalize(nc, sc, s1, s2, epst, nt):
    mean = sc.tile([P, nt], F32, tag=f"mean_{nt}")
    var = sc.tile([P, nt], F32, tag=f"var_{nt}")
    nc.vector.tensor_scalar_mul(out=mean, in0=s1, scalar1=1.0 / C)
    nc.vector.tensor_scalar_mul(out=var, in0=s2, scalar1=1.0 / C)
    mm = sc.tile([P, nt], F32, tag=f"mm_{nt}")
    nc.vector.tensor_tensor(out=mm, in0=mean, in1=mean, op=OP.mult)
    nc.vector.tensor_tensor(out=var, in0=var, in1=mm, op=OP.subtract)
    sd = sc.tile([P, nt], F32, tag=f"sd_{nt}")
    nc.scalar.activation(out=sd, in_=var, func=AF.Sqrt, bias=epst, scale=1.0)
    g = sc.tile([P, nt], F32, tag=f"g_{nt}")
    nc.vector.reciprocal(out=g, in_=sd)
    mg = sc.tile([P, nt], F32, tag=f"mg_{nt}")
    nc.vector.tensor_tensor(out=mg, in0=mean, in1=g, op=OP.mult)
    return g, mg


def _build_nc():
    nc = bass.Bass("TRN2")
    x_d = nc.dram_tensor("x", [N, C], F32, kind="ExternalInput")
    out_d = nc.dram_tensor("out", [N, C], F32, kind="ExternalOutput")
    wq_d = nc.dram_tensor("wq", [C, C], BF16, kind="ExternalInput")
    bq_d = nc.dram_tensor("bq", [C, 1], F32, kind="ExternalInput")
    wsr_d = nc.dram_tensor("wsr", [C, 64, C], BF16, kind="ExternalInput")
    bsr_d = nc.dram_tensor("bsr", [C, 1], F32, kind="ExternalInput")
    wkv_d = nc.dram_tensor("wkv", [C, 2 * C], BF16, kind="ExternalInput")
    bkv_d = nc.dram_tensor("bkv", [2 * C, 1], F32, kind="ExternalInput")
    wpj_d = nc.dram_tensor("wpj", [C + 1, C + 1], BF16, kind="ExternalInput")
    wmp_d = nc.dram_tensor("wmp", [128, 6, 128], BF16, kind="ExternalInput")
    wms_d = nc.dram_tensor("wms", [C, 6, 128], BF16, kind="ExternalInput")
    bg_d = nc.dram_tensor("bg", [128, 2], F32, kind="ExternalInput")
    wf2_d = nc.dram_tensor("wf2", [128, 2, C], BF16, kind="ExternalInput")
    bf2_d = nc.dram_tensor("bf2", [C, 1], F32, kind="ExternalInput")

    with tile.TileContext(nc) as tc:
        with (
            tc.tile_pool(name="consts", bufs=1) as consts,
            tc.tile_pool(name="big", bufs=1) as big,
            tc.tile_pool(name="sc", bufs=2) as sc,
            tc.tile_pool(name="ch", bufs=3) as ch,
            tc.tile_pool(name="psA", bufs=6, space="PSUM") as psA,
            tc.tile_pool(name="psT", bufs=2, space="PSUM") as psT,
        ):
            ident = consts.tile([128, 128], BF16)
            make_identity(nc, ident)
            wq = consts.tile([C, C], BF16)
            nc.gpsimd.dma_start(out=wq, in_=wq_d[:, :])
            wsr = consts.tile([C, 64, C], BF16)
            nc.gpsimd.dma_start(out=wsr, in_=wsr_d[:, :, :])
            wkv = consts.tile([C, 2 * C], BF16)
            nc.gpsimd.dma_start(out=wkv, in_=wkv_d[:, :])
            wpj = consts.tile([C + 1, C + 1], BF16)
            nc.gpsimd.dma_start(out=wpj, in_=wpj_d[:, :])
            wmp = consts.tile([128, 6, 128], BF16)
            nc.gpsimd.dma_start(out=wmp, in_=wmp_d[:, :, :])
            wms = consts.tile([C, 6, 128], BF16)
            nc.gpsimd.dma_start(out=wms, in_=wms_d[:, :, :])
            wf2 = consts.tile([128, 2, C], BF16)
            nc.gpsimd.dma_start(out=wf2, in_=wf2_d[:, :, :])
            bq = consts.tile([C, 1], F32)
            nc.gpsimd.dma_start(out=bq, in_=bq_d[:, :])
            bsr = consts.tile([C, 1], F32)
            nc.gpsimd.dma_start(out=bsr, in_=bsr_d[:, :])
            bkv = consts.tile([2 * C, 1], F32)
            nc.gpsimd.dma_start(out=bkv, in_=bkv_d[:, :])
            bg = consts.tile([128, 2], F32)
            nc.gpsimd.dma_start(out=bg, in_=bg_d[:, :])
            bf2 = consts.tile([C, 1], F32)
            nc.gpsimd.dma_start(out=bf2, in_=bf2_d[:, :])
            epst = consts.tile([P, 1], F32)
            nc.vector.memset(epst, 1e-5)

            # ---- load x (token-major), LN1 stats overlapped per slice ----
            x_tm = big.tile([P, T, C], F32, tag="xr")
            x_v = x_d.rearrange("(t p) c -> p t c", p=P)
            sq_scr = big.tile([P, T * C], BF16, tag="scr2", name="sq")
            sqv = sq_scr.rearrange("p (t c) -> p t c", c=C)
            s1 = sc.tile([P, T], F32, tag="s1")
            s2 = sc.tile([P, T], F32, tag="s2")
            for q8 in range(8):
                sl = slice(16 * q8, 16 * (q8 + 1))
                eng = nc.sync if q8 % 2 == 0 else nc.scalar
                eng.dma_start(out=x_tm[:, sl, :], in_=x_v[:, sl, :])
                nc.scalar.square(out=sqv[:, sl, :], in_=x_tm[:, sl, :])
                nc.vector.tensor_reduce(out=s1[:, sl], in_=x_tm[:, sl, :],
                                        axis=AX.X, op=OP.add)
                nc.vector.tensor_reduce(out=s2[:, sl], in_=sqv[:, sl, :],
                                        axis=AX.X, op=OP.add)
            g1, mg1 = _ln_finalize(nc, sc, s1, s2, epst, T)
            # warm up the PE so HAM is at 8/8 when real matmuls start
            for wd in range(15):
                pw = psT.tile([128, 128], F32, tag="tp", name="pw")
                nc.tensor.matmul(out=pw, lhsT=ident, rhs=ident,
                                 start=True, stop=True)
            a1tm = big.tile([P, T, C], BF16, tag="scr2")
            for q8 in range(8):
                sl = slice(16 * q8, 16 * (q8 + 1))
                nc.vector.tensor_tensor(
                    out=a1tm[:, sl, :], in0=x_tm[:, sl, :],
                    in1=g1[:, sl, None].broadcast_to([P, 16, C]), op=OP.mult)
                nc.vector.tensor_tensor(
                    out=a1tm[:, sl, :], in0=a1tm[:, sl, :],
                    in1=mg1[:, sl, None].broadcast_to([P, 16, C]),
                    op=OP.subtract)

            # transpose A1 to channel-major [64, N]: two tiles per transpose
            a1cm = big.tile([C, N], BF16, tag="acm")
            a1cm_v = a1cm.rearrange("c (j a b n) -> c j a b n", a=4, b=2, n=128)
            a1tm_v = a1tm.rearrange("p t c -> p (t c)")
            for j in range(16):
                pt = psT.tile([128, 4, 128], BF16, tag="tp")
                for k in range(4):
                    tt = 8 * j + 2 * k
                    nc.tensor.transpose(out=pt[:, k, :],
                                        in_=a1tm_v[:, 64 * tt:64 * (tt + 2)],
                                        identity=ident)
                nc.scalar.copy(out=a1cm_v[:, j, :, 0, :], in_=pt[0:C, :, :])
                nc.vector.tensor_copy(out=a1cm_v[:, j, :, 1, :],
                                      in_=pt[C:128, :, :])

            # ---- Q^T = wq @ A1 ----
            qt = big.tile([C, N], BF16, tag="qt")
            for i in range(32):
                ps = psA.tile([128, 512], F32, tag="ps", name="ps")[0:C, :]
                nc.tensor.matmul(out=ps, lhsT=wq,
                                 rhs=a1cm[:, 512 * i:512 * (i + 1)],
                                 start=True, stop=True)
                nc.scalar.activation(out=qt[:, 512 * i:512 * (i + 1)], in_=ps,
                                     func=AF.Identity, bias=bq, scale=1.0)

            # ---- spatial reduction conv (8x8 stride 8) ----
            a1sr = a1cm.rearrange("c (Y ky X kx) -> c ky kx Y X", ky=SR, kx=SR, X=16)
            psr = psA.tile([128, 512], F32, tag="ps", name="ps").rearrange("c (a y x) -> c a y x", a=2, y=16)[0:C, 0, :, :]
            for kk in range(64):
                ky, kx = kk // 8, kk % 8
                nc.tensor.matmul(out=psr, lhsT=wsr[:, kk, :],
                                 rhs=a1sr[:, ky, kx, :, :],
                                 start=(kk == 0), stop=(kk == 63))
            xrcm = consts.tile([C, NR], BF16)
            nc.scalar.activation(out=xrcm.rearrange("c (y x) -> c y x", x=16),
                                 in_=psr, func=AF.Identity,
                                 bias=bsr, scale=1.0)

            # ---- LN on reduced tokens (srn), token-major ----
            xr_tm = consts.tile([P, 2, C], F32)
            for hh in range(2):
                pv = psT.tile([128, C], BF16, tag="tp")
                nc.tensor.transpose(out=pv, in_=xrcm[:, 128 * hh:128 * (hh + 1)],
                                    identity=ident[0:C, 0:C])
                nc.vector.tensor_copy(out=xr_tm[:, hh, :], in_=pv)
            g_r, mg_r = _ln_stats(nc, sc, consts, xr_tm, epst, 2)
            ar_tm = consts.tile([P, 2, C], BF16)
            nc.vector.tensor_tensor(
                out=ar_tm, in0=xr_tm,
                in1=g_r[:, :, None].broadcast_to([P, 2, C]), op=OP.mult)
            mgb = sc.tile([P, 2, C], BF16, tag="mgb")
            nc.vector.tensor_tensor(
                out=mgb, in0=mg_r[:, :, None].broadcast_to([P, 2, C]),
                in1=g_r[:, :, None].broadcast_to([P, 2, C]), op=OP.bypass)
            nc.vector.tensor_tensor(out=ar_tm, in0=ar_tm, in1=mgb, op=OP.subtract)
            arcm = consts.tile([C, NR], BF16)
            for hh in range(2):
                pv = psT.tile([C, 128], BF16, tag="tp")
                nc.tensor.transpose(out=pv, in_=ar_tm[:, hh, :], identity=ident)
                nc.vector.tensor_copy(out=arcm[:, 128 * hh:128 * (hh + 1)], in_=pv)

            # ---- KV ----
            pkv = psA.tile([128, 512], F32, tag="ps", name="ps")[:, 0:NR]
            nc.tensor.matmul(out=pkv, lhsT=wkv, rhs=arcm, start=True, stop=True)
            kvcm = consts.tile([2 * C, NR], BF16)
            nc.scalar.activation(out=kvcm, in_=pkv, func=AF.Identity,
                                 bias=bkv, scale=1.0)
            vp = consts.tile([128, 2, C + 1], BF16)
            nc.vector.memset(vp[:, :, C:C + 1], 1.0)
            for hh in range(2):
                pv = psT.tile([128, C], BF16, tag="tp")
                nc.tensor.transpose(out=pv,
                                    in_=kvcm[C:2 * C, 128 * hh:128 * (hh + 1)],
                                    identity=ident[C:2 * C, C:2 * C])
                nc.vector.tensor_copy(out=vp[:, hh, 0:C], in_=pv)

            # ---- attention, streamed in 512-column chunks ----
            y_tm = big.tile([P, T, C], F32, tag="y")
            sq2 = big.tile([P, T * C], BF16, tag="scr2", name="sq2")
            sq2v = sq2.rearrange("p (t c) -> p t c", c=C)
            s1y = sc.tile([P, T], F32, tag="s1y")
            s2y = sc.tile([P, T], F32, tag="s2y")
            for i in range(32):
                ech = ch.tile([128, 2, 512], BF16, tag="e")
                for hh in range(2):
                    pS = psA.tile([128, 512], F32, tag="ps", name="ps")
                    nc.tensor.matmul(out=pS,
                                     lhsT=kvcm[0:C, 128 * hh:128 * (hh + 1)],
                                     rhs=qt[:, 512 * i:512 * (i + 1)],
                                     start=True, stop=True)
                    nc.scalar.activation(out=ech[:, hh, :], in_=pS, func=AF.Exp)
                pO = psA.tile([128, 512], F32, tag="ps", name="ps")[0:C + 1, :]
                for hh in range(2):
                    nc.tensor.matmul(out=pO, lhsT=vp[:, hh, :],
                                     rhs=ech[:, hh, :],
                                     start=(hh == 0), stop=(hh == 1))
                pod = ch.tile([C + 1, 512], BF16, tag="pod")
                nc.vector.tensor_copy(out=pod, in_=pO)
                ptr = psT.tile([128, 4, C + 1], F32, tag="tp")
                for k in range(4):
                    nc.tensor.matmul(out=ptr[:, k, :],
                                     lhsT=pod[:, 128 * k:128 * (k + 1)],
                                     rhs=wpj, start=True, stop=True)
                rt = sc.tile([P, 4, 1], F32, tag="rt")
                nc.vector.reciprocal(out=rt, in_=ptr[:, :, C:C + 1])
                tmp = ch.tile([P, 4, C], F32, tag="tmp")
                nc.vector.tensor_tensor(out=tmp, in0=ptr[:, :, 0:C],
                                        in1=rt.broadcast_to([P, 4, C]),
                                        op=OP.mult)
                nc.vector.tensor_tensor(out=y_tm[:, 4 * i:4 * (i + 1), :],
                                        in0=tmp, in1=x_tm[:, 4 * i:4 * (i + 1), :],
                                        op=OP.add)
                if i % 4 == 3:
                    sl = slice(16 * (i // 4), 16 * (i // 4 + 1))
                    nc.scalar.square(out=sq2v[:, sl, :], in_=y_tm[:, sl, :])
                    nc.vector.tensor_reduce(out=s1y[:, sl], in_=y_tm[:, sl, :],
                                            axis=AX.X, op=OP.add)
                    nc.vector.tensor_reduce(out=s2y[:, sl], in_=sq2v[:, sl, :],
                                            axis=AX.X, op=OP.add)

            # ---- LN2 ----
            g2, mg2 = _ln_finalize(nc, sc, s1y, s2y, epst, T)
            a2tm = big.tile([P, T, C], BF16, tag="scr2")
            for q8 in range(8):
                sl = slice(16 * q8, 16 * (q8 + 1))
                nc.vector.tensor_tensor(
                    out=a2tm[:, sl, :], in0=y_tm[:, sl, :],
                    in1=g2[:, sl, None].broadcast_to([P, 16, C]), op=OP.mult)
                nc.vector.tensor_tensor(
                    out=a2tm[:, sl, :], in0=a2tm[:, sl, :],
                    in1=mg2[:, sl, None].broadcast_to([P, 16, C]),
                    op=OP.subtract)
            # re-warm PE after the LN2 lull
            for wd in range(8):
                pw = psT.tile([128, 128], F32, tag="tp", name="pw")
                nc.tensor.matmul(out=pw, lhsT=ident, rhs=ident,
                                 start=True, stop=True)

            # ---- A2 guarded channel-major, doubled: rows 64:128 shifted by +1 ----
            a2g = big.tile([128, NG], BF16, tag="acm")
            nc.vector.memset(a2g[:, 0:PAD + RP], 0.0)
            nc.vector.memset(a2g[:, NG - PAD - RP:NG], 0.0)
            a2rows = a2g[0:C, PAD + RP:PAD + RP * (H + 1)].rearrange(
                "c (y w) -> c y w", w=RP)
            a2rowsB = a2g[C:128, PAD + RP:PAD + RP * (H + 1)].rearrange(
                "c (y w) -> c y w", w=RP)
            nc.vector.memset(a2rows[:, :, 0:1], 0.0)
            nc.vector.memset(a2rows[:, :, RP - 1:RP], 0.0)
            nc.vector.memset(a2rowsB[:, :, RP - 2:RP], 0.0)
            a2tm_v = a2tm.rearrange("p t c -> p (t c)")
            for j in range(16):
                pt = psT.tile([128, 4, 128], BF16, tag="tp")
                for k in range(4):
                    tt = 8 * j + 2 * k
                    nc.tensor.transpose(out=pt[:, k, :],
                                        in_=a2tm_v[:, 64 * tt:64 * (tt + 2)],
                                        identity=ident)
                ro = a2rows.rearrange("c (j a b) w -> c j a b w", a=4, b=2)
                nc.scalar.copy(out=ro[:, j, :, 0, 1:W + 1], in_=pt[0:C, :, :])
                nc.vector.tensor_copy(out=ro[:, j, :, 1, 1:W + 1],
                                      in_=pt[C:128, :, :])
                nc.vector.tensor_copy(out=a2rowsB[:, 8 * j:8 * (j + 1), 0:W],
                                      in_=a2rows[:, 8 * j:8 * (j + 1), 1:W + 1])

            # ---- MLP: fused fc1 (+) 3x3 depthwise conv, gelu, fc2 ----
            o2cm = big.tile([C, NG], BF16, tag="qt")  # reuses qt slot
            n_mlp = 33
            for j in range(n_mlp):
                cb = PAD + RP + 512 * j
                size = min(512, PAD + RP * (H + 1) - cb)
                gch = []
                for g in range(2):
                    pG = psA.tile([128, 512], F32, tag="ps", name="ps")
                    for dy in (-1, 0, 1):
                        nc.tensor.matmul(
                            out=pG[:, 0:size], lhsT=wmp[:, 2 * (dy + 1) + g, :],
                            rhs=a2g[:, cb + RP * dy - 1:cb + RP * dy - 1 + size],
                            start=(dy == -1), stop=False)
                    for dy in (-1, 0, 1):
                        nc.tensor.matmul(
                            out=pG[:, 0:size], lhsT=wms[:, 2 * (dy + 1) + g, :],
                            rhs=a2g[0:C, cb + RP * dy + 1:cb + RP * dy + 1 + size],
                            start=False, stop=(dy == 1))
                    gc = ch.tile([128, 512], BF16, tag=f"gc{g}")
                    nc.scalar.activation(out=gc[:, 0:size], in_=pG[:, 0:size],
                                         func=AF.Gelu, bias=bg[:, g:g + 1],
                                         scale=1.0)
                    gch.append(gc)
                pF = psA.tile([128, 512], F32, tag="ps", name="ps")
                for g in range(2):
                    nc.tensor.matmul(out=pF[0:C, 0:size], lhsT=wf2[:, g, :],
                                     rhs=gch[g][:, 0:size],
                                     start=(g == 0), stop=(g == 1))
                nc.scalar.activation(out=o2cm[:, cb:cb + size],
                                     in_=pF[0:C, 0:size], func=AF.Identity,
                                     bias=bf2, scale=1.0)

            # ---- MLP epilogue: transpose back, residual, store ----
            y2_tm = big.tile([P, T, C], F32, tag="xr")  # reuses x_tm slot
            out_v = out_d.rearrange("(t p) c -> p t c", p=P)
            for j in range(32):
                pt2 = psT.tile([128, 4, C], BF16, tag="tp")
                for k in range(4):
                    t = 4 * j + k
                    s = PAD + RP * (t + 1) + 1
                    nc.tensor.transpose(out=pt2[:, k, :],
                                        in_=o2cm[:, s:s + W],
                                        identity=ident[0:C, 0:C])
                nc.vector.tensor_tensor(out=y2_tm[:, 4 * j:4 * (j + 1), :],
                                        in0=pt2, in1=y_tm[:, 4 * j:4 * (j + 1), :],
                                        op=OP.add)
                if j % 4 == 3:
                    q8 = j // 4
                    nc.gpsimd.dma_start(out=out_v[:, 16 * q8:16 * (q8 + 1), :],
                                        in_=y2_tm[:, 16 * q8:16 * (q8 + 1), :])

    _split_excess_waits(nc)
    return nc


@functools.cache
def _get_nc():
    return _build_nc()


def _prep_weights(inp):
    f = lambda v: np.asarray(v, np.float32)
    n1w, n1b = f(inp["n1_w"]), f(inp["n1_b"])
    q_w, q_b = f(inp["q_w"]), f(inp["q_b"])
    kv_w, kv_b = f(inp["kv_w"]), f(inp["kv_b"])
    sr_w, sr_b = f(inp["sr_w"]), f(inp["sr_b"])
    srnw, srnb = f(inp["srn_w"]), f(inp["srn_b"])
    pj_w, pj_b = f(inp["proj_w"]), f(inp["proj_b"])
    n2w, n2b = f(inp["n2_w"]), f(inp["n2_b"])
    f1w, f1b = f(inp["fc1_w"]), f(inp["fc1_b"])
    dww, dwb = f(inp["dw_w"]), f(inp["dw_b"])
    f2w, f2b = f(inp["fc2_w"]), f(inp["fc2_b"])

    scale = (C // 1) ** -0.5
    wq_l = (q_w * n1w[None, :]).T * scale
    bq_l = ((q_w @ n1b + q_b) * scale)[:, None]

    wsr_l = np.zeros((64, C, C), np.float32)
    for kk in range(64):
        ky, kx = kk // 8, kk % 8
        wsr_l[kk, :, :] = (sr_w[:, :, ky, kx] * n1w[None, :]).T
    wsr_l = wsr_l.transpose(1, 0, 2)
    bsr_l = (sr_w.sum((2, 3)) @ n1b + sr_b)[:, None]

    wkv_l = (kv_w * srnw[None, :]).T
    bkv_l = (kv_w @ srnb + kv_b)[:, None]

    wpj_l = np.zeros((C + 1, C + 1), np.float32)
    wpj_l[:C, :C] = pj_w.T
    wpj_l[C, :C] = pj_b
    wpj_l[C, C] = 1.0

    k9 = dww[:, 0, :, :].reshape(HID, 9)          # [256, 9]
    wmp_l = np.zeros((6, 128, 128), np.float32)
    wms_l = np.zeros((6, C, 128), np.float32)
    for dy in range(3):
        for g in range(2):
            Ma = (k9[:, dy * 3 + 0][:, None] * f1w * n2w[None, :])[128 * g:128 * (g + 1)]
            Mb = (k9[:, dy * 3 + 1][:, None] * f1w * n2w[None, :])[128 * g:128 * (g + 1)]
            Mc = (k9[:, dy * 3 + 2][:, None] * f1w * n2w[None, :])[128 * g:128 * (g + 1)]
            wmp_l[2 * dy + g, :C, :] = Ma.T
            wmp_l[2 * dy + g, C:, :] = Mb.T
            wms_l[2 * dy + g, :, :] = Mc.T
    wmp_l = wmp_l.transpose(1, 0, 2)
    wms_l = wms_l.transpose(1, 0, 2)
    bg_full = k9.sum(1) * (f1w @ n2b + f1b) + dwb  # [256]
    bg_l = np.ascontiguousarray(bg_full.reshape(2, 128).T)

    wf2_l = np.stack([f2w[:, :128].T, f2w[:, 128:].T], 0).transpose(1, 0, 2)
    bf2_l = f2b[:, None]

    bfc = lambda a: np.ascontiguousarray(a).astype(BF)
    return {
        "wq": bfc(wq_l), "bq": np.ascontiguousarray(bq_l),
        "wsr": bfc(wsr_l), "bsr": np.ascontiguousarray(bsr_l),
        "wkv": bfc(wkv_l), "bkv": np.ascontiguousarray(bkv_l),
        "wpj": bfc(wpj_l),
        "wmp": bfc(wmp_l), "wms": bfc(wms_l),
        "bg": np.ascontiguousarray(bg_l),
        "wf2": bfc(wf2_l), "bf2": np.ascontiguousarray(bf2_l),
    }


def kernel(trace=False, tmpdir=None, **inputs):
    nc = _get_nc()
    x = np.asarray(inputs["x"], np.float32)
    wts = _prep_weights(inputs)
    in_maps = [dict(wts, x=np.ascontiguousarray(x[b])) for b in range(B)]
    res = run_bass_kernel_spmd(nc, in_maps, core_ids=list(range(8)),
                               trace=trace, tmpdir=tmpdir)
    out = np.stack([res.results[b]["out"] for b in range(B)], 0)
    kernel.last_exec_time_ns = res.exec_time_ns
    return out


# revision 22
# speedup vs baseline: 1.2841x; 1.1317x over previous
"""Trainium2 Bass kernel for nn_Block_523986010339 (PVT-style transformer block).

Sharding: data-parallel over batch B=8 -> one batch element per NeuronCore.
Per-core layouts:
  - residual stream token-major fp32 [128p=token%128, 128t=token//128, 64c]
  - matmul operands channel-major bf16 [c, n], n = 128*y + x
  - LN mean folded into matmul weights via an extra "m*g" row; rsqrt scale
    applied token-major with broadcast APs
  - attention: S^T channel-major, exp without max-subtraction (tiny logits),
    denominator via fused ones-column in the V matmul, divided out after proj
  - MLP: fc1 and 3x3 depthwise conv fused into 9 accumulated matmuls over a
    zero-guarded channel-major layout (row pitch 130)
"""

import functools
import json

import numpy as np
import ml_dtypes

import concourse.bass as bass
import concourse.mybir as mybir
import concourse.tile as tile
from concourse.bass_utils import run_bass_kernel_spmd
from concourse.masks import make_identity

F32 = mybir.dt.float32
BF16 = mybir.dt.bfloat16
BF = ml_dtypes.bfloat16

B, N, C, H, W = 8, 16384, 64, 128, 128
SR, HID, NR = 8, 256, 256
P, T = 128, 128
RP = W + 2          # guarded row pitch
PAD = RP + 1        # head/tail pad so all tap offsets stay in-bounds
NG = PAD + RP * (H + 2) + PAD
AX = mybir.AxisListType
OP = mybir.AluOpType
AF = mybir.ActivationFunctionType


def _split_excess_waits(nc, max_waits=1):
    """walrus in this container rejects >1 sync wait per instruction; move
    excess waits onto injected Drain instructions just before the owner."""
    d = json.loads(mybir.module_to_json_string(nc.m))
    n_split = [0]

    def fix(insts):
        out = []
        for inst in insts:
            si = inst.get("sync_info") or {}
            waits = si.get("on_wait") or []
            if len(waits) > max_waits:
                extra = waits[:-max_waits]
                for i in range(0, len(extra), max_waits):
                    n_split[0] += 1
                    out.append({
                        "name": f"WSPLIT-{n_split[0]}",
                        "opcode": "NoOp",
                        "engine": inst["engine"],
                        "ins": [],
                        "outs": [],
                        "is_reset_sema": False,
                        "sync_info": {"on_update": [],
                                      "on_wait": extra[i:i + max_waits]},
                    })
                si["on_wait"] = waits[-max_waits:]
                inst["sync_info"] = si
            out.append(inst)
        return out

    for f in d.get("functions", []):
        for bb in f.get("blocks", []):
            bb["instructions"] = fix(bb["instructions"])
    nc.m = mybir.module_from_json_string(json.dumps(d))


def _ln_stats(nc, sc, big, x_tm, epst, nt):
    """Token-major LN stats: returns (g, mg) tiles [128, nt] fp32 given
    x_tm [128, nt, 64] fp32."""
    sq_scr = big.tile([P, nt * C], BF16, tag="scr2", name="sq")
    xsq_view = sq_scr.rearrange("p (t c) -> p t c", c=C)
    nc.scalar.square(out=sq_scr, in_=x_tm.rearrange("p t c -> p (t c)"))
    s1 = sc.tile([P, nt], F32, tag=f"s1_{nt}")
    s2 = sc.tile([P, nt], F32, tag=f"s2_{nt}")
    nc.vector.tensor_reduce(out=s1, in_=x_tm, axis=AX.X, op=OP.add)
    nc.vector.tensor_reduce(out=s2, in_=xsq_view, axis=AX.X, op=OP.add)
    return _ln_finalize(nc, sc, s1, s2, epst, nt)


def _ln_finalize(nc, sc, s1, s2, epst, nt):
    mean = sc.tile([P, nt], F32, tag=f"mean_{nt}")
    var = sc.tile([P, nt], F32, tag=f"var_{nt}")
    nc.vector.tensor_scalar_mul(out=mean, in0=s1, scalar1=1.0 / C)
    nc.vector.tensor_scalar_mul(out=var, in0=s2, scalar1=1.0 / C)
    mm = sc.tile([P, nt], F32, tag=f"mm_{nt}")
    nc.vector.tensor_tensor(out=mm, in0=mean, in1=mean, op=OP.mult)
    nc.vector.tensor_tensor(out=var, in0=var, in1=mm, op=OP.subtract)
    sd = sc.tile([P, nt], F32, tag=f"sd_{nt}")
    nc.scalar.activation(out=sd, in_=var, func=AF.Sqrt, bias=epst, scale=1.0)
    g = sc.tile([P, nt], F32, tag=f"g_{nt}")
    nc.vector.reciprocal(out=g, in_=sd)
    mg = sc.tile([P, nt], F32, tag=f"mg_{nt}")
    nc.vector.tensor_tensor(out=mg, in0=mean, in1=g, op=OP.mult)
    return g, mg


def _build_nc():
    nc = bass.Bass("TRN2")
    x_d = nc.dram_tensor("x", [N, C], F32, kind="ExternalInput")
    out_d = nc.dram_tensor("out", [N, C], F32, kind="ExternalOutput")
    wq_d = nc.dram_tensor("wq", [C, C], BF16, kind="ExternalInput")
    bq_d = nc.dram_tensor("bq", [C, 1], F32, kind="ExternalInput")
    wsr_d = nc.dram_tensor("wsr", [C, 64, C], BF16, kind="ExternalInput")
    bsr_d = nc.dram_tensor("bsr", [C, 1], F32, kind="ExternalInput")
    wkv_d = nc.dram_tensor("wkv", [C, 2 * C], BF16, kind="ExternalInput")
    bkv_d = nc.dram_tensor("bkv", [2 * C, 1], F32, kind="ExternalInput")
    wpj_d = nc.dram_tensor("wpj", [C + 1, C + 1], BF16, kind="ExternalInput")
    wmp_d = nc.dram_tensor("wmp", [128, 6, 128], BF16, kind="ExternalInput")
    wms_d = nc.dram_tensor("wms", [C, 6, 128], BF16, kind="ExternalInput")
    bg_d = nc.dram_tensor("bg", [128, 2], F32, kind="ExternalInput")
    wf2_d = nc.dram_tensor("wf2", [128, 2, C], BF16, kind="ExternalInput")
    bf2_d = nc.dram_tensor("bf2", [C, 1], F32, kind="ExternalInput")

    with tile.TileContext(nc) as tc:
        with (
            tc.tile_pool(name="consts", bufs=1) as consts,
            tc.tile_pool(name="big", bufs=1) as big,
            tc.tile_pool(name="sc", bufs=2) as sc,
            tc.tile_pool(name="ch", bufs=3) as ch,
            tc.tile_pool(name="psA", bufs=6, space="PSUM") as psA,
            tc.tile_pool(name="psT", bufs=2, space="PSUM") as psT,
        ):
            ident = consts.tile([128, 128], BF16)
            make_identity(nc, ident)
            wq = consts.tile([C, C], BF16)
            nc.gpsimd.dma_start(out=wq, in_=wq_d[:, :])
            wsr = consts.tile([C, 64, C], BF16)
            nc.gpsimd.dma_start(out=wsr, in_=wsr_d[:, :, :])
            wkv = consts.tile([C, 2 * C], BF16)
            nc.gpsimd.dma_start(out=wkv, in_=wkv_d[:, :])
            wpj = consts.tile([C + 1, C + 1], BF16)
            nc.gpsimd.dma_start(out=wpj, in_=wpj_d[:, :])
            wmp = consts.tile([128, 6, 128], BF16)
            nc.gpsimd.dma_start(out=wmp, in_=wmp_d[:, :, :])
            wms = consts.tile([C, 6, 128], BF16)
            nc.gpsimd.dma_start(out=wms, in_=wms_d[:, :, :])
            wf2 = consts.tile([128, 2, C], BF16)
            nc.gpsimd.dma_start(out=wf2, in_=wf2_d[:, :, :])
            bq = consts.tile([C, 1], F32)
            nc.gpsimd.dma_start(out=bq, in_=bq_d[:, :])
            bsr = consts.tile([C, 1], F32)
            nc.gpsimd.dma_start(out=bsr, in_=bsr_d[:, :])
            bkv = consts.tile([2 * C, 1], F32)
            nc.gpsimd.dma_start(out=bkv, in_=bkv_d[:, :])
            bg = consts.tile([128, 2], F32)
            nc.gpsimd.dma_start(out=bg, in_=bg_d[:, :])
            bf2 = consts.tile([C, 1], F32)
            nc.gpsimd.dma_start(out=bf2, in_=bf2_d[:, :])
            epst = consts.tile([P, 1], F32)
            nc.vector.memset(epst, 1e-5)

            # ---- load x (token-major), LN1 stats overlapped per slice ----
            x_tm = big.tile([P, T, C], F32, tag="xr")
            x_v = x_d.rearrange("(t p) c -> p t c", p=P)
            sq_scr = big.tile([P, T * C], BF16, tag="scr2", name="sq")
            sqv = sq_scr.rearrange("p (t c) -> p t c", c=C)
            s1 = sc.tile([P, T], F32, tag="s1")
            s2 = sc.tile([P, T], F32, tag="s2")
            for q8 in range(8):
                sl = slice(16 * q8, 16 * (q8 + 1))
                eng = nc.sync if q8 % 2 == 0 else nc.scalar
                eng.dma_start(out=x_tm[:, sl, :], in_=x_v[:, sl, :])
                nc.scalar.square(out=sqv[:, sl, :], in_=x_tm[:, sl, :])
                nc.vector.tensor_reduce(out=s1[:, sl], in_=x_tm[:, sl, :],
                                        axis=AX.X, op=OP.add)
                nc.vector.tensor_reduce(out=s2[:, sl], in_=sqv[:, sl, :],
                                        axis=AX.X, op=OP.add)
            g1, mg1 = _ln_finalize(nc, sc, s1, s2, epst, T)
            # warm up the PE so HAM is at 8/8 when real matmuls start
            for wd in range(15):
                pw = psT.tile([128, 128], F32, tag="tp", name="pw")
                nc.tensor.matmul(out=pw, lhsT=ident, rhs=ident,
                                 start=True, stop=True)
            a1tm = big.tile([P, T, C], BF16, tag="scr2")
            for q8 in range(8):
                sl = slice(16 * q8, 16 * (q8 + 1))
                nc.vector.tensor_tensor(
                    out=a1tm[:, sl, :], in0=x_tm[:, sl, :],
                    in1=g1[:, sl, None].broadcast_to([P, 16, C]), op=OP.mult)
                nc.vector.tensor_tensor(
                    out=a1tm[:, sl, :], in0=a1tm[:, sl, :],
                    in1=mg1[:, sl, None].broadcast_to([P, 16, C]),
                    op=OP.subtract)

            # transpose A1 to channel-major [64, N]: two tiles per transpose
            a1cm = big.tile([C, N], BF16, tag="acm")
            a1cm_v = a1cm.rearrange("c (j a b n) -> c j a b n", a=4, b=2, n=128)
            a1tm_v = a1tm.rearrange("p t c -> p (t c)")
            for j in range(16):
                pt = psT.tile([128, 4, 128], BF16, tag="tp")
                for k in range(4):
                    tt = 8 * j + 2 * k
                    nc.tensor.transpose(out=pt[:, k, :],
                                        in_=a1tm_v[:, 64 * tt:64 * (tt + 2)],
                                        identity=ident)
                nc.scalar.copy(out=a1cm_v[:, j, :, 0, :], in_=pt[0:C, :, :])
                nc.vector.tensor_copy(out=a1cm_v[:, j, :, 1, :],
                                      in_=pt[C:128, :, :])

            # ---- Q^T = wq @ A1 ----
            qt = big.tile([C, N], BF16, tag="qt")
            for i in range(32):
                ps = psA.tile([128, 512], F32, tag="ps", name="ps")[0:C, :]
                nc.tensor.matmul(out=ps, lhsT=wq,
                                 rhs=a1cm[:, 512 * i:512 * (i + 1)],
                                 start=True, stop=True)
                nc.scalar.activation(out=qt[:, 512 * i:512 * (i + 1)], in_=ps,
                                     func=AF.Identity, bias=bq, scale=1.0)

            # ---- spatial reduction conv (8x8 stride 8) ----
            a1sr = a1cm.rearrange("c (Y ky X kx) -> c ky kx Y X", ky=SR, kx=SR, X=16)
            psr = psA.tile([128, 512], F32, tag="ps", name="ps").rearrange("c (a y x) -> c a y x", a=2, y=16)[0:C, 0, :, :]
            for kk in range(64):
                ky, kx = kk // 8, kk % 8
                nc.tensor.matmul(out=psr, lhsT=wsr[:, kk, :],
                                 rhs=a1sr[:, ky, kx, :, :],
                                 start=(kk == 0), stop=(kk == 63))
            xrcm = consts.tile([C, NR], BF16)
            nc.scalar.activation(out=xrcm.rearrange("c (y x) -> c y x", x=16),
                                 in_=psr, func=AF.Identity,
                                 bias=bsr, scale=1.0)

            # ---- LN on reduced tokens (srn), token-major ----
            xr_tm = consts.tile([P, 2, C], F32)
            for hh in range(2):
                pv = psT.tile([128, C], BF16, tag="tp")
                nc.tensor.transpose(out=pv, in_=xrcm[:, 128 * hh:128 * (hh + 1)],
                                    identity=ident[0:C, 0:C])
                nc.vector.tensor_copy(out=xr_tm[:, hh, :], in_=pv)
            g_r, mg_r = _ln_stats(nc, sc, consts, xr_tm, epst, 2)
            ar_tm = consts.tile([P, 2, C], BF16)
            nc.vector.tensor_tensor(
                out=ar_tm, in0=xr_tm,
                in1=g_r[:, :, None].broadcast_to([P, 2, C]), op=OP.mult)
            mgb = sc.tile([P, 2, C], BF16, tag="mgb")
            nc.vector.tensor_tensor(
                out=mgb, in0=mg_r[:, :, None].broadcast_to([P, 2, C]),
                in1=g_r[:, :, None].broadcast_to([P, 2, C]), op=OP.bypass)
            nc.vector.tensor_tensor(out=ar_tm, in0=ar_tm, in1=mgb, op=OP.subtract)
            arcm = consts.tile([C, NR], BF16)
            for hh in range(2):
                pv = psT.tile([C, 128], BF16, tag="tp")
                nc.tensor.transpose(out=pv, in_=ar_tm[:, hh, :], identity=ident)
                nc.vector.tensor_copy(out=arcm[:, 128 * hh:128 * (hh + 1)], in_=pv)

            # ---- KV ----
            pkv = psA.tile([128, 512], F32, tag="ps", name="ps")[:, 0:NR]
            nc.tensor.matmul(out=pkv, lhsT=wkv, rhs=arcm, start=True, stop=True)
            kvcm = consts.tile([2 * C, NR], BF16)
            nc.scalar.activation(out=kvcm, in_=pkv, func=AF.Identity,
                                 bias=bkv, scale=1.0)
            vp = consts.tile([128, 2, C + 1], BF16)
            nc.vector.memset(vp[:, :, C:C + 1], 1.0)
            for hh in range(2):
                pv = psT.tile([128, C], BF16, tag="tp")
                nc.tensor.transpose(out=pv,
                                    in_=kvcm[C:2 * C, 128 * hh:128 * (hh + 1)],
                                    identity=ident[C:2 * C, C:2 * C])
                nc.vector.tensor_copy(out=vp[:, hh, 0:C], in_=pv)

            # ---- attention, streamed in 512-column chunks ----
            y_tm = big.tile([P, T, C], F32, tag="y")
            sq2 = big.tile([P, T * C], BF16, tag="scr2", name="sq2")
            sq2v = sq2.rearrange("p (t c) -> p t c", c=C)
            s1y = sc.tile([P, T], F32, tag="s1y")
            s2y = sc.tile([P, T], F32, tag="s2y")
            for i in range(32):
                ech = ch.tile([128, 2, 512], BF16, tag="e")
                for hh in range(2):
                    pS = psA.tile([128, 512], F32, tag="ps", name="ps")
                    nc.tensor.matmul(out=pS,
                                     lhsT=kvcm[0:C, 128 * hh:128 * (hh + 1)],
                                     rhs=qt[:, 512 * i:512 * (i + 1)],
                                     start=True, stop=True)
                    nc.scalar.activation(out=ech[:, hh, :], in_=pS, func=AF.Exp)
                pO = psA.tile([128, 512], F32, tag="ps", name="ps")[0:C + 1, :]
                for hh in range(2):
                    nc.tensor.matmul(out=pO, lhsT=vp[:, hh, :],
                                     rhs=ech[:, hh, :],
                                     start=(hh == 0), stop=(hh == 1))
                pod = ch.tile([C + 1, 512], BF16, tag="pod")
                nc.vector.tensor_copy(out=pod, in_=pO)
                ptr = psT.tile([128, 4, C + 1], F32, tag="tp")
                for k in range(4):
                    nc.tensor.matmul(out=ptr[:, k, :],
                                     lhsT=pod[:, 128 * k:128 * (k + 1)],
                                     rhs=wpj, start=True, stop=True)
                rt = sc.tile([P, 4, 1], F32, tag="rt")
                nc.vector.reciprocal(out=rt, in_=ptr[:, :, C:C + 1])
                tmp = ch.tile([P, 4, C], F32, tag="tmp")
                nc.vector.tensor_tensor(out=tmp, in0=ptr[:, :, 0:C],
                                        in1=rt.broadcast_to([P, 4, C]),
                                        op=OP.mult)
                nc.vector.tensor_tensor(out=y_tm[:, 4 * i:4 * (i + 1), :],
                                        in0=tmp, in1=x_tm[:, 4 * i:4 * (i + 1), :],
                                        op=OP.add)
                if i % 4 == 3:
                    sl = slice(16 * (i // 4), 16 * (i // 4 + 1))
                    nc.scalar.square(out=sq2v[:, sl, :], in_=y_tm[:, sl, :])
                    nc.vector.tensor_reduce(out=s1y[:, sl], in_=y_tm[:, sl, :],
                                            axis=AX.X, op=OP.add)
                    nc.vector.tensor_reduce(out=s2y[:, sl], in_=sq2v[:, sl, :],
                                            axis=AX.X, op=OP.add)

            # ---- LN2 ----
            g2, mg2 = _ln_finalize(nc, sc, s1y, s2y, epst, T)
            a2tm = big.tile([P, T, C], BF16, tag="scr2")
            for q8 in range(8):
                sl = slice(16 * q8, 16 * (q8 + 1))
                nc.vector.tensor_tensor(
                    out=a2tm[:, sl, :], in0=y_tm[:, sl, :],
                    in1=g2[:, sl, None].broadcast_to([P, 16, C]), op=OP.mult)
                nc.vector.tensor_tensor(
                    out=a2tm[:, sl, :], in0=a2tm[:, sl, :],
                    in1=mg2[:, sl, None].broadcast_to([P, 16, C]),
                    op=OP.subtract)
            # re-warm PE after the LN2 lull
            for wd in range(8):
                pw = psT.tile([128, 128], F32, tag="tp", name="pw")
                nc.tensor.matmul(out=pw, lhsT=ident, rhs=ident,
                                 start=True, stop=True)

            # ---- A2 guarded channel-major, doubled: rows 64:128 shifted by +1 ----
            a2g = big.tile([128, NG], BF16, tag="acm")
            nc.vector.memset(a2g[:, 0:PAD + RP], 0.0)
            nc.vector.memset(a2g[:, NG - PAD - RP:NG], 0.0)
            a2rows = a2g[0:C, PAD + RP:PAD + RP * (H + 1)].rearrange(
                "c (y w) -> c y w", w=RP)
            a2rowsB = a2g[C:128, PAD + RP:PAD + RP * (H + 1)].rearrange(
                "c (y w) -> c y w", w=RP)
            nc.vector.memset(a2rows[:, :, 0:1], 0.0)
            nc.vector.memset(a2rows[:, :, RP - 1:RP], 0.0)
            nc.vector.memset(a2rowsB[:, :, RP - 2:RP], 0.0)
            a2tm_v = a2tm.rearrange("p t c -> p (t c)")
            for j in range(16):
                pt = psT.tile([128, 4, 128], BF16, tag="tp")
                for k in range(4):
                    tt = 8 * j + 2 * k
                    nc.tensor.transpose(out=pt[:, k, :],
                                        in_=a2tm_v[:, 64 * tt:64 * (tt + 2)],
                                        identity=ident)
                ro = a2rows.rearrange("c (j a b) w -> c j a b w", a=4, b=2)
                nc.scalar.copy(out=ro[:, j, :, 0, 1:W + 1], in_=pt[0:C, :, :])
                nc.vector.tensor_copy(out=ro[:, j, :, 1, 1:W + 1],
                                      in_=pt[C:128, :, :])
                nc.vector.tensor_copy(out=a2rowsB[:, 8 * j:8 * (j + 1), 0:W],
                                      in_=a2rows[:, 8 * j:8 * (j + 1), 1:W + 1])

            # ---- MLP: fused fc1 (+) 3x3 depthwise conv, gelu, fc2 ----
            o2cm = big.tile([C, NG], BF16, tag="qt")  # reuses qt slot
            n_mlp = 33
            for j in range(n_mlp):
                cb = PAD + RP + 512 * j
                size = min(512, PAD + RP * (H + 1) - cb)
                gch = []
                for g in range(2):
                    pG = psA.tile([128, 512], F32, tag="ps", name="ps")
                    for dy in (-1, 0, 1):
                        nc.tensor.matmul(
                            out=pG[:, 0:size], lhsT=wmp[:, 2 * (dy + 1) + g, :],
                            rhs=a2g[:, cb + RP * dy - 1:cb + RP * dy - 1 + size],
                            start=(dy == -1), stop=False)
                    for dy in (-1, 0, 1):
                        nc.tensor.matmul(
                            out=pG[:, 0:size], lhsT=wms[:, 2 * (dy + 1) + g, :],
                            rhs=a2g[0:C, cb + RP * dy + 1:cb + RP * dy + 1 + size],
                            start=False, stop=(dy == 1))
                    gc = ch.tile([128, 512], BF16, tag=f"gc{g}")
                    nc.scalar.activation(out=gc[:, 0:size], in_=pG[:, 0:size],
                                         func=AF.Gelu, bias=bg[:, g:g + 1],
                                         scale=1.0)
                    gch.append(gc)
                pF = psA.tile([128, 512], F32, tag="ps", name="ps")
                for g in range(2):
                    nc.tensor.matmul(out=pF[0:C, 0:size], lhsT=wf2[:, g, :],
                                     rhs=gch[g][:, 0:size],
                                     start=(g == 0), stop=(g == 1))
                nc.vector.tensor_scalar(out=o2cm[:, cb:cb + size],
                                        in0=pF[0:C, 0:size], scalar1=bf2,
                                        scalar2=None, op0=OP.add)

            # ---- MLP epilogue: transpose back, residual, store ----
            y2_tm = big.tile([P, T, C], F32, tag="xr")  # reuses x_tm slot
            out_v = out_d.rearrange("(t p) c -> p t c", p=P)
            for j in range(32):
                pt2 = psT.tile([128, 4, C], BF16, tag="tp")
                for k in range(4):
                    t = 4 * j + k
                    s = PAD + RP * (t + 1) + 1
                    nc.tensor.transpose(out=pt2[:, k, :],
                                        in_=o2cm[:, s:s + W],
                                        identity=ident[0:C, 0:C])
                nc.vector.tensor_tensor(out=y2_tm[:, 4 * j:4 * (j + 1), :],
                                        in0=pt2, in1=y_tm[:, 4 * j:4 * (j + 1), :],
                                        op=OP.add)
                if j % 4 == 3:
                    q8 = j // 4
                    nc.sync.dma_start(out=out_v[:, 16 * q8:16 * (q8 + 1), :],
                                       in_=y2_tm[:, 16 * q8:16 * (q8 + 1), :])

    _split_excess_waits(nc)
    return nc


@functools.cache
def _get_nc():
    return _build_nc()


def _prep_weights(inp):
    f = lambda v: np.asarray(v, np.float32)
    n1w, n1b = f(inp["n1_w"]), f(inp["n1_b"])
    q_w, q_b = f(inp["q_w"]), f(inp["q_b"])
    kv_w, kv_b = f(inp["kv_w"]), f(inp["kv_b"])
    sr_w, sr_b = f(inp["sr_w"]), f(inp["sr_b"])
    srnw, srnb = f(inp["srn_w"]), f(inp["srn_b"])
    pj_w, pj_b = f(inp["proj_w"]), f(inp["proj_b"])
    n2w, n2b = f(inp["n2_w"]), f(inp["n2_b"])
    f1w, f1b = f(inp["fc1_w"]), f(inp["fc1_b"])
    dww, dwb = f(inp["dw_w"]), f(inp["dw_b"])
    f2w, f2b = f(inp["fc2_w"]), f(inp["fc2_b"])

    scale = (C // 1) ** -0.5
    wq_l = (q_w * n1w[None, :]).T * scale
    bq_l = ((q_w @ n1b + q_b) * scale)[:, None]

    wsr_l = np.zeros((64, C, C), np.float32)
    for kk in range(64):
        ky, kx = kk // 8, kk % 8
        wsr_l[kk, :, :] = (sr_w[:, :, ky, kx] * n1w[None, :]).T
    wsr_l = wsr_l.transpose(1, 0, 2)
    bsr_l = (sr_w.sum((2, 3)) @ n1b + sr_b)[:, None]

    wkv_l = (kv_w * srnw[None, :]).T
    bkv_l = (kv_w @ srnb + kv_b)[:, None]

    wpj_l = np.zeros((C + 1, C + 1), np.float32)
    wpj_l[:C, :C] = pj_w.T
    wpj_l[C, :C] = pj_b
    wpj_l[C, C] = 1.0

    k9 = dww[:, 0, :, :].reshape(HID, 9)          # [256, 9]
    wmp_l = np.zeros((6, 128, 128), np.float32)
    wms_l = np.zeros((6, C, 128), np.float32)
    for dy in range(3):
        for g in range(2):
            Ma = (k9[:, dy * 3 + 0][:, None] * f1w * n2w[None, :])[128 * g:128 * (g + 1)]
            Mb = (k9[:, dy * 3 + 1][:, None] * f1w * n2w[None, :])[128 * g:128 * (g + 1)]
            Mc = (k9[:, dy * 3 + 2][:, None] * f1w * n2w[None, :])[128 * g:128 * (g + 1)]
            wmp_l[2 * dy + g, :C, :] = Ma.T
            wmp_l[2 * dy + g, C:, :] = Mb.T
            wms_l[2 * dy + g, :, :] = Mc.T
    wmp_l = wmp_l.transpose(1, 0, 2)
    wms_l = wms_l.transpose(1, 0, 2)
    bg_full = k9.sum(1) * (f1w @ n2b + f1b) + dwb  # [256]
    bg_l = np.ascontiguousarray(bg_full.reshape(2, 128).T)

    wf2_l = np.stack([f2w[:, :128].T, f2w[:, 128:].T], 0).transpose(1, 0, 2)
    bf2_l = f2b[:, None]

    bfc = lambda a: np.ascontiguousarray(a).astype(BF)
    return {
        "wq": bfc(wq_l), "bq": np.ascontiguousarray(bq_l),
        "wsr": bfc(wsr_l), "bsr": np.ascontiguousarray(bsr_l),
        "wkv": bfc(wkv_l), "bkv": np.ascontiguousarray(bkv_l),
        "wpj": bfc(wpj_l),
        "wmp": bfc(wmp_l), "wms": bfc(wms_l),
        "bg": np.ascontiguousarray(bg_l),
        "wf2": bfc(wf2_l), "bf2": np.ascontiguousarray(bf2_l),
    }


def kernel(trace=False, tmpdir=None, **inputs):
    nc = _get_nc()
    x = np.asarray(inputs["x"], np.float32)
    wts = _prep_weights(inputs)
    in_maps = [dict(wts, x=np.ascontiguousarray(x[b])) for b in range(B)]
    res = run_bass_kernel_spmd(nc, in_maps, core_ids=list(range(8)),
                               trace=trace, tmpdir=tmpdir)
    out = np.stack([res.results[b]["out"] for b in range(B)], 0)
    kernel.last_exec_time_ns = res.exec_time_ns
    return out


# revision 23
# speedup vs baseline: 1.3249x; 1.0318x over previous
"""Trainium2 Bass kernel for nn_Block_523986010339 (PVT-style transformer block).

Sharding: data-parallel over batch B=8 -> one batch element per NeuronCore.
Per-core layouts:
  - residual stream token-major fp32 [128p=token%128, 128t=token//128, 64c]
  - matmul operands channel-major bf16 [c, n], n = 128*y + x
  - LN mean folded into matmul weights via an extra "m*g" row; rsqrt scale
    applied token-major with broadcast APs
  - attention: S^T channel-major, exp without max-subtraction (tiny logits),
    denominator via fused ones-column in the V matmul, divided out after proj
  - MLP: fc1 and 3x3 depthwise conv fused into 9 accumulated matmuls over a
    zero-guarded channel-major layout (row pitch 130)
"""

import functools
import json

import numpy as np
import ml_dtypes

import concourse.bass as bass
import concourse.mybir as mybir
import concourse.tile as tile
from concourse.bass_utils import run_bass_kernel_spmd
from concourse.masks import make_identity

F32 = mybir.dt.float32
BF16 = mybir.dt.bfloat16
BF = ml_dtypes.bfloat16

B, N, C, H, W = 8, 16384, 64, 128, 128
SR, HID, NR = 8, 256, 256
P, T = 128, 128
RP = W + 2          # guarded row pitch
PAD = RP + 1        # head/tail pad so all tap offsets stay in-bounds
NG = PAD + RP * (H + 2) + PAD
AX = mybir.AxisListType
OP = mybir.AluOpType
AF = mybir.ActivationFunctionType


def _split_excess_waits(nc, max_waits=1):
    """walrus in this container rejects >1 sync wait per instruction; move
    excess waits onto injected Drain instructions just before the owner."""
    d = json.loads(mybir.module_to_json_string(nc.m))
    n_split = [0]

    def fix(insts):
        out = []
        for inst in insts:
            si = inst.get("sync_info") or {}
            waits = si.get("on_wait") or []
            if len(waits) > max_waits:
                extra = waits[:-max_waits]
                for i in range(0, len(extra), max_waits):
                    n_split[0] += 1
                    out.append({
                        "name": f"WSPLIT-{n_split[0]}",
                        "opcode": "NoOp",
                        "engine": inst["engine"],
                        "ins": [],
                        "outs": [],
                        "is_reset_sema": False,
                        "sync_info": {"on_update": [],
                                      "on_wait": extra[i:i + max_waits]},
                    })
                si["on_wait"] = waits[-max_waits:]
                inst["sync_info"] = si
            out.append(inst)
        return out

    for f in d.get("functions", []):
        for bb in f.get("blocks", []):
            bb["instructions"] = fix(bb["instructions"])
    nc.m = mybir.module_from_json_string(json.dumps(d))


def _ln_stats(nc, sc, big, x_tm, epst, nt):
    """Token-major LN stats: returns (g, mg) tiles [128, nt] fp32 given
    x_tm [128, nt, 64] fp32."""
    sq_scr = big.tile([P, nt * C], BF16, tag="scr2", name="sq")
    xsq_view = sq_scr.rearrange("p (t c) -> p t c", c=C)
    nc.scalar.square(out=sq_scr, in_=x_tm.rearrange("p t c -> p (t c)"))
    s1 = sc.tile([P, nt], F32, tag=f"s1_{nt}")
    s2 = sc.tile([P, nt], F32, tag=f"s2_{nt}")
    nc.vector.tensor_reduce(out=s1, in_=x_tm, axis=AX.X, op=OP.add)
    nc.vector.tensor_reduce(out=s2, in_=xsq_view, axis=AX.X, op=OP.add)
    return _ln_finalize(nc, sc, s1, s2, epst, nt)


def _ln_finalize(nc, sc, s1, s2, epst, nt):
    mean = sc.tile([P, nt], F32, tag=f"mean_{nt}")
    var = sc.tile([P, nt], F32, tag=f"var_{nt}")
    nc.vector.tensor_scalar_mul(out=mean, in0=s1, scalar1=1.0 / C)
    nc.vector.tensor_scalar_mul(out=var, in0=s2, scalar1=1.0 / C)
    mm = sc.tile([P, nt], F32, tag=f"mm_{nt}")
    nc.vector.tensor_tensor(out=mm, in0=mean, in1=mean, op=OP.mult)
    nc.vector.tensor_tensor(out=var, in0=var, in1=mm, op=OP.subtract)
    sd = sc.tile([P, nt], F32, tag=f"sd_{nt}")
    nc.scalar.activation(out=sd, in_=var, func=AF.Sqrt, bias=epst, scale=1.0)
    g = sc.tile([P, nt], F32, tag=f"g_{nt}")
    nc.vector.reciprocal(out=g, in_=sd)
    mg = sc.tile([P, nt], F32, tag=f"mg_{nt}")
    nc.vector.tensor_tensor(out=mg, in0=mean, in1=g, op=OP.mult)
    return g, mg


def _build_nc():
    nc = bass.Bass("TRN2")
    x_d = nc.dram_tensor("x", [N, C], F32, kind="ExternalInput")
    out_d = nc.dram_tensor("out", [N, C], F32, kind="ExternalOutput")
    wq_d = nc.dram_tensor("wq", [C, C], BF16, kind="ExternalInput")
    bq_d = nc.dram_tensor("bq", [C, 1], F32, kind="ExternalInput")
    wsr_d = nc.dram_tensor("wsr", [C, 64, C], BF16, kind="ExternalInput")
    bsr_d = nc.dram_tensor("bsr", [C, 1], F32, kind="ExternalInput")
    wkv_d = nc.dram_tensor("wkv", [C, 2 * C], BF16, kind="ExternalInput")
    bkv_d = nc.dram_tensor("bkv", [2 * C, 1], F32, kind="ExternalInput")
    wpj_d = nc.dram_tensor("wpj", [C + 1, C + 1], BF16, kind="ExternalInput")
    wmp_d = nc.dram_tensor("wmp", [128, 6, 128], BF16, kind="ExternalInput")
    wms_d = nc.dram_tensor("wms", [C, 6, 128], BF16, kind="ExternalInput")
    bg_d = nc.dram_tensor("bg", [128, 2], F32, kind="ExternalInput")
    wf2_d = nc.dram_tensor("wf2", [128, 2, C], BF16, kind="ExternalInput")
    bf2_d = nc.dram_tensor("bf2", [C, 1], F32, kind="ExternalInput")

    with tile.TileContext(nc) as tc:
        with (
            tc.tile_pool(name="consts", bufs=1) as consts,
            tc.tile_pool(name="big", bufs=1) as big,
            tc.tile_pool(name="sc", bufs=2) as sc,
            tc.tile_pool(name="ch", bufs=3) as ch,
            tc.tile_pool(name="psA", bufs=6, space="PSUM") as psA,
            tc.tile_pool(name="psT", bufs=2, space="PSUM") as psT,
        ):
            ident = consts.tile([128, 128], BF16)
            make_identity(nc, ident)
            wq = consts.tile([C, C], BF16)
            nc.gpsimd.dma_start(out=wq, in_=wq_d[:, :])
            wsr = consts.tile([C, 64, C], BF16)
            nc.gpsimd.dma_start(out=wsr, in_=wsr_d[:, :, :])
            wkv = consts.tile([C, 2 * C], BF16)
            nc.gpsimd.dma_start(out=wkv, in_=wkv_d[:, :])
            wpj = consts.tile([C + 1, C + 1], BF16)
            nc.gpsimd.dma_start(out=wpj, in_=wpj_d[:, :])
            wmp = consts.tile([128, 6, 128], BF16)
            nc.gpsimd.dma_start(out=wmp, in_=wmp_d[:, :, :])
            wms = consts.tile([C, 6, 128], BF16)
            nc.gpsimd.dma_start(out=wms, in_=wms_d[:, :, :])
            wf2 = consts.tile([128, 2, C], BF16)
            nc.gpsimd.dma_start(out=wf2, in_=wf2_d[:, :, :])
            bq = consts.tile([C, 1], F32)
            nc.gpsimd.dma_start(out=bq, in_=bq_d[:, :])
            bsr = consts.tile([C, 1], F32)
            nc.gpsimd.dma_start(out=bsr, in_=bsr_d[:, :])
            bkv = consts.tile([2 * C, 1], F32)
            nc.gpsimd.dma_start(out=bkv, in_=bkv_d[:, :])
            bg = consts.tile([128, 2], F32)
            nc.gpsimd.dma_start(out=bg, in_=bg_d[:, :])
            bf2 = consts.tile([C, 1], F32)
            nc.gpsimd.dma_start(out=bf2, in_=bf2_d[:, :])
            epst = consts.tile([P, 1], F32)
            nc.vector.memset(epst, 1e-5)

            # ---- load x (token-major), LN1 stats overlapped per slice ----
            x_tm = big.tile([P, T, C], F32, tag="xr")
            x_v = x_d.rearrange("(t p) c -> p t c", p=P)
            sq_scr = big.tile([P, T * C], BF16, tag="scr2", name="sq")
            sqv = sq_scr.rearrange("p (t c) -> p t c", c=C)
            s1 = sc.tile([P, T], F32, tag="s1")
            s2 = sc.tile([P, T], F32, tag="s2")
            for q8 in range(8):
                sl = slice(16 * q8, 16 * (q8 + 1))
                eng = nc.sync if q8 % 2 == 0 else nc.scalar
                eng.dma_start(out=x_tm[:, sl, :], in_=x_v[:, sl, :])
                nc.scalar.square(out=sqv[:, sl, :], in_=x_tm[:, sl, :])
                nc.vector.tensor_reduce(out=s1[:, sl], in_=x_tm[:, sl, :],
                                        axis=AX.X, op=OP.add)
                nc.vector.tensor_reduce(out=s2[:, sl], in_=sqv[:, sl, :],
                                        axis=AX.X, op=OP.add)
            g1, mg1 = _ln_finalize(nc, sc, s1, s2, epst, T)
            # warm up the PE so HAM is at 8/8 when real matmuls start
            for wd in range(15):
                pw = psT.tile([128, 128], F32, tag="tp", name="pw")
                nc.tensor.matmul(out=pw, lhsT=ident, rhs=ident,
                                 start=True, stop=True)
            a1tm = big.tile([P, T, C], BF16, tag="scr2")
            for q8 in range(8):
                sl = slice(16 * q8, 16 * (q8 + 1))
                nc.vector.tensor_tensor(
                    out=a1tm[:, sl, :], in0=x_tm[:, sl, :],
                    in1=g1[:, sl, None].broadcast_to([P, 16, C]), op=OP.mult)
                nc.vector.tensor_tensor(
                    out=a1tm[:, sl, :], in0=a1tm[:, sl, :],
                    in1=mg1[:, sl, None].broadcast_to([P, 16, C]),
                    op=OP.subtract)

            # transpose A1 to channel-major [64, N]: two tiles per transpose
            a1cm = big.tile([C, N], BF16, tag="acm")
            a1cm_v = a1cm.rearrange("c (j a b n) -> c j a b n", a=4, b=2, n=128)
            a1tm_v = a1tm.rearrange("p t c -> p (t c)")
            for j in range(16):
                pt = psT.tile([128, 4, 128], BF16, tag="tp")
                for k in range(4):
                    tt = 8 * j + 2 * k
                    nc.tensor.transpose(out=pt[:, k, :],
                                        in_=a1tm_v[:, 64 * tt:64 * (tt + 2)],
                                        identity=ident)
                nc.scalar.copy(out=a1cm_v[:, j, :, 0, :], in_=pt[0:C, :, :])
                nc.vector.tensor_copy(out=a1cm_v[:, j, :, 1, :],
                                      in_=pt[C:128, :, :])

            # ---- spatial reduction conv (8x8 stride 8) ----
            a1sr = a1cm.rearrange("c (Y ky X kx) -> c ky kx Y X", ky=SR, kx=SR, X=16)
            psr = psA.tile([128, 512], F32, tag="ps", name="ps").rearrange("c (a y x) -> c a y x", a=2, y=16)[0:C, 0, :, :]
            for kk in range(64):
                ky, kx = kk // 8, kk % 8
                nc.tensor.matmul(out=psr, lhsT=wsr[:, kk, :],
                                 rhs=a1sr[:, ky, kx, :, :],
                                 start=(kk == 0), stop=(kk == 63))
            xrcm = consts.tile([C, NR], BF16)
            nc.scalar.activation(out=xrcm.rearrange("c (y x) -> c y x", x=16),
                                 in_=psr, func=AF.Identity,
                                 bias=bsr, scale=1.0)

            # ---- LN on reduced tokens (srn), token-major ----
            xr_tm = consts.tile([P, 2, C], F32)
            for hh in range(2):
                pv = psT.tile([128, C], BF16, tag="tp")
                nc.tensor.transpose(out=pv, in_=xrcm[:, 128 * hh:128 * (hh + 1)],
                                    identity=ident[0:C, 0:C])
                nc.vector.tensor_copy(out=xr_tm[:, hh, :], in_=pv)
            g_r, mg_r = _ln_stats(nc, sc, consts, xr_tm, epst, 2)
            ar_tm = consts.tile([P, 2, C], BF16)
            nc.vector.tensor_tensor(
                out=ar_tm, in0=xr_tm,
                in1=g_r[:, :, None].broadcast_to([P, 2, C]), op=OP.mult)
            mgb = sc.tile([P, 2, C], BF16, tag="mgb")
            nc.vector.tensor_tensor(
                out=mgb, in0=mg_r[:, :, None].broadcast_to([P, 2, C]),
                in1=g_r[:, :, None].broadcast_to([P, 2, C]), op=OP.bypass)
            nc.vector.tensor_tensor(out=ar_tm, in0=ar_tm, in1=mgb, op=OP.subtract)
            arcm = consts.tile([C, NR], BF16)
            for hh in range(2):
                pv = psT.tile([C, 128], BF16, tag="tp")
                nc.tensor.transpose(out=pv, in_=ar_tm[:, hh, :], identity=ident)
                nc.vector.tensor_copy(out=arcm[:, 128 * hh:128 * (hh + 1)], in_=pv)

            # ---- KV ----
            pkv = psA.tile([128, 512], F32, tag="ps", name="ps")[:, 0:NR]
            nc.tensor.matmul(out=pkv, lhsT=wkv, rhs=arcm, start=True, stop=True)
            kvcm = consts.tile([2 * C, NR], BF16)
            nc.scalar.activation(out=kvcm, in_=pkv, func=AF.Identity,
                                 bias=bkv, scale=1.0)
            # fold q-projection into K:  S^T = (K @ Wq) @ A1
            bqb = consts.tile([C, 1], BF16)
            nc.vector.tensor_copy(out=bqb, in_=bq)
            pkw = psT.tile([C, NR], F32, tag="tp", name="pkw")
            nc.tensor.matmul(out=pkw, lhsT=wq, rhs=kvcm[0:C, :],
                             start=True, stop=True)
            kwt = consts.tile([C, NR], BF16)
            nc.scalar.copy(out=kwt, in_=pkw)
            sbias = consts.tile([128, 2], F32)
            for hh in range(2):
                pb = psT.tile([128, 1], F32, tag="tp", name="pb")
                nc.tensor.matmul(out=pb,
                                 lhsT=kvcm[0:C, 128 * hh:128 * (hh + 1)],
                                 rhs=bqb, start=True, stop=True)
                nc.vector.tensor_copy(out=sbias[:, hh:hh + 1], in_=pb)
            vp = consts.tile([128, 2, C + 1], BF16)
            nc.vector.memset(vp[:, :, C:C + 1], 1.0)
            for hh in range(2):
                pv = psT.tile([128, C], BF16, tag="tp")
                nc.tensor.transpose(out=pv,
                                    in_=kvcm[C:2 * C, 128 * hh:128 * (hh + 1)],
                                    identity=ident[C:2 * C, C:2 * C])
                nc.vector.tensor_copy(out=vp[:, hh, 0:C], in_=pv)

            # ---- attention, streamed in 512-column chunks ----
            y_tm = big.tile([P, T, C], F32, tag="y")
            sq2 = big.tile([P, T * C], BF16, tag="scr2", name="sq2")
            sq2v = sq2.rearrange("p (t c) -> p t c", c=C)
            s1y = sc.tile([P, T], F32, tag="s1y")
            s2y = sc.tile([P, T], F32, tag="s2y")
            for i in range(32):
                ech = ch.tile([128, 2, 512], BF16, tag="e")
                for hh in range(2):
                    pS = psA.tile([128, 512], F32, tag="ps", name="ps")
                    nc.tensor.matmul(out=pS,
                                     lhsT=kwt[:, 128 * hh:128 * (hh + 1)],
                                     rhs=a1cm[:, 512 * i:512 * (i + 1)],
                                     start=True, stop=True)
                    nc.scalar.activation(out=ech[:, hh, :], in_=pS, func=AF.Exp,
                                         bias=sbias[:, hh:hh + 1], scale=1.0)
                pO = psA.tile([128, 512], F32, tag="ps", name="ps")[0:C + 1, :]
                for hh in range(2):
                    nc.tensor.matmul(out=pO, lhsT=vp[:, hh, :],
                                     rhs=ech[:, hh, :],
                                     start=(hh == 0), stop=(hh == 1))
                pod = ch.tile([C + 1, 512], BF16, tag="pod")
                nc.vector.tensor_copy(out=pod, in_=pO)
                ptr = psT.tile([128, 4, C + 1], F32, tag="tp")
                for k in range(4):
                    nc.tensor.matmul(out=ptr[:, k, :],
                                     lhsT=pod[:, 128 * k:128 * (k + 1)],
                                     rhs=wpj, start=True, stop=True)
                rt = sc.tile([P, 4, 1], F32, tag="rt")
                nc.vector.reciprocal(out=rt, in_=ptr[:, :, C:C + 1])
                tmp = ch.tile([P, 4, C], F32, tag="tmp")
                nc.vector.tensor_tensor(out=tmp, in0=ptr[:, :, 0:C],
                                        in1=rt.broadcast_to([P, 4, C]),
                                        op=OP.mult)
                nc.vector.tensor_tensor(out=y_tm[:, 4 * i:4 * (i + 1), :],
                                        in0=tmp, in1=x_tm[:, 4 * i:4 * (i + 1), :],
                                        op=OP.add)
                if i % 4 == 3:
                    sl = slice(16 * (i // 4), 16 * (i // 4 + 1))
                    nc.scalar.square(out=sq2v[:, sl, :], in_=y_tm[:, sl, :])
                    nc.vector.tensor_reduce(out=s1y[:, sl], in_=y_tm[:, sl, :],
                                            axis=AX.X, op=OP.add)
                    nc.vector.tensor_reduce(out=s2y[:, sl], in_=sq2v[:, sl, :],
                                            axis=AX.X, op=OP.add)

            # ---- LN2 ----
            g2, mg2 = _ln_finalize(nc, sc, s1y, s2y, epst, T)
            a2tm = big.tile([P, T, C], BF16, tag="scr2")
            for q8 in range(8):
                sl = slice(16 * q8, 16 * (q8 + 1))
                nc.vector.tensor_tensor(
                    out=a2tm[:, sl, :], in0=y_tm[:, sl, :],
                    in1=g2[:, sl, None].broadcast_to([P, 16, C]), op=OP.mult)
                nc.vector.tensor_tensor(
                    out=a2tm[:, sl, :], in0=a2tm[:, sl, :],
                    in1=mg2[:, sl, None].broadcast_to([P, 16, C]),
                    op=OP.subtract)
            # re-warm PE after the LN2 lull
            for wd in range(8):
                pw = psT.tile([128, 128], F32, tag="tp", name="pw")
                nc.tensor.matmul(out=pw, lhsT=ident, rhs=ident,
                                 start=True, stop=True)

            # ---- A2 guarded channel-major, doubled: rows 64:128 shifted by +1 ----
            a2g = big.tile([128, NG], BF16, tag="acm")
            nc.vector.memset(a2g[:, 0:PAD + RP], 0.0)
            nc.vector.memset(a2g[:, NG - PAD - RP:NG], 0.0)
            a2rows = a2g[0:C, PAD + RP:PAD + RP * (H + 1)].rearrange(
                "c (y w) -> c y w", w=RP)
            a2rowsB = a2g[C:128, PAD + RP:PAD + RP * (H + 1)].rearrange(
                "c (y w) -> c y w", w=RP)
            nc.vector.memset(a2rows[:, :, 0:1], 0.0)
            nc.vector.memset(a2rows[:, :, RP - 1:RP], 0.0)
            nc.vector.memset(a2rowsB[:, :, RP - 2:RP], 0.0)
            a2tm_v = a2tm.rearrange("p t c -> p (t c)")
            for j in range(16):
                pt = psT.tile([128, 4, 128], BF16, tag="tp")
                for k in range(4):
                    tt = 8 * j + 2 * k
                    nc.tensor.transpose(out=pt[:, k, :],
                                        in_=a2tm_v[:, 64 * tt:64 * (tt + 2)],
                                        identity=ident)
                ro = a2rows.rearrange("c (j a b) w -> c j a b w", a=4, b=2)
                nc.scalar.copy(out=ro[:, j, :, 0, 1:W + 1], in_=pt[0:C, :, :])
                nc.vector.tensor_copy(out=ro[:, j, :, 1, 1:W + 1],
                                      in_=pt[C:128, :, :])
                nc.vector.tensor_copy(out=a2rowsB[:, 8 * j:8 * (j + 1), 0:W],
                                      in_=a2rows[:, 8 * j:8 * (j + 1), 1:W + 1])

            # ---- MLP: fused fc1 (+) 3x3 depthwise conv, gelu, fc2 ----
            o2cm = big.tile([C, NG], BF16, tag="qt")
            n_mlp = 33
            for j in range(n_mlp):
                cb = PAD + RP + 512 * j
                size = min(512, PAD + RP * (H + 1) - cb)
                gch = []
                for g in range(2):
                    pG = psA.tile([128, 512], F32, tag="ps", name="ps")
                    for dy in (-1, 0, 1):
                        nc.tensor.matmul(
                            out=pG[:, 0:size], lhsT=wmp[:, 2 * (dy + 1) + g, :],
                            rhs=a2g[:, cb + RP * dy - 1:cb + RP * dy - 1 + size],
                            start=(dy == -1), stop=False)
                    for dy in (-1, 0, 1):
                        nc.tensor.matmul(
                            out=pG[:, 0:size], lhsT=wms[:, 2 * (dy + 1) + g, :],
                            rhs=a2g[0:C, cb + RP * dy + 1:cb + RP * dy + 1 + size],
                            start=False, stop=(dy == 1))
                    gc = ch.tile([128, 512], BF16, tag=f"gc{g}")
                    nc.scalar.activation(out=gc[:, 0:size], in_=pG[:, 0:size],
                                         func=AF.Gelu, bias=bg[:, g:g + 1],
                                         scale=1.0)
                    gch.append(gc)
                pF = psA.tile([128, 512], F32, tag="ps", name="ps")
                for g in range(2):
                    nc.tensor.matmul(out=pF[0:C, 0:size], lhsT=wf2[:, g, :],
                                     rhs=gch[g][:, 0:size],
                                     start=(g == 0), stop=(g == 1))
                nc.vector.tensor_scalar(out=o2cm[:, cb:cb + size],
                                        in0=pF[0:C, 0:size], scalar1=bf2,
                                        scalar2=None, op0=OP.add)

            # ---- MLP epilogue: transpose back, residual, store ----
            y2_tm = big.tile([P, T, C], F32, tag="xr")  # reuses x_tm slot
            out_v = out_d.rearrange("(t p) c -> p t c", p=P)
            for j in range(32):
                pt2 = psT.tile([128, 4, C], BF16, tag="tp")
                for k in range(4):
                    t = 4 * j + k
                    s = PAD + RP * (t + 1) + 1
                    nc.tensor.transpose(out=pt2[:, k, :],
                                        in_=o2cm[:, s:s + W],
                                        identity=ident[0:C, 0:C])
                nc.vector.tensor_tensor(out=y2_tm[:, 4 * j:4 * (j + 1), :],
                                        in0=pt2, in1=y_tm[:, 4 * j:4 * (j + 1), :],
                                        op=OP.add)
                if j % 4 == 3:
                    q8 = j // 4
                    nc.sync.dma_start(out=out_v[:, 16 * q8:16 * (q8 + 1), :],
                                       in_=y2_tm[:, 16 * q8:16 * (q8 + 1), :])

    _split_excess_waits(nc)
    return nc


@functools.cache
def _get_nc():
    return _build_nc()


def _prep_weights(inp):
    f = lambda v: np.asarray(v, np.float32)
    n1w, n1b = f(inp["n1_w"]), f(inp["n1_b"])
    q_w, q_b = f(inp["q_w"]), f(inp["q_b"])
    kv_w, kv_b = f(inp["kv_w"]), f(inp["kv_b"])
    sr_w, sr_b = f(inp["sr_w"]), f(inp["sr_b"])
    srnw, srnb = f(inp["srn_w"]), f(inp["srn_b"])
    pj_w, pj_b = f(inp["proj_w"]), f(inp["proj_b"])
    n2w, n2b = f(inp["n2_w"]), f(inp["n2_b"])
    f1w, f1b = f(inp["fc1_w"]), f(inp["fc1_b"])
    dww, dwb = f(inp["dw_w"]), f(inp["dw_b"])
    f2w, f2b = f(inp["fc2_w"]), f(inp["fc2_b"])

    scale = (C // 1) ** -0.5
    wq_l = (q_w * n1w[None, :]).T * scale
    bq_l = ((q_w @ n1b + q_b) * scale)[:, None]

    wsr_l = np.zeros((64, C, C), np.float32)
    for kk in range(64):
        ky, kx = kk // 8, kk % 8
        wsr_l[kk, :, :] = (sr_w[:, :, ky, kx] * n1w[None, :]).T
    wsr_l = wsr_l.transpose(1, 0, 2)
    bsr_l = (sr_w.sum((2, 3)) @ n1b + sr_b)[:, None]

    wkv_l = (kv_w * srnw[None, :]).T
    bkv_l = (kv_w @ srnb + kv_b)[:, None]

    wpj_l = np.zeros((C + 1, C + 1), np.float32)
    wpj_l[:C, :C] = pj_w.T
    wpj_l[C, :C] = pj_b
    wpj_l[C, C] = 1.0

    k9 = dww[:, 0, :, :].reshape(HID, 9)          # [256, 9]
    wmp_l = np.zeros((6, 128, 128), np.float32)
    wms_l = np.zeros((6, C, 128), np.float32)
    for dy in range(3):
        for g in range(2):
            Ma = (k9[:, dy * 3 + 0][:, None] * f1w * n2w[None, :])[128 * g:128 * (g + 1)]
            Mb = (k9[:, dy * 3 + 1][:, None] * f1w * n2w[None, :])[128 * g:128 * (g + 1)]
            Mc = (k9[:, dy * 3 + 2][:, None] * f1w * n2w[None, :])[128 * g:128 * (g + 1)]
            wmp_l[2 * dy + g, :C, :] = Ma.T
            wmp_l[2 * dy + g, C:, :] = Mb.T
            wms_l[2 * dy + g, :, :] = Mc.T
    wmp_l = wmp_l.transpose(1, 0, 2)
    wms_l = wms_l.transpose(1, 0, 2)
    bg_full = k9.sum(1) * (f1w @ n2b + f1b) + dwb  # [256]
    bg_l = np.ascontiguousarray(bg_full.reshape(2, 128).T)

    wf2_l = np.stack([f2w[:, :128].T, f2w[:, 128:].T], 0).transpose(1, 0, 2)
    bf2_l = f2b[:, None]

    bfc = lambda a: np.ascontiguousarray(a).astype(BF)
    return {
        "wq": bfc(wq_l), "bq": np.ascontiguousarray(bq_l),
        "wsr": bfc(wsr_l), "bsr": np.ascontiguousarray(bsr_l),
        "wkv": bfc(wkv_l), "bkv": np.ascontiguousarray(bkv_l),
        "wpj": bfc(wpj_l),
        "wmp": bfc(wmp_l), "wms": bfc(wms_l),
        "bg": np.ascontiguousarray(bg_l),
        "wf2": bfc(wf2_l), "bf2": np.ascontiguousarray(bf2_l),
    }


def kernel(trace=False, tmpdir=None, **inputs):
    nc = _get_nc()
    x = np.asarray(inputs["x"], np.float32)
    wts = _prep_weights(inputs)
    in_maps = [dict(wts, x=np.ascontiguousarray(x[b])) for b in range(B)]
    res = run_bass_kernel_spmd(nc, in_maps, core_ids=list(range(8)),
                               trace=trace, tmpdir=tmpdir)
    out = np.stack([res.results[b]["out"] for b in range(B)], 0)
    kernel.last_exec_time_ns = res.exec_time_ns
    return out


# revision 26
# speedup vs baseline: 1.3436x; 1.0141x over previous
"""Trainium2 Bass kernel for nn_Block_523986010339 (PVT-style transformer block).

Sharding: data-parallel over batch B=8 -> one batch element per NeuronCore.
Per-core layouts:
  - residual stream token-major fp32 [128p=token%128, 128t=token//128, 64c]
  - matmul operands channel-major bf16 [c, n], n = 128*y + x
  - LN mean folded into matmul weights via an extra "m*g" row; rsqrt scale
    applied token-major with broadcast APs
  - attention: S^T channel-major, exp without max-subtraction (tiny logits),
    denominator via fused ones-column in the V matmul, divided out after proj
  - MLP: fc1 and 3x3 depthwise conv fused into 9 accumulated matmuls over a
    zero-guarded channel-major layout (row pitch 130)
"""

import functools
import json

import numpy as np
import ml_dtypes

import concourse.bass as bass
import concourse.mybir as mybir
import concourse.tile as tile
from concourse.bass_utils import run_bass_kernel_spmd
from concourse.masks import make_identity

F32 = mybir.dt.float32
BF16 = mybir.dt.bfloat16
BF = ml_dtypes.bfloat16

B, N, C, H, W = 8, 16384, 64, 128, 128
SR, HID, NR = 8, 256, 256
P, T = 128, 128
RP = W + 2          # guarded row pitch
PAD = RP + 1        # head/tail pad so all tap offsets stay in-bounds
NG = PAD + RP * (H + 2) + PAD
AX = mybir.AxisListType
OP = mybir.AluOpType
AF = mybir.ActivationFunctionType


def _split_excess_waits(nc, max_waits=1):
    """walrus in this container rejects >1 sync wait per instruction; move
    excess waits onto injected Drain instructions just before the owner."""
    d = json.loads(mybir.module_to_json_string(nc.m))
    n_split = [0]

    def fix(insts):
        out = []
        for inst in insts:
            si = inst.get("sync_info") or {}
            waits = si.get("on_wait") or []
            if len(waits) > max_waits:
                extra = waits[:-max_waits]
                for i in range(0, len(extra), max_waits):
                    n_split[0] += 1
                    out.append({
                        "name": f"WSPLIT-{n_split[0]}",
                        "opcode": "NoOp",
                        "engine": inst["engine"],
                        "ins": [],
                        "outs": [],
                        "is_reset_sema": False,
                        "sync_info": {"on_update": [],
                                      "on_wait": extra[i:i + max_waits]},
                    })
                si["on_wait"] = waits[-max_waits:]
                inst["sync_info"] = si
            out.append(inst)
        return out

    for f in d.get("functions", []):
        for bb in f.get("blocks", []):
            bb["instructions"] = fix(bb["instructions"])
    nc.m = mybir.module_from_json_string(json.dumps(d))


def _ln_stats(nc, sc, big, x_tm, epst, nt):
    """Token-major LN stats: returns (g, mg) tiles [128, nt] fp32 given
    x_tm [128, nt, 64] fp32."""
    sq_scr = big.tile([P, nt * C], BF16, tag="scr2", name="sq")
    xsq_view = sq_scr.rearrange("p (t c) -> p t c", c=C)
    nc.scalar.square(out=sq_scr, in_=x_tm.rearrange("p t c -> p (t c)"))
    s1 = sc.tile([P, nt], F32, tag=f"s1_{nt}")
    s2 = sc.tile([P, nt], F32, tag=f"s2_{nt}")
    nc.vector.tensor_reduce(out=s1, in_=x_tm, axis=AX.X, op=OP.add)
    nc.vector.tensor_reduce(out=s2, in_=xsq_view, axis=AX.X, op=OP.add)
    return _ln_finalize(nc, sc, s1, s2, epst, nt)


def _ln_finalize(nc, sc, s1, s2, epst, nt):
    mean = sc.tile([P, nt], F32, tag=f"mean_{nt}")
    var = sc.tile([P, nt], F32, tag=f"var_{nt}")
    nc.vector.tensor_scalar_mul(out=mean, in0=s1, scalar1=1.0 / C)
    nc.vector.tensor_scalar_mul(out=var, in0=s2, scalar1=1.0 / C)
    mm = sc.tile([P, nt], F32, tag=f"mm_{nt}")
    nc.vector.tensor_tensor(out=mm, in0=mean, in1=mean, op=OP.mult)
    nc.vector.tensor_tensor(out=var, in0=var, in1=mm, op=OP.subtract)
    sd = sc.tile([P, nt], F32, tag=f"sd_{nt}")
    nc.scalar.activation(out=sd, in_=var, func=AF.Sqrt, bias=epst, scale=1.0)
    g = sc.tile([P, nt], F32, tag=f"g_{nt}")
    nc.vector.reciprocal(out=g, in_=sd)
    mg = sc.tile([P, nt], F32, tag=f"mg_{nt}")
    nc.vector.tensor_tensor(out=mg, in0=mean, in1=g, op=OP.mult)
    return g, mg


def _build_nc():
    nc = bass.Bass("TRN2")
    x_d = nc.dram_tensor("x", [N, C], F32, kind="ExternalInput")
    out_d = nc.dram_tensor("out", [N, C], F32, kind="ExternalOutput")
    wq_d = nc.dram_tensor("wq", [C, C], BF16, kind="ExternalInput")
    bq_d = nc.dram_tensor("bq", [C, 1], F32, kind="ExternalInput")
    wsr_d = nc.dram_tensor("wsr", [C, 64, C], BF16, kind="ExternalInput")
    bsr_d = nc.dram_tensor("bsr", [C, 1], F32, kind="ExternalInput")
    wkv_d = nc.dram_tensor("wkv", [C, 2 * C], BF16, kind="ExternalInput")
    bkv_d = nc.dram_tensor("bkv", [2 * C, 1], F32, kind="ExternalInput")
    wpj_d = nc.dram_tensor("wpj", [C + 1, C + 1], BF16, kind="ExternalInput")
    wmp_d = nc.dram_tensor("wmp", [128, 6, 128], BF16, kind="ExternalInput")
    wms_d = nc.dram_tensor("wms", [C, 6, 128], BF16, kind="ExternalInput")
    bg_d = nc.dram_tensor("bg", [128, 2], F32, kind="ExternalInput")
    wf2_d = nc.dram_tensor("wf2", [128, 2, C], BF16, kind="ExternalInput")
    bf2_d = nc.dram_tensor("bf2", [C, 1], F32, kind="ExternalInput")

    with tile.TileContext(nc) as tc:
        with (
            tc.tile_pool(name="consts", bufs=1) as consts,
            tc.tile_pool(name="big", bufs=1) as big,
            tc.tile_pool(name="sc", bufs=2) as sc,
            tc.tile_pool(name="ch", bufs=3) as ch,
            tc.tile_pool(name="psA", bufs=6, space="PSUM") as psA,
            tc.tile_pool(name="psT", bufs=2, space="PSUM") as psT,
        ):
            ident = consts.tile([128, 128], BF16)
            make_identity(nc, ident)
            wq = consts.tile([C, C], BF16)
            nc.gpsimd.dma_start(out=wq, in_=wq_d[:, :])
            wsr = consts.tile([C, 64, C], BF16)
            nc.gpsimd.dma_start(out=wsr, in_=wsr_d[:, :, :])
            wkv = consts.tile([C, 2 * C], BF16)
            nc.gpsimd.dma_start(out=wkv, in_=wkv_d[:, :])
            wpj = consts.tile([C + 1, C + 1], BF16)
            nc.gpsimd.dma_start(out=wpj, in_=wpj_d[:, :])
            wmp = consts.tile([128, 6, 128], BF16)
            nc.gpsimd.dma_start(out=wmp, in_=wmp_d[:, :, :])
            wms = consts.tile([C, 6, 128], BF16)
            nc.gpsimd.dma_start(out=wms, in_=wms_d[:, :, :])
            wf2 = consts.tile([128, 2, C], BF16)
            nc.gpsimd.dma_start(out=wf2, in_=wf2_d[:, :, :])
            bq = consts.tile([C, 1], F32)
            nc.gpsimd.dma_start(out=bq, in_=bq_d[:, :])
            bsr = consts.tile([C, 1], F32)
            nc.gpsimd.dma_start(out=bsr, in_=bsr_d[:, :])
            bkv = consts.tile([2 * C, 1], F32)
            nc.gpsimd.dma_start(out=bkv, in_=bkv_d[:, :])
            bg = consts.tile([128, 2], F32)
            nc.gpsimd.dma_start(out=bg, in_=bg_d[:, :])
            bf2 = consts.tile([C, 1], F32)
            nc.gpsimd.dma_start(out=bf2, in_=bf2_d[:, :])
            epst = consts.tile([P, 1], F32)
            nc.vector.memset(epst, 1e-5)

            # ---- load x (token-major), LN1 stats overlapped per slice ----
            x_tm = big.tile([P, T, C], F32, tag="xr")
            x_v = x_d.rearrange("(t p) c -> p t c", p=P)
            sq_scr = big.tile([P, T * C], BF16, tag="scr2", name="sq")
            sqv = sq_scr.rearrange("p (t c) -> p t c", c=C)
            s1 = sc.tile([P, T], F32, tag="s1")
            s2 = sc.tile([P, T], F32, tag="s2")
            for q8 in range(8):
                sl = slice(16 * q8, 16 * (q8 + 1))
                eng = nc.sync if q8 % 2 == 0 else nc.scalar
                eng.dma_start(out=x_tm[:, sl, :], in_=x_v[:, sl, :])
                nc.scalar.square(out=sqv[:, sl, :], in_=x_tm[:, sl, :])
                nc.vector.tensor_reduce(out=s1[:, sl], in_=x_tm[:, sl, :],
                                        axis=AX.X, op=OP.add)
                nc.vector.tensor_reduce(out=s2[:, sl], in_=sqv[:, sl, :],
                                        axis=AX.X, op=OP.add)
            g1, mg1 = _ln_finalize(nc, sc, s1, s2, epst, T)
            # warm up the PE so HAM is at 8/8 when real matmuls start
            for wd in range(15):
                pw = psT.tile([128, 128], F32, tag="tp", name="pw")
                nc.tensor.matmul(out=pw, lhsT=ident, rhs=ident,
                                 start=True, stop=True)
            a1tm = big.tile([P, T, C], BF16, tag="scr2")
            for q8 in range(8):
                sl = slice(16 * q8, 16 * (q8 + 1))
                nc.vector.tensor_tensor(
                    out=a1tm[:, sl, :], in0=x_tm[:, sl, :],
                    in1=g1[:, sl, None].broadcast_to([P, 16, C]), op=OP.mult)
                nc.vector.tensor_tensor(
                    out=a1tm[:, sl, :], in0=a1tm[:, sl, :],
                    in1=mg1[:, sl, None].broadcast_to([P, 16, C]),
                    op=OP.subtract)

            # transpose A1 to channel-major [64, N]: two tiles per transpose
            a1cm = big.tile([C, N], BF16, tag="acm")
            a1cm_v = a1cm.rearrange("c (j a b n) -> c j a b n", a=4, b=2, n=128)
            a1tm_v = a1tm.rearrange("p t c -> p (t c)")
            for j in range(16):
                pt = psT.tile([128, 4, 128], BF16, tag="tp")
                for k in range(4):
                    tt = 8 * j + 2 * k
                    nc.tensor.transpose(out=pt[:, k, :],
                                        in_=a1tm_v[:, 64 * tt:64 * (tt + 2)],
                                        identity=ident)
                nc.scalar.copy(out=a1cm_v[:, j, :, 0, :], in_=pt[0:C, :, :])
                nc.vector.tensor_copy(out=a1cm_v[:, j, :, 1, :],
                                      in_=pt[C:128, :, :])

            # ---- spatial reduction conv (8x8 stride 8) ----
            a1sr = a1cm.rearrange("c (Y ky X kx) -> c ky kx Y X", ky=SR, kx=SR, X=16)
            psr = psA.tile([128, 512], F32, tag="ps", name="ps").rearrange("c (a y x) -> c a y x", a=2, y=16)[0:C, 0, :, :]
            for kk in range(64):
                ky, kx = kk // 8, kk % 8
                nc.tensor.matmul(out=psr, lhsT=wsr[:, kk, :],
                                 rhs=a1sr[:, ky, kx, :, :],
                                 start=(kk == 0), stop=(kk == 63))
            xrcm = consts.tile([C, NR], BF16)
            nc.scalar.activation(out=xrcm.rearrange("c (y x) -> c y x", x=16),
                                 in_=psr, func=AF.Identity,
                                 bias=bsr, scale=1.0)

            # ---- LN on reduced tokens (srn), token-major ----
            xr_tm = consts.tile([P, 2, C], F32)
            for hh in range(2):
                pv = psT.tile([128, C], BF16, tag="tp")
                nc.tensor.transpose(out=pv, in_=xrcm[:, 128 * hh:128 * (hh + 1)],
                                    identity=ident[0:C, 0:C])
                nc.vector.tensor_copy(out=xr_tm[:, hh, :], in_=pv)
            g_r, mg_r = _ln_stats(nc, sc, consts, xr_tm, epst, 2)
            ar_tm = consts.tile([P, 2, C], BF16)
            nc.vector.tensor_tensor(
                out=ar_tm, in0=xr_tm,
                in1=g_r[:, :, None].broadcast_to([P, 2, C]), op=OP.mult)
            mgb = sc.tile([P, 2, C], BF16, tag="mgb")
            nc.vector.tensor_tensor(
                out=mgb, in0=mg_r[:, :, None].broadcast_to([P, 2, C]),
                in1=g_r[:, :, None].broadcast_to([P, 2, C]), op=OP.bypass)
            nc.vector.tensor_tensor(out=ar_tm, in0=ar_tm, in1=mgb, op=OP.subtract)
            arcm = consts.tile([C, NR], BF16)
            for hh in range(2):
                pv = psT.tile([C, 128], BF16, tag="tp")
                nc.tensor.transpose(out=pv, in_=ar_tm[:, hh, :], identity=ident)
                nc.vector.tensor_copy(out=arcm[:, 128 * hh:128 * (hh + 1)], in_=pv)

            # ---- KV ----
            pkv = psA.tile([128, 512], F32, tag="ps", name="ps")[:, 0:NR]
            nc.tensor.matmul(out=pkv, lhsT=wkv, rhs=arcm, start=True, stop=True)
            kvcm = consts.tile([2 * C, NR], BF16)
            nc.scalar.activation(out=kvcm, in_=pkv, func=AF.Identity,
                                 bias=bkv, scale=1.0)
            # fold q-projection into K:  S^T = (K @ Wq) @ A1
            bqb = consts.tile([C, 1], BF16)
            nc.vector.tensor_copy(out=bqb, in_=bq)
            pkw = psT.tile([C, NR], F32, tag="tp", name="pkw")
            nc.tensor.matmul(out=pkw, lhsT=wq, rhs=kvcm[0:C, :],
                             start=True, stop=True)
            kwt = consts.tile([C, NR], BF16)
            nc.scalar.copy(out=kwt, in_=pkw)
            sbias = consts.tile([128, 2], F32)
            for hh in range(2):
                pb = psT.tile([128, 1], F32, tag="tp", name="pb")
                nc.tensor.matmul(out=pb,
                                 lhsT=kvcm[0:C, 128 * hh:128 * (hh + 1)],
                                 rhs=bqb, start=True, stop=True)
                nc.vector.tensor_copy(out=sbias[:, hh:hh + 1], in_=pb)
            vp = consts.tile([128, 2, C + 1], BF16)
            nc.vector.memset(vp[:, :, C:C + 1], 1.0)
            for hh in range(2):
                pv = psT.tile([128, C], BF16, tag="tp")
                nc.tensor.transpose(out=pv,
                                    in_=kvcm[C:2 * C, 128 * hh:128 * (hh + 1)],
                                    identity=ident[C:2 * C, C:2 * C])
                nc.vector.tensor_copy(out=vp[:, hh, 0:C], in_=pv)

            # ---- attention, streamed in 512-column chunks ----
            y_tm = big.tile([P, T, C], F32, tag="y")
            sq2 = big.tile([P, T * C], BF16, tag="scr2", name="sq2")
            sq2v = sq2.rearrange("p (t c) -> p t c", c=C)
            s1y = sc.tile([P, T], F32, tag="s1y")
            s2y = sc.tile([P, T], F32, tag="s2y")
            for i in range(32):
                ech = ch.tile([128, 2, 512], BF16, tag="e")
                for hh in range(2):
                    pS = psA.tile([128, 512], F32, tag="ps", name="ps")
                    nc.tensor.matmul(out=pS,
                                     lhsT=kwt[:, 128 * hh:128 * (hh + 1)],
                                     rhs=a1cm[:, 512 * i:512 * (i + 1)],
                                     start=True, stop=True)
                    nc.scalar.activation(out=ech[:, hh, :], in_=pS, func=AF.Exp,
                                         bias=sbias[:, hh:hh + 1], scale=1.0)
                pO = psA.tile([128, 512], F32, tag="ps", name="ps")[0:C + 1, :]
                for hh in range(2):
                    nc.tensor.matmul(out=pO, lhsT=vp[:, hh, :],
                                     rhs=ech[:, hh, :],
                                     start=(hh == 0), stop=(hh == 1))
                pod = ch.tile([C + 1, 512], BF16, tag="pod")
                nc.vector.tensor_copy(out=pod, in_=pO)
                ptr = psT.tile([128, 4, C + 1], F32, tag="tp")
                for k in range(4):
                    nc.tensor.matmul(out=ptr[:, k, :],
                                     lhsT=pod[:, 128 * k:128 * (k + 1)],
                                     rhs=wpj, start=True, stop=True)
                rt = sc.tile([P, 4, 1], F32, tag="rt")
                nc.vector.reciprocal(out=rt, in_=ptr[:, :, C:C + 1])
                tmp = ch.tile([P, 4, C], F32, tag="tmp")
                nc.vector.tensor_tensor(out=tmp, in0=ptr[:, :, 0:C],
                                        in1=rt.broadcast_to([P, 4, C]),
                                        op=OP.mult)
                nc.vector.tensor_tensor(out=y_tm[:, 4 * i:4 * (i + 1), :],
                                        in0=tmp, in1=x_tm[:, 4 * i:4 * (i + 1), :],
                                        op=OP.add)
                if i % 4 == 3:
                    sl = slice(16 * (i // 4), 16 * (i // 4 + 1))
                    nc.scalar.square(out=sq2v[:, sl, :], in_=y_tm[:, sl, :])
                    nc.vector.tensor_reduce(out=s1y[:, sl], in_=y_tm[:, sl, :],
                                            axis=AX.X, op=OP.add)
                    nc.vector.tensor_reduce(out=s2y[:, sl], in_=sq2v[:, sl, :],
                                            axis=AX.X, op=OP.add)

            # ---- LN2 ----
            g2, mg2 = _ln_finalize(nc, sc, s1y, s2y, epst, T)
            a2tm = big.tile([P, T, C], BF16, tag="scr2")
            for q8 in range(8):
                sl = slice(16 * q8, 16 * (q8 + 1))
                nc.vector.tensor_tensor(
                    out=a2tm[:, sl, :], in0=y_tm[:, sl, :],
                    in1=g2[:, sl, None].broadcast_to([P, 16, C]), op=OP.mult)
                nc.vector.tensor_tensor(
                    out=a2tm[:, sl, :], in0=a2tm[:, sl, :],
                    in1=mg2[:, sl, None].broadcast_to([P, 16, C]),
                    op=OP.subtract)
            # re-warm PE after the LN2 lull
            for wd in range(8):
                pw = psT.tile([128, 128], F32, tag="tp", name="pw")
                nc.tensor.matmul(out=pw, lhsT=ident, rhs=ident,
                                 start=True, stop=True)

            # ---- A2 guarded channel-major, doubled: rows 64:128 shifted by +1 ----
            a2g = big.tile([128, NG], BF16, tag="acm")
            nc.vector.memset(a2g[:, 0:PAD + RP], 0.0)
            nc.vector.memset(a2g[:, NG - PAD - RP:NG], 0.0)
            a2rows = a2g[0:C, PAD + RP:PAD + RP * (H + 1)].rearrange(
                "c (y w) -> c y w", w=RP)
            a2rowsB = a2g[C:128, PAD + RP:PAD + RP * (H + 1)].rearrange(
                "c (y w) -> c y w", w=RP)
            nc.vector.memset(a2rows[:, :, 0:1], 0.0)
            nc.vector.memset(a2rows[:, :, RP - 1:RP], 0.0)
            nc.vector.memset(a2rowsB[:, :, RP - 2:RP], 0.0)
            a2tm_v = a2tm.rearrange("p t c -> p (t c)")
            for j in range(16):
                pt = psT.tile([128, 4, 128], BF16, tag="tp")
                for k in range(4):
                    tt = 8 * j + 2 * k
                    nc.tensor.transpose(out=pt[:, k, :],
                                        in_=a2tm_v[:, 64 * tt:64 * (tt + 2)],
                                        identity=ident)
                ro = a2rows.rearrange("c (j a b) w -> c j a b w", a=4, b=2)
                nc.scalar.copy(out=ro[:, j, :, 0, 1:W + 1], in_=pt[0:C, :, :])
                nc.vector.tensor_copy(out=ro[:, j, :, 1, 1:W + 1],
                                      in_=pt[C:128, :, :])
                nc.vector.tensor_copy(out=a2rowsB[:, 8 * j:8 * (j + 1), 0:W],
                                      in_=a2rows[:, 8 * j:8 * (j + 1), 1:W + 1])

            # ---- MLP: fused fc1 (+) 3x3 depthwise conv, gelu, fc2 ----
            o2cm = big.tile([C, NG], BF16, tag="qt")
            n_mlp = 33
            for j in range(n_mlp):
                cb = PAD + RP + 512 * j
                size = min(512, PAD + RP * (H + 1) - cb)
                gch = []
                for g in range(2):
                    pG = psA.tile([128, 512], F32, tag="ps", name="ps")
                    for dy in (-1, 0, 1):
                        nc.tensor.matmul(
                            out=pG[:, 0:size], lhsT=wmp[:, 2 * (dy + 1) + g, :],
                            rhs=a2g[:, cb + RP * dy - 1:cb + RP * dy - 1 + size],
                            start=(dy == -1), stop=False)
                    for dy in (-1, 0, 1):
                        nc.tensor.matmul(
                            out=pG[:, 0:size], lhsT=wms[:, 2 * (dy + 1) + g, :],
                            rhs=a2g[0:C, cb + RP * dy + 1:cb + RP * dy + 1 + size],
                            start=False, stop=(dy == 1))
                    gc = ch.tile([128, 512], BF16, tag=f"gc{g}")
                    nc.scalar.activation(out=gc[:, 0:size], in_=pG[:, 0:size],
                                         func=AF.Gelu, bias=bg[:, g:g + 1],
                                         scale=1.0)
                    gch.append(gc)
                pF = psA.tile([128, 512], F32, tag="ps", name="ps")
                for g in range(2):
                    nc.tensor.matmul(out=pF[0:C, 0:size], lhsT=wf2[:, g, :],
                                     rhs=gch[g][:, 0:size],
                                     start=(g == 0), stop=(g == 1))
                nc.vector.tensor_scalar(out=o2cm[:, cb:cb + size],
                                        in0=pF[0:C, 0:size], scalar1=bf2,
                                        scalar2=None, op0=OP.add)

            # ---- MLP epilogue: transpose back, residual, store ----
            y2_tm = big.tile([P, T, C], F32, tag="xr")  # reuses x_tm slot
            out_v = out_d.rearrange("(t p) c -> p t c", p=P)
            for j in range(32):
                pt2 = psT.tile([128, 4, C], BF16, tag="tp")
                for k in range(4):
                    t = 4 * j + k
                    s = PAD + RP * (t + 1) + 1
                    nc.tensor.transpose(out=pt2[:, k, :],
                                        in_=o2cm[:, s:s + W],
                                        identity=ident[0:C, 0:C])
                nc.vector.tensor_tensor(out=y2_tm[:, 4 * j:4 * (j + 1), :],
                                        in0=pt2, in1=y_tm[:, 4 * j:4 * (j + 1), :],
                                        op=OP.add)
                if j % 4 == 3:
                    q8 = j // 4
                    nc.sync.dma_start(out=out_v[:, 16 * q8:16 * (q8 + 1), :],
                                       in_=y2_tm[:, 16 * q8:16 * (q8 + 1), :])

    _split_excess_waits(nc)
    return nc


@functools.cache
def _get_nc():
    return _build_nc()


def _prep_weights(inp):
    f = lambda v: np.asarray(v, np.float32)
    n1w, n1b = f(inp["n1_w"]), f(inp["n1_b"])
    q_w, q_b = f(inp["q_w"]), f(inp["q_b"])
    kv_w, kv_b = f(inp["kv_w"]), f(inp["kv_b"])
    sr_w, sr_b = f(inp["sr_w"]), f(inp["sr_b"])
    srnw, srnb = f(inp["srn_w"]), f(inp["srn_b"])
    pj_w, pj_b = f(inp["proj_w"]), f(inp["proj_b"])
    n2w, n2b = f(inp["n2_w"]), f(inp["n2_b"])
    f1w, f1b = f(inp["fc1_w"]), f(inp["fc1_b"])
    dww, dwb = f(inp["dw_w"]), f(inp["dw_b"])
    f2w, f2b = f(inp["fc2_w"]), f(inp["fc2_b"])

    scale = (C // 1) ** -0.5
    wq_l = (q_w * n1w[None, :]).T * scale
    bq_l = ((q_w @ n1b + q_b) * scale)[:, None]

    wsr_l = np.zeros((64, C, C), np.float32)
    for kk in range(64):
        ky, kx = kk // 8, kk % 8
        wsr_l[kk, :, :] = (sr_w[:, :, ky, kx] * n1w[None, :]).T
    wsr_l = wsr_l.transpose(1, 0, 2)
    bsr_l = (sr_w.sum((2, 3)) @ n1b + sr_b)[:, None]

    wkv_l = (kv_w * srnw[None, :]).T
    bkv_l = (kv_w @ srnb + kv_b)[:, None]

    wpj_l = np.zeros((C + 1, C + 1), np.float32)
    wpj_l[:C, :C] = pj_w.T
    wpj_l[C, :C] = pj_b
    wpj_l[C, C] = 1.0

    k9 = dww[:, 0, :, :].reshape(HID, 9)          # [256, 9]
    wmp_l = np.zeros((6, 128, 128), np.float32)
    wms_l = np.zeros((6, C, 128), np.float32)
    for dy in range(3):
        for g in range(2):
            Ma = (k9[:, dy * 3 + 0][:, None] * f1w * n2w[None, :])[128 * g:128 * (g + 1)]
            Mb = (k9[:, dy * 3 + 1][:, None] * f1w * n2w[None, :])[128 * g:128 * (g + 1)]
            Mc = (k9[:, dy * 3 + 2][:, None] * f1w * n2w[None, :])[128 * g:128 * (g + 1)]
            wmp_l[2 * dy + g, :C, :] = Ma.T
            wmp_l[2 * dy + g, C:, :] = Mb.T
            wms_l[2 * dy + g, :, :] = Mc.T
    wmp_l = wmp_l.transpose(1, 0, 2)
    wms_l = wms_l.transpose(1, 0, 2)
    bg_full = k9.sum(1) * (f1w @ n2b + f1b) + dwb  # [256]
    bg_l = np.ascontiguousarray(bg_full.reshape(2, 128).T)

    wf2_l = np.stack([f2w[:, :128].T, f2w[:, 128:].T], 0).transpose(1, 0, 2)
    bf2_l = f2b[:, None]

    bfc = lambda a: np.ascontiguousarray(a).astype(BF)
    return {
        "wq": bfc(wq_l), "bq": np.ascontiguousarray(bq_l),
        "wsr": bfc(wsr_l), "bsr": np.ascontiguousarray(bsr_l),
        "wkv": bfc(wkv_l), "bkv": np.ascontiguousarray(bkv_l),
        "wpj": bfc(wpj_l),
        "wmp": bfc(wmp_l), "wms": bfc(wms_l),
        "bg": np.ascontiguousarray(bg_l),
        "wf2": bfc(wf2_l), "bf2": np.ascontiguousarray(bf2_l),
    }


def kernel(trace=False, tmpdir=None, **inputs):
    nc = _get_nc()
    x = np.asarray(inputs["x"], np.float32)
    wts = _prep_weights(inputs)
    in_maps = [dict(wts, x=np.ascontiguousarray(x[b])) for b in range(B)]
    res = run_bass_kernel_spmd(nc, in_maps, core_ids=list(range(8)),
                               trace=trace, tmpdir=tmpdir)
    out = np.stack([res.results[b]["out"] for b in range(B)], 0)
    kernel.last_exec_time_ns = res.exec_time_ns
    return out


# revision 27
# speedup vs baseline: 1.3696x; 1.0194x over previous
"""Trainium2 Bass kernel for nn_Block_523986010339 (PVT-style transformer block).

Sharding: data-parallel over batch B=8 -> one batch element per NeuronCore.
Per-core layouts:
  - residual stream token-major fp32 [128p=token%128, 128t=token//128, 64c]
  - matmul operands channel-major bf16 [c, n], n = 128*y + x
  - LN mean folded into matmul weights via an extra "m*g" row; rsqrt scale
    applied token-major with broadcast APs
  - attention: S^T channel-major, exp without max-subtraction (tiny logits),
    denominator via fused ones-column in the V matmul, divided out after proj
  - MLP: fc1 and 3x3 depthwise conv fused into 9 accumulated matmuls over a
    zero-guarded channel-major layout (row pitch 130)
"""

import functools
import json

import numpy as np
import ml_dtypes

import concourse.bass as bass
import concourse.mybir as mybir
import concourse.tile as tile
from concourse.bass_utils import run_bass_kernel_spmd
from concourse.masks import make_identity

F32 = mybir.dt.float32
BF16 = mybir.dt.bfloat16
BF = ml_dtypes.bfloat16

B, N, C, H, W = 8, 16384, 64, 128, 128
SR, HID, NR = 8, 256, 256
P, T = 128, 128
RP = W + 2          # guarded row pitch
PAD = RP + 1        # head/tail pad so all tap offsets stay in-bounds
NG = PAD + RP * (H + 2) + PAD
AX = mybir.AxisListType
OP = mybir.AluOpType
AF = mybir.ActivationFunctionType


def _split_excess_waits(nc, max_waits=1):
    """walrus in this container rejects >1 sync wait per instruction; move
    excess waits onto injected Drain instructions just before the owner."""
    d = json.loads(mybir.module_to_json_string(nc.m))
    n_split = [0]

    def fix(insts):
        out = []
        for inst in insts:
            si = inst.get("sync_info") or {}
            waits = si.get("on_wait") or []
            if len(waits) > max_waits:
                extra = waits[:-max_waits]
                for i in range(0, len(extra), max_waits):
                    n_split[0] += 1
                    out.append({
                        "name": f"WSPLIT-{n_split[0]}",
                        "opcode": "NoOp",
                        "engine": inst["engine"],
                        "ins": [],
                        "outs": [],
                        "is_reset_sema": False,
                        "sync_info": {"on_update": [],
                                      "on_wait": extra[i:i + max_waits]},
                    })
                si["on_wait"] = waits[-max_waits:]
                inst["sync_info"] = si
            out.append(inst)
        return out

    for f in d.get("functions", []):
        for bb in f.get("blocks", []):
            bb["instructions"] = fix(bb["instructions"])
    nc.m = mybir.module_from_json_string(json.dumps(d))


def _ln_stats(nc, sc, big, x_tm, epst, nt):
    """Token-major LN stats: returns (g, mg) tiles [128, nt] fp32 given
    x_tm [128, nt, 64] fp32."""
    sq_scr = big.tile([P, nt * C], BF16, tag="scr2", name="sq")
    xsq_view = sq_scr.rearrange("p (t c) -> p t c", c=C)
    nc.scalar.square(out=sq_scr, in_=x_tm.rearrange("p t c -> p (t c)"))
    s1 = sc.tile([P, nt], F32, tag=f"s1_{nt}")
    s2 = sc.tile([P, nt], F32, tag=f"s2_{nt}")
    nc.vector.tensor_reduce(out=s1, in_=x_tm, axis=AX.X, op=OP.add)
    nc.vector.tensor_reduce(out=s2, in_=xsq_view, axis=AX.X, op=OP.add)
    return _ln_finalize(nc, sc, s1, s2, epst, nt)


def _ln_finalize(nc, sc, s1, s2, epst, nt):
    mean = sc.tile([P, nt], F32, tag=f"mean_{nt}")
    var = sc.tile([P, nt], F32, tag=f"var_{nt}")
    nc.vector.tensor_scalar_mul(out=mean, in0=s1, scalar1=1.0 / C)
    nc.vector.tensor_scalar_mul(out=var, in0=s2, scalar1=1.0 / C)
    mm = sc.tile([P, nt], F32, tag=f"mm_{nt}")
    nc.vector.tensor_tensor(out=mm, in0=mean, in1=mean, op=OP.mult)
    nc.vector.tensor_tensor(out=var, in0=var, in1=mm, op=OP.subtract)
    sd = sc.tile([P, nt], F32, tag=f"sd_{nt}")
    nc.scalar.activation(out=sd, in_=var, func=AF.Sqrt, bias=epst, scale=1.0)
    g = sc.tile([P, nt], F32, tag=f"g_{nt}")
    nc.vector.reciprocal(out=g, in_=sd)
    mg = sc.tile([P, nt], F32, tag=f"mg_{nt}")
    nc.vector.tensor_tensor(out=mg, in0=mean, in1=g, op=OP.mult)
    return g, mg


def _build_nc():
    nc = bass.Bass("TRN2")
    x_d = nc.dram_tensor("x", [N, C], F32, kind="ExternalInput")
    out_d = nc.dram_tensor("out", [N, C], F32, kind="ExternalOutput")
    wq_d = nc.dram_tensor("wq", [C, C], BF16, kind="ExternalInput")
    bq_d = nc.dram_tensor("bq", [C, 1], F32, kind="ExternalInput")
    wsr_d = nc.dram_tensor("wsr", [128, 32, C], BF16, kind="ExternalInput")
    bsr_d = nc.dram_tensor("bsr", [C, 1], F32, kind="ExternalInput")
    wkv_d = nc.dram_tensor("wkv", [C, 2 * C], BF16, kind="ExternalInput")
    bkv_d = nc.dram_tensor("bkv", [2 * C, 1], F32, kind="ExternalInput")
    wpj_d = nc.dram_tensor("wpj", [C + 1, C + 1], BF16, kind="ExternalInput")
    wmp_d = nc.dram_tensor("wmp", [128, 6, 128], BF16, kind="ExternalInput")
    wms_d = nc.dram_tensor("wms", [C, 6, 128], BF16, kind="ExternalInput")
    bg_d = nc.dram_tensor("bg", [128, 2], F32, kind="ExternalInput")
    wf2_d = nc.dram_tensor("wf2", [128, 2, C], BF16, kind="ExternalInput")
    bf2_d = nc.dram_tensor("bf2", [C, 1], F32, kind="ExternalInput")

    with tile.TileContext(nc) as tc:
        with (
            tc.tile_pool(name="consts", bufs=1) as consts,
            tc.tile_pool(name="big", bufs=1) as big,
            tc.tile_pool(name="sc", bufs=2) as sc,
            tc.tile_pool(name="ch", bufs=3) as ch,
            tc.tile_pool(name="psA", bufs=6, space="PSUM") as psA,
            tc.tile_pool(name="psT", bufs=2, space="PSUM") as psT,
        ):
            ident = consts.tile([128, 128], BF16)
            make_identity(nc, ident)
            wq = consts.tile([C, C], BF16)
            nc.gpsimd.dma_start(out=wq, in_=wq_d[:, :])
            wsr = consts.tile([128, 32, C], BF16)
            nc.gpsimd.dma_start(out=wsr, in_=wsr_d[:, :, :])
            wkv = consts.tile([C, 2 * C], BF16)
            nc.gpsimd.dma_start(out=wkv, in_=wkv_d[:, :])
            wpj = consts.tile([C + 1, C + 1], BF16)
            nc.gpsimd.dma_start(out=wpj, in_=wpj_d[:, :])
            wmp = consts.tile([128, 6, 128], BF16)
            nc.gpsimd.dma_start(out=wmp, in_=wmp_d[:, :, :])
            wms = consts.tile([C, 6, 128], BF16)
            nc.gpsimd.dma_start(out=wms, in_=wms_d[:, :, :])
            wf2 = consts.tile([128, 2, C], BF16)
            nc.gpsimd.dma_start(out=wf2, in_=wf2_d[:, :, :])
            bq = consts.tile([C, 1], F32)
            nc.gpsimd.dma_start(out=bq, in_=bq_d[:, :])
            bsr = consts.tile([C, 1], F32)
            nc.gpsimd.dma_start(out=bsr, in_=bsr_d[:, :])
            bkv = consts.tile([2 * C, 1], F32)
            nc.gpsimd.dma_start(out=bkv, in_=bkv_d[:, :])
            bg = consts.tile([128, 2], F32)
            nc.gpsimd.dma_start(out=bg, in_=bg_d[:, :])
            bf2 = consts.tile([C, 1], F32)
            nc.gpsimd.dma_start(out=bf2, in_=bf2_d[:, :])
            epst = consts.tile([P, 1], F32)
            nc.vector.memset(epst, 1e-5)

            # ---- load x (token-major), LN1 stats overlapped per slice ----
            x_tm = big.tile([P, T, C], F32, tag="xr")
            x_v = x_d.rearrange("(t p) c -> p t c", p=P)
            sq_scr = big.tile([P, T * C], BF16, tag="scr2", name="sq")
            sqv = sq_scr.rearrange("p (t c) -> p t c", c=C)
            s1 = sc.tile([P, T], F32, tag="s1")
            s2 = sc.tile([P, T], F32, tag="s2")
            for q8 in range(8):
                sl = slice(16 * q8, 16 * (q8 + 1))
                eng = nc.sync if q8 % 2 == 0 else nc.scalar
                eng.dma_start(out=x_tm[:, sl, :], in_=x_v[:, sl, :])
                nc.scalar.square(out=sqv[:, sl, :], in_=x_tm[:, sl, :])
                nc.vector.tensor_reduce(out=s1[:, sl], in_=x_tm[:, sl, :],
                                        axis=AX.X, op=OP.add)
                nc.vector.tensor_reduce(out=s2[:, sl], in_=sqv[:, sl, :],
                                        axis=AX.X, op=OP.add)
            g1, mg1 = _ln_finalize(nc, sc, s1, s2, epst, T)
            # warm up the PE so HAM is at 8/8 when real matmuls start
            for wd in range(15):
                pw = psT.tile([128, 128], F32, tag="tp", name="pw")
                nc.tensor.matmul(out=pw, lhsT=ident, rhs=ident,
                                 start=True, stop=True)
            a1tm = big.tile([P, T, C], BF16, tag="scr2")
            for q8 in range(8):
                sl = slice(16 * q8, 16 * (q8 + 1))
                nc.vector.tensor_tensor(
                    out=a1tm[:, sl, :], in0=x_tm[:, sl, :],
                    in1=g1[:, sl, None].broadcast_to([P, 16, C]), op=OP.mult)
                nc.vector.tensor_tensor(
                    out=a1tm[:, sl, :], in0=a1tm[:, sl, :],
                    in1=mg1[:, sl, None].broadcast_to([P, 16, C]),
                    op=OP.subtract)

            # transpose A1 to channel-major [64, N]: two tiles per transpose
            a1cm = big.tile([128, N], BF16, tag="acm")
            a1cm_v = a1cm[0:C, :].rearrange("c (j a b n) -> c j a b n", a=4, b=2, n=128)
            a1tm_v = a1tm.rearrange("p t c -> p (t c)")
            for j in range(16):
                pt = psT.tile([128, 4, 128], BF16, tag="tp")
                for k in range(4):
                    tt = 8 * j + 2 * k
                    nc.tensor.transpose(out=pt[:, k, :],
                                        in_=a1tm_v[:, 64 * tt:64 * (tt + 2)],
                                        identity=ident)
                nc.scalar.copy(out=a1cm_v[:, j, :, 0, :], in_=pt[0:C, :, :])
                nc.vector.tensor_copy(out=a1cm_v[:, j, :, 1, :],
                                      in_=pt[C:128, :, :])
                nc.vector.tensor_copy(
                    out=a1cm[C:128, 1024 * j:1024 * (j + 1) - 1],
                    in_=a1cm[0:C, 1024 * j + 1:1024 * (j + 1)])
                if j > 0:
                    nc.gpsimd.tensor_copy(
                        out=a1cm[C:128, 1024 * j - 1:1024 * j],
                        in_=a1cm[0:C, 1024 * j:1024 * j + 1])

            # ---- spatial reduction conv (8x8 stride 8) ----
            a1sr = a1cm.rearrange("c (Y ky X kx) -> c ky kx Y X", ky=SR, kx=SR, X=16)
            psr = psA.tile([128, 512], F32, tag="ps", name="ps").rearrange("c (a y x) -> c a y x", a=2, y=16)[0:C, 0, :, :]
            for pp in range(32):
                ky, kx = pp // 4, (pp % 4) * 2
                nc.tensor.matmul(out=psr, lhsT=wsr[:, pp, :],
                                 rhs=a1sr[:, ky, kx, :, :],
                                 start=(pp == 0), stop=(pp == 31))
            xrcm = consts.tile([C, NR], BF16)
            nc.scalar.activation(out=xrcm.rearrange("c (y x) -> c y x", x=16),
                                 in_=psr, func=AF.Identity,
                                 bias=bsr, scale=1.0)

            # ---- LN on reduced tokens (srn), token-major ----
            xr_tm = consts.tile([P, 2, C], F32)
            for hh in range(2):
                pv = psT.tile([128, C], BF16, tag="tp")
                nc.tensor.transpose(out=pv, in_=xrcm[:, 128 * hh:128 * (hh + 1)],
                                    identity=ident[0:C, 0:C])
                nc.vector.tensor_copy(out=xr_tm[:, hh, :], in_=pv)
            g_r, mg_r = _ln_stats(nc, sc, consts, xr_tm, epst, 2)
            ar_tm = consts.tile([P, 2, C], BF16)
            nc.vector.tensor_tensor(
                out=ar_tm, in0=xr_tm,
                in1=g_r[:, :, None].broadcast_to([P, 2, C]), op=OP.mult)
            mgb = sc.tile([P, 2, C], BF16, tag="mgb")
            nc.vector.tensor_tensor(
                out=mgb, in0=mg_r[:, :, None].broadcast_to([P, 2, C]),
                in1=g_r[:, :, None].broadcast_to([P, 2, C]), op=OP.bypass)
            nc.vector.tensor_tensor(out=ar_tm, in0=ar_tm, in1=mgb, op=OP.subtract)
            arcm = consts.tile([C, NR], BF16)
            for hh in range(2):
                pv = psT.tile([C, 128], BF16, tag="tp")
                nc.tensor.transpose(out=pv, in_=ar_tm[:, hh, :], identity=ident)
                nc.vector.tensor_copy(out=arcm[:, 128 * hh:128 * (hh + 1)], in_=pv)

            # ---- KV ----
            pkv = psA.tile([128, 512], F32, tag="ps", name="ps")[:, 0:NR]
            nc.tensor.matmul(out=pkv, lhsT=wkv, rhs=arcm, start=True, stop=True)
            kvcm = consts.tile([2 * C, NR], BF16)
            nc.scalar.activation(out=kvcm, in_=pkv, func=AF.Identity,
                                 bias=bkv, scale=1.0)
            # fold q-projection into K:  S^T = (K @ Wq) @ A1
            bqb = consts.tile([C, 1], BF16)
            nc.vector.tensor_copy(out=bqb, in_=bq)
            pkw = psT.tile([C, NR], F32, tag="tp", name="pkw")
            nc.tensor.matmul(out=pkw, lhsT=wq, rhs=kvcm[0:C, :],
                             start=True, stop=True)
            kwt = consts.tile([C, NR], BF16)
            nc.scalar.copy(out=kwt, in_=pkw)
            sbias = consts.tile([128, 2], F32)
            for hh in range(2):
                pb = psT.tile([128, 1], F32, tag="tp", name="pb")
                nc.tensor.matmul(out=pb,
                                 lhsT=kvcm[0:C, 128 * hh:128 * (hh + 1)],
                                 rhs=bqb, start=True, stop=True)
                nc.vector.tensor_copy(out=sbias[:, hh:hh + 1], in_=pb)
            vp = consts.tile([128, 2, C + 1], BF16)
            nc.vector.memset(vp[:, :, C:C + 1], 1.0)
            for hh in range(2):
                pv = psT.tile([128, C], BF16, tag="tp")
                nc.tensor.transpose(out=pv,
                                    in_=kvcm[C:2 * C, 128 * hh:128 * (hh + 1)],
                                    identity=ident[C:2 * C, C:2 * C])
                nc.vector.tensor_copy(out=vp[:, hh, 0:C], in_=pv)

            # ---- attention, streamed in 512-column chunks ----
            y_tm = big.tile([P, T, C], F32, tag="y")
            sq2 = big.tile([P, T * C], BF16, tag="scr2", name="sq2")
            sq2v = sq2.rearrange("p (t c) -> p t c", c=C)
            s1y = sc.tile([P, T], F32, tag="s1y")
            s2y = sc.tile([P, T], F32, tag="s2y")
            for i in range(32):
                ech = ch.tile([128, 2, 512], BF16, tag="e")
                for hh in range(2):
                    pS = psA.tile([128, 512], F32, tag="ps", name="ps")
                    nc.tensor.matmul(out=pS,
                                     lhsT=kwt[:, 128 * hh:128 * (hh + 1)],
                                     rhs=a1cm[0:C, 512 * i:512 * (i + 1)],
                                     start=True, stop=True)
                    nc.scalar.activation(out=ech[:, hh, :], in_=pS, func=AF.Exp,
                                         bias=sbias[:, hh:hh + 1], scale=1.0)
                pO = psA.tile([128, 512], F32, tag="ps", name="ps")[0:C + 1, :]
                for hh in range(2):
                    nc.tensor.matmul(out=pO, lhsT=vp[:, hh, :],
                                     rhs=ech[:, hh, :],
                                     start=(hh == 0), stop=(hh == 1))
                pod = ch.tile([C + 1, 512], BF16, tag="pod")
                nc.vector.tensor_copy(out=pod, in_=pO)
                ptr = psT.tile([128, 4, C + 1], F32, tag="tp")
                for k in range(4):
                    nc.tensor.matmul(out=ptr[:, k, :],
                                     lhsT=pod[:, 128 * k:128 * (k + 1)],
                                     rhs=wpj, start=True, stop=True)
                rt = sc.tile([P, 4, 1], F32, tag="rt")
                nc.vector.reciprocal(out=rt, in_=ptr[:, :, C:C + 1])
                tmp = ch.tile([P, 4, C], F32, tag="tmp")
                nc.vector.tensor_tensor(out=tmp, in0=ptr[:, :, 0:C],
                                        in1=rt.broadcast_to([P, 4, C]),
                                        op=OP.mult)
                nc.vector.tensor_tensor(out=y_tm[:, 4 * i:4 * (i + 1), :],
                                        in0=tmp, in1=x_tm[:, 4 * i:4 * (i + 1), :],
                                        op=OP.add)
                if i % 4 == 3:
                    sl = slice(16 * (i // 4), 16 * (i // 4 + 1))
                    nc.scalar.square(out=sq2v[:, sl, :], in_=y_tm[:, sl, :])
                    nc.vector.tensor_reduce(out=s1y[:, sl], in_=y_tm[:, sl, :],
                                            axis=AX.X, op=OP.add)
                    nc.vector.tensor_reduce(out=s2y[:, sl], in_=sq2v[:, sl, :],
                                            axis=AX.X, op=OP.add)

            # ---- LN2 ----
            g2, mg2 = _ln_finalize(nc, sc, s1y, s2y, epst, T)
            a2tm = big.tile([P, T, C], BF16, tag="scr2")
            for q8 in range(8):
                sl = slice(16 * q8, 16 * (q8 + 1))
                nc.vector.tensor_tensor(
                    out=a2tm[:, sl, :], in0=y_tm[:, sl, :],
                    in1=g2[:, sl, None].broadcast_to([P, 16, C]), op=OP.mult)
                nc.vector.tensor_tensor(
                    out=a2tm[:, sl, :], in0=a2tm[:, sl, :],
                    in1=mg2[:, sl, None].broadcast_to([P, 16, C]),
                    op=OP.subtract)
            # re-warm PE after the LN2 lull
            for wd in range(8):
                pw = psT.tile([128, 128], F32, tag="tp", name="pw")
                nc.tensor.matmul(out=pw, lhsT=ident, rhs=ident,
                                 start=True, stop=True)

            # ---- A2 guarded channel-major, doubled: rows 64:128 shifted by +1 ----
            a2g = big.tile([128, NG], BF16, tag="acm")
            nc.vector.memset(a2g[:, 0:PAD + RP], 0.0)
            nc.vector.memset(a2g[:, NG - PAD - RP:NG], 0.0)
            a2rows = a2g[0:C, PAD + RP:PAD + RP * (H + 1)].rearrange(
                "c (y w) -> c y w", w=RP)
            a2rowsB = a2g[C:128, PAD + RP:PAD + RP * (H + 1)].rearrange(
                "c (y w) -> c y w", w=RP)
            nc.vector.memset(a2rows[:, :, 0:1], 0.0)
            nc.vector.memset(a2rows[:, :, RP - 1:RP], 0.0)
            nc.vector.memset(a2rowsB[:, :, RP - 2:RP], 0.0)
            a2tm_v = a2tm.rearrange("p t c -> p (t c)")
            for j in range(16):
                pt = psT.tile([128, 4, 128], BF16, tag="tp")
                for k in range(4):
                    tt = 8 * j + 2 * k
                    nc.tensor.transpose(out=pt[:, k, :],
                                        in_=a2tm_v[:, 64 * tt:64 * (tt + 2)],
                                        identity=ident)
                ro = a2rows.rearrange("c (j a b) w -> c j a b w", a=4, b=2)
                nc.scalar.copy(out=ro[:, j, :, 0, 1:W + 1], in_=pt[0:C, :, :])
                nc.vector.tensor_copy(out=ro[:, j, :, 1, 1:W + 1],
                                      in_=pt[C:128, :, :])
                nc.vector.tensor_copy(out=a2rowsB[:, 8 * j:8 * (j + 1), 0:W],
                                      in_=a2rows[:, 8 * j:8 * (j + 1), 1:W + 1])

            # ---- MLP: fused fc1 (+) 3x3 depthwise conv, gelu, fc2 ----
            o2cm = big.tile([C, NG], BF16, tag="qt")
            n_mlp = 33
            for j in range(n_mlp):
                cb = PAD + RP + 512 * j
                size = min(512, PAD + RP * (H + 1) - cb)
                gch = []
                for g in range(2):
                    pG = psA.tile([128, 512], F32, tag="ps", name="ps")
                    for dy in (-1, 0, 1):
                        nc.tensor.matmul(
                            out=pG[:, 0:size], lhsT=wmp[:, 2 * (dy + 1) + g, :],
                            rhs=a2g[:, cb + RP * dy - 1:cb + RP * dy - 1 + size],
                            start=(dy == -1), stop=False)
                    for dy in (-1, 0, 1):
                        nc.tensor.matmul(
                            out=pG[:, 0:size], lhsT=wms[:, 2 * (dy + 1) + g, :],
                            rhs=a2g[0:C, cb + RP * dy + 1:cb + RP * dy + 1 + size],
                            start=False, stop=(dy == 1))
                    gc = ch.tile([128, 512], BF16, tag=f"gc{g}")
                    nc.scalar.activation(out=gc[:, 0:size], in_=pG[:, 0:size],
                                         func=AF.Gelu, bias=bg[:, g:g + 1],
                                         scale=1.0)
                    gch.append(gc)
                pF = psA.tile([128, 512], F32, tag="ps", name="ps")
                for g in range(2):
                    nc.tensor.matmul(out=pF[0:C, 0:size], lhsT=wf2[:, g, :],
                                     rhs=gch[g][:, 0:size],
                                     start=(g == 0), stop=(g == 1))
                nc.vector.tensor_scalar(out=o2cm[:, cb:cb + size],
                                        in0=pF[0:C, 0:size], scalar1=bf2,
                                        scalar2=None, op0=OP.add)

            # ---- MLP epilogue: transpose back, residual, store ----
            y2_tm = big.tile([P, T, C], F32, tag="xr")  # reuses x_tm slot
            out_v = out_d.rearrange("(t p) c -> p t c", p=P)
            for j in range(32):
                pt2 = psT.tile([128, 4, C], BF16, tag="tp")
                for k in range(4):
                    t = 4 * j + k
                    s = PAD + RP * (t + 1) + 1
                    nc.tensor.transpose(out=pt2[:, k, :],
                                        in_=o2cm[:, s:s + W],
                                        identity=ident[0:C, 0:C])
                nc.vector.tensor_tensor(out=y2_tm[:, 4 * j:4 * (j + 1), :],
                                        in0=pt2, in1=y_tm[:, 4 * j:4 * (j + 1), :],
                                        op=OP.add)
                if j % 4 == 3:
                    q8 = j // 4
                    nc.sync.dma_start(out=out_v[:, 16 * q8:16 * (q8 + 1), :],
                                       in_=y2_tm[:, 16 * q8:16 * (q8 + 1), :])

    _split_excess_waits(nc)
    return nc


@functools.cache
def _get_nc():
    return _build_nc()


def _prep_weights(inp):
    f = lambda v: np.asarray(v, np.float32)
    n1w, n1b = f(inp["n1_w"]), f(inp["n1_b"])
    q_w, q_b = f(inp["q_w"]), f(inp["q_b"])
    kv_w, kv_b = f(inp["kv_w"]), f(inp["kv_b"])
    sr_w, sr_b = f(inp["sr_w"]), f(inp["sr_b"])
    srnw, srnb = f(inp["srn_w"]), f(inp["srn_b"])
    pj_w, pj_b = f(inp["proj_w"]), f(inp["proj_b"])
    n2w, n2b = f(inp["n2_w"]), f(inp["n2_b"])
    f1w, f1b = f(inp["fc1_w"]), f(inp["fc1_b"])
    dww, dwb = f(inp["dw_w"]), f(inp["dw_b"])
    f2w, f2b = f(inp["fc2_w"]), f(inp["fc2_b"])

    scale = (C // 1) ** -0.5
    wq_l = (q_w * n1w[None, :]).T * scale
    bq_l = ((q_w @ n1b + q_b) * scale)[:, None]

    wsr_l = np.zeros((32, 128, C), np.float32)
    for pp in range(32):
        ky, kx = pp // 4, (pp % 4) * 2
        wsr_l[pp, :C, :] = (sr_w[:, :, ky, kx] * n1w[None, :]).T
        wsr_l[pp, C:, :] = (sr_w[:, :, ky, kx + 1] * n1w[None, :]).T
    wsr_l = wsr_l.transpose(1, 0, 2)
    bsr_l = (sr_w.sum((2, 3)) @ n1b + sr_b)[:, None]

    wkv_l = (kv_w * srnw[None, :]).T
    bkv_l = (kv_w @ srnb + kv_b)[:, None]

    wpj_l = np.zeros((C + 1, C + 1), np.float32)
    wpj_l[:C, :C] = pj_w.T
    wpj_l[C, :C] = pj_b
    wpj_l[C, C] = 1.0

    k9 = dww[:, 0, :, :].reshape(HID, 9)          # [256, 9]
    wmp_l = np.zeros((6, 128, 128), np.float32)
    wms_l = np.zeros((6, C, 128), np.float32)
    for dy in range(3):
        for g in range(2):
            Ma = (k9[:, dy * 3 + 0][:, None] * f1w * n2w[None, :])[128 * g:128 * (g + 1)]
            Mb = (k9[:, dy * 3 + 1][:, None] * f1w * n2w[None, :])[128 * g:128 * (g + 1)]
            Mc = (k9[:, dy * 3 + 2][:, None] * f1w * n2w[None, :])[128 * g:128 * (g + 1)]
            wmp_l[2 * dy + g, :C, :] = Ma.T
            wmp_l[2 * dy + g, C:, :] = Mb.T
            wms_l[2 * dy + g, :, :] = Mc.T
    wmp_l = wmp_l.transpose(1, 0, 2)
    wms_l = wms_l.transpose(1, 0, 2)
    bg_full = k9.sum(1) * (f1w @ n2b + f1b) + dwb  # [256]
    bg_l = np.ascontiguousarray(bg_full.reshape(2, 128).T)

    wf2_l = np.stack([f2w[:, :128].T, f2w[:, 128:].T], 0).transpose(1, 0, 2)
    bf2_l = f2b[:, None]

    bfc = lambda a: np.ascontiguousarray(a).astype(BF)
    return {
        "wq": bfc(wq_l), "bq": np.ascontiguousarray(bq_l),
        "wsr": bfc(wsr_l), "bsr": np.ascontiguousarray(bsr_l),
        "wkv": bfc(wkv_l), "bkv": np.ascontiguousarray(bkv_l),
        "wpj": bfc(wpj_l),
        "wmp": bfc(wmp_l), "wms": bfc(wms_l),
        "bg": np.ascontiguousarray(bg_l),
        "wf2": bfc(wf2_l), "bf2": np.ascontiguousarray(bf2_l),
    }


def kernel(trace=False, tmpdir=None, **inputs):
    nc = _get_nc()
    x = np.asarray(inputs["x"], np.float32)
    wts = _prep_weights(inputs)
    in_maps = [dict(wts, x=np.ascontiguousarray(x[b])) for b in range(B)]
    res = run_bass_kernel_spmd(nc, in_maps, core_ids=list(range(8)),
                               trace=trace, tmpdir=tmpdir)
    out = np.stack([res.results[b]["out"] for b in range(B)], 0)
    kernel.last_exec_time_ns = res.exec_time_ns
    return out


# revision 28
# speedup vs baseline: 1.3750x; 1.0039x over previous
"""Trainium2 Bass kernel for nn_Block_523986010339 (PVT-style transformer block).

Sharding: data-parallel over batch B=8 -> one batch element per NeuronCore.
Per-core layouts:
  - residual stream token-major fp32 [128p=token%128, 128t=token//128, 64c]
  - matmul operands channel-major bf16 [c, n], n = 128*y + x
  - LN mean folded into matmul weights via an extra "m*g" row; rsqrt scale
    applied token-major with broadcast APs
  - attention: S^T channel-major, exp without max-subtraction (tiny logits),
    denominator via fused ones-column in the V matmul, divided out after proj
  - MLP: fc1 and 3x3 depthwise conv fused into 9 accumulated matmuls over a
    zero-guarded channel-major layout (row pitch 130)
"""

import functools
import json

import numpy as np
import ml_dtypes

import concourse.bass as bass
import concourse.mybir as mybir
import concourse.tile as tile
from concourse.bass_utils import run_bass_kernel_spmd
from concourse.masks import make_identity

F32 = mybir.dt.float32
BF16 = mybir.dt.bfloat16
BF = ml_dtypes.bfloat16

B, N, C, H, W = 8, 16384, 64, 128, 128
SR, HID, NR = 8, 256, 256
P, T = 128, 128
RP = W + 2          # guarded row pitch
PAD = RP + 1        # head/tail pad so all tap offsets stay in-bounds
NG = PAD + RP * (H + 2) + PAD
AX = mybir.AxisListType
OP = mybir.AluOpType
AF = mybir.ActivationFunctionType


def _split_excess_waits(nc, max_waits=1):
    """walrus in this container rejects >1 sync wait per instruction; move
    excess waits onto injected Drain instructions just before the owner."""
    d = json.loads(mybir.module_to_json_string(nc.m))
    n_split = [0]

    def fix(insts):
        out = []
        for inst in insts:
            si = inst.get("sync_info") or {}
            waits = si.get("on_wait") or []
            if len(waits) > max_waits:
                extra = waits[:-max_waits]
                for i in range(0, len(extra), max_waits):
                    n_split[0] += 1
                    out.append({
                        "name": f"WSPLIT-{n_split[0]}",
                        "opcode": "NoOp",
                        "engine": inst["engine"],
                        "ins": [],
                        "outs": [],
                        "is_reset_sema": False,
                        "sync_info": {"on_update": [],
                                      "on_wait": extra[i:i + max_waits]},
                    })
                si["on_wait"] = waits[-max_waits:]
                inst["sync_info"] = si
            out.append(inst)
        return out

    for f in d.get("functions", []):
        for bb in f.get("blocks", []):
            bb["instructions"] = fix(bb["instructions"])
    nc.m = mybir.module_from_json_string(json.dumps(d))


def _ln_stats(nc, sc, big, x_tm, epst, nt):
    """Token-major LN stats: returns (g, mg) tiles [128, nt] fp32 given
    x_tm [128, nt, 64] fp32."""
    sq_scr = big.tile([P, nt * C], BF16, tag="scr2", name="sq")
    xsq_view = sq_scr.rearrange("p (t c) -> p t c", c=C)
    nc.scalar.square(out=sq_scr, in_=x_tm.rearrange("p t c -> p (t c)"))
    s1 = sc.tile([P, nt], F32, tag=f"s1_{nt}")
    s2 = sc.tile([P, nt], F32, tag=f"s2_{nt}")
    nc.vector.tensor_reduce(out=s1, in_=x_tm, axis=AX.X, op=OP.add)
    nc.vector.tensor_reduce(out=s2, in_=xsq_view, axis=AX.X, op=OP.add)
    return _ln_finalize(nc, sc, s1, s2, epst, nt)


def _ln_finalize(nc, sc, s1, s2, epst, nt):
    mean = sc.tile([P, nt], F32, tag=f"mean_{nt}")
    var = sc.tile([P, nt], F32, tag=f"var_{nt}")
    nc.vector.tensor_scalar_mul(out=mean, in0=s1, scalar1=1.0 / C)
    nc.vector.tensor_scalar_mul(out=var, in0=s2, scalar1=1.0 / C)
    mm = sc.tile([P, nt], F32, tag=f"mm_{nt}")
    nc.vector.tensor_tensor(out=mm, in0=mean, in1=mean, op=OP.mult)
    nc.vector.tensor_tensor(out=var, in0=var, in1=mm, op=OP.subtract)
    sd = sc.tile([P, nt], F32, tag=f"sd_{nt}")
    nc.scalar.activation(out=sd, in_=var, func=AF.Sqrt, bias=epst, scale=1.0)
    g = sc.tile([P, nt], F32, tag=f"g_{nt}")
    nc.vector.reciprocal(out=g, in_=sd)
    mg = sc.tile([P, nt], F32, tag=f"mg_{nt}")
    nc.vector.tensor_tensor(out=mg, in0=mean, in1=g, op=OP.mult)
    return g, mg


def _build_nc():
    nc = bass.Bass("TRN2")
    x_d = nc.dram_tensor("x", [N, C], F32, kind="ExternalInput")
    out_d = nc.dram_tensor("out", [N, C], F32, kind="ExternalOutput")
    wq_d = nc.dram_tensor("wq", [C, C], BF16, kind="ExternalInput")
    bq_d = nc.dram_tensor("bq", [C, 1], F32, kind="ExternalInput")
    wsr_d = nc.dram_tensor("wsr", [128, 32, C], BF16, kind="ExternalInput")
    bsr_d = nc.dram_tensor("bsr", [C, 1], F32, kind="ExternalInput")
    wkv_d = nc.dram_tensor("wkv", [C, 2 * C], BF16, kind="ExternalInput")
    bkv_d = nc.dram_tensor("bkv", [2 * C, 1], F32, kind="ExternalInput")
    wpj_d = nc.dram_tensor("wpj", [C + 1, C + 1], BF16, kind="ExternalInput")
    wmp_d = nc.dram_tensor("wmp", [128, 6, 128], BF16, kind="ExternalInput")
    wms_d = nc.dram_tensor("wms", [C, 6, 128], BF16, kind="ExternalInput")
    bg_d = nc.dram_tensor("bg", [128, 2], F32, kind="ExternalInput")
    wf2_d = nc.dram_tensor("wf2", [128, 2, C], BF16, kind="ExternalInput")
    bf2_d = nc.dram_tensor("bf2", [C, 1], F32, kind="ExternalInput")

    with tile.TileContext(nc) as tc:
        with (
            tc.tile_pool(name="consts", bufs=1) as consts,
            tc.tile_pool(name="big", bufs=1) as big,
            tc.tile_pool(name="sc", bufs=2) as sc,
            tc.tile_pool(name="ch", bufs=3) as ch,
            tc.tile_pool(name="psA", bufs=6, space="PSUM") as psA,
            tc.tile_pool(name="psT", bufs=2, space="PSUM") as psT,
        ):
            ident = consts.tile([128, 128], BF16)
            make_identity(nc, ident)
            wq = consts.tile([C, C], BF16)
            nc.gpsimd.dma_start(out=wq, in_=wq_d[:, :])
            wsr = consts.tile([128, 32, C], BF16)
            nc.gpsimd.dma_start(out=wsr, in_=wsr_d[:, :, :])
            wkv = consts.tile([C, 2 * C], BF16)
            nc.gpsimd.dma_start(out=wkv, in_=wkv_d[:, :])
            wpj = consts.tile([C + 1, C + 1], BF16)
            nc.gpsimd.dma_start(out=wpj, in_=wpj_d[:, :])
            wmp = consts.tile([128, 6, 128], BF16)
            nc.gpsimd.dma_start(out=wmp, in_=wmp_d[:, :, :])
            wms = consts.tile([C, 6, 128], BF16)
            nc.gpsimd.dma_start(out=wms, in_=wms_d[:, :, :])
            wf2 = consts.tile([128, 2, C], BF16)
            nc.gpsimd.dma_start(out=wf2, in_=wf2_d[:, :, :])
            bq = consts.tile([C, 1], F32)
            nc.gpsimd.dma_start(out=bq, in_=bq_d[:, :])
            bsr = consts.tile([C, 1], F32)
            nc.gpsimd.dma_start(out=bsr, in_=bsr_d[:, :])
            bkv = consts.tile([2 * C, 1], F32)
            nc.gpsimd.dma_start(out=bkv, in_=bkv_d[:, :])
            bg = consts.tile([128, 2], F32)
            nc.gpsimd.dma_start(out=bg, in_=bg_d[:, :])
            bf2 = consts.tile([C, 1], F32)
            nc.gpsimd.dma_start(out=bf2, in_=bf2_d[:, :])
            epst = consts.tile([P, 1], F32)
            nc.vector.memset(epst, 1e-5)

            # ---- load x (token-major), LN1 stats overlapped per slice ----
            x_tm = big.tile([P, T, C], F32, tag="xr")
            x_v = x_d.rearrange("(t p) c -> p t c", p=P)
            sq_scr = big.tile([P, T * C], BF16, tag="scr2", name="sq")
            sqv = sq_scr.rearrange("p (t c) -> p t c", c=C)
            s1 = sc.tile([P, T], F32, tag="s1")
            s2 = sc.tile([P, T], F32, tag="s2")
            for q8 in range(8):
                sl = slice(16 * q8, 16 * (q8 + 1))
                eng = nc.sync if q8 % 2 == 0 else nc.scalar
                eng.dma_start(out=x_tm[:, sl, :], in_=x_v[:, sl, :])
                nc.scalar.square(out=sqv[:, sl, :], in_=x_tm[:, sl, :])
                nc.vector.tensor_reduce(out=s1[:, sl], in_=x_tm[:, sl, :],
                                        axis=AX.X, op=OP.add)
                nc.vector.tensor_reduce(out=s2[:, sl], in_=sqv[:, sl, :],
                                        axis=AX.X, op=OP.add)
            g1, mg1 = _ln_finalize(nc, sc, s1, s2, epst, T)
            # warm up the PE so HAM is at 8/8 when real matmuls start
            for wd in range(15):
                pw = psT.tile([128, 128], F32, tag="tp", name="pw")
                nc.tensor.matmul(out=pw, lhsT=ident, rhs=ident,
                                 start=True, stop=True)
            a1tm = big.tile([P, T, C], BF16, tag="scr2")
            a1cm = big.tile([128, N], BF16, tag="acm")
            a1cm_v = a1cm[0:C, :].rearrange("c (j a b n) -> c j a b n", a=4, b=2, n=128)
            a1tm_v = a1tm.rearrange("p t c -> p (t c)")
            for q8 in range(8):
                sl = slice(16 * q8, 16 * (q8 + 1))
                nc.vector.tensor_tensor(
                    out=a1tm[:, sl, :], in0=x_tm[:, sl, :],
                    in1=g1[:, sl, None].broadcast_to([P, 16, C]), op=OP.mult)
                nc.vector.tensor_tensor(
                    out=a1tm[:, sl, :], in0=a1tm[:, sl, :],
                    in1=mg1[:, sl, None].broadcast_to([P, 16, C]),
                    op=OP.subtract)
                for j in (2 * q8, 2 * q8 + 1):
                    pt = psT.tile([128, 4, 128], BF16, tag="tp")
                    for k in range(4):
                        tt = 8 * j + 2 * k
                        nc.tensor.transpose(out=pt[:, k, :],
                                            in_=a1tm_v[:, 64 * tt:64 * (tt + 2)],
                                            identity=ident)
                    nc.scalar.copy(out=a1cm_v[:, j, :, 0, :], in_=pt[0:C, :, :])
                    nc.vector.tensor_copy(out=a1cm_v[:, j, :, 1, :],
                                          in_=pt[C:128, :, :])
                    nc.vector.tensor_copy(
                        out=a1cm[C:128, 1024 * j:1024 * (j + 1) - 1],
                        in_=a1cm[0:C, 1024 * j + 1:1024 * (j + 1)])
                    if j > 0:
                        nc.gpsimd.tensor_copy(
                            out=a1cm[C:128, 1024 * j - 1:1024 * j],
                            in_=a1cm[0:C, 1024 * j:1024 * j + 1])

            # ---- spatial reduction conv (8x8 stride 8) ----
            a1sr = a1cm.rearrange("c (Y ky X kx) -> c ky kx Y X", ky=SR, kx=SR, X=16)
            psr = psA.tile([128, 512], F32, tag="ps", name="ps").rearrange("c (a y x) -> c a y x", a=2, y=16)[0:C, 0, :, :]
            for pp in range(32):
                ky, kx = pp // 4, (pp % 4) * 2
                nc.tensor.matmul(out=psr, lhsT=wsr[:, pp, :],
                                 rhs=a1sr[:, ky, kx, :, :],
                                 start=(pp == 0), stop=(pp == 31))
            xrcm = consts.tile([C, NR], BF16)
            nc.scalar.activation(out=xrcm.rearrange("c (y x) -> c y x", x=16),
                                 in_=psr, func=AF.Identity,
                                 bias=bsr, scale=1.0)

            # ---- LN on reduced tokens (srn), token-major ----
            xr_tm = consts.tile([P, 2, C], F32)
            for hh in range(2):
                pv = psT.tile([128, C], BF16, tag="tp")
                nc.tensor.transpose(out=pv, in_=xrcm[:, 128 * hh:128 * (hh + 1)],
                                    identity=ident[0:C, 0:C])
                nc.vector.tensor_copy(out=xr_tm[:, hh, :], in_=pv)
            g_r, mg_r = _ln_stats(nc, sc, consts, xr_tm, epst, 2)
            ar_tm = consts.tile([P, 2, C], BF16)
            nc.vector.tensor_tensor(
                out=ar_tm, in0=xr_tm,
                in1=g_r[:, :, None].broadcast_to([P, 2, C]), op=OP.mult)
            mgb = sc.tile([P, 2, C], BF16, tag="mgb")
            nc.vector.tensor_tensor(
                out=mgb, in0=mg_r[:, :, None].broadcast_to([P, 2, C]),
                in1=g_r[:, :, None].broadcast_to([P, 2, C]), op=OP.bypass)
            nc.vector.tensor_tensor(out=ar_tm, in0=ar_tm, in1=mgb, op=OP.subtract)
            arcm = consts.tile([C, NR], BF16)
            for hh in range(2):
                pv = psT.tile([C, 128], BF16, tag="tp")
                nc.tensor.transpose(out=pv, in_=ar_tm[:, hh, :], identity=ident)
                nc.vector.tensor_copy(out=arcm[:, 128 * hh:128 * (hh + 1)], in_=pv)

            # ---- KV ----
            pkv = psA.tile([128, 512], F32, tag="ps", name="ps")[:, 0:NR]
            nc.tensor.matmul(out=pkv, lhsT=wkv, rhs=arcm, start=True, stop=True)
            kvcm = consts.tile([2 * C, NR], BF16)
            nc.scalar.activation(out=kvcm, in_=pkv, func=AF.Identity,
                                 bias=bkv, scale=1.0)
            # fold q-projection into K:  S^T = (K @ Wq) @ A1
            bqb = consts.tile([C, 1], BF16)
            nc.vector.tensor_copy(out=bqb, in_=bq)
            pkw = psT.tile([C, NR], F32, tag="tp", name="pkw")
            nc.tensor.matmul(out=pkw, lhsT=wq, rhs=kvcm[0:C, :],
                             start=True, stop=True)
            kwt = consts.tile([C, NR], BF16)
            nc.scalar.copy(out=kwt, in_=pkw)
            sbias = consts.tile([128, 2], F32)
            for hh in range(2):
                pb = psT.tile([128, 1], F32, tag="tp", name="pb")
                nc.tensor.matmul(out=pb,
                                 lhsT=kvcm[0:C, 128 * hh:128 * (hh + 1)],
                                 rhs=bqb, start=True, stop=True)
                nc.vector.tensor_copy(out=sbias[:, hh:hh + 1], in_=pb)
            vp = consts.tile([128, 2, C + 1], BF16)
            nc.vector.memset(vp[:, :, C:C + 1], 1.0)
            for hh in range(2):
                pv = psT.tile([128, C], BF16, tag="tp")
                nc.tensor.transpose(out=pv,
                                    in_=kvcm[C:2 * C, 128 * hh:128 * (hh + 1)],
                                    identity=ident[C:2 * C, C:2 * C])
                nc.vector.tensor_copy(out=vp[:, hh, 0:C], in_=pv)

            # ---- attention, streamed in 512-column chunks ----
            y_tm = big.tile([P, T, C], F32, tag="y")
            sq2 = big.tile([P, T * C], BF16, tag="scr2", name="sq2")
            sq2v = sq2.rearrange("p (t c) -> p t c", c=C)
            s1y = sc.tile([P, T], F32, tag="s1y")
            s2y = sc.tile([P, T], F32, tag="s2y")
            for i in range(32):
                ech = ch.tile([128, 2, 512], BF16, tag="e")
                for hh in range(2):
                    pS = psA.tile([128, 512], F32, tag="ps", name="ps")
                    nc.tensor.matmul(out=pS,
                                     lhsT=kwt[:, 128 * hh:128 * (hh + 1)],
                                     rhs=a1cm[0:C, 512 * i:512 * (i + 1)],
                                     start=True, stop=True)
                    nc.scalar.activation(out=ech[:, hh, :], in_=pS, func=AF.Exp,
                                         bias=sbias[:, hh:hh + 1], scale=1.0)
                pO = psA.tile([128, 512], F32, tag="ps", name="ps")[0:C + 1, :]
                for hh in range(2):
                    nc.tensor.matmul(out=pO, lhsT=vp[:, hh, :],
                                     rhs=ech[:, hh, :],
                                     start=(hh == 0), stop=(hh == 1))
                pod = ch.tile([C + 1, 512], BF16, tag="pod")
                nc.vector.tensor_copy(out=pod, in_=pO)
                ptr = psT.tile([128, 4, C + 1], F32, tag="tp")
                for k in range(4):
                    nc.tensor.matmul(out=ptr[:, k, :],
                                     lhsT=pod[:, 128 * k:128 * (k + 1)],
                                     rhs=wpj, start=True, stop=True)
                rt = sc.tile([P, 4, 1], F32, tag="rt")
                nc.vector.reciprocal(out=rt, in_=ptr[:, :, C:C + 1])
                tmp = ch.tile([P, 4, C], F32, tag="tmp")
                nc.vector.tensor_tensor(out=tmp, in0=ptr[:, :, 0:C],
                                        in1=rt.broadcast_to([P, 4, C]),
                                        op=OP.mult)
                nc.vector.tensor_tensor(out=y_tm[:, 4 * i:4 * (i + 1), :],
                                        in0=tmp, in1=x_tm[:, 4 * i:4 * (i + 1), :],
                                        op=OP.add)
                if i % 4 == 3:
                    sl = slice(16 * (i // 4), 16 * (i // 4 + 1))
                    nc.scalar.square(out=sq2v[:, sl, :], in_=y_tm[:, sl, :])
                    nc.vector.tensor_reduce(out=s1y[:, sl], in_=y_tm[:, sl, :],
                                            axis=AX.X, op=OP.add)
                    nc.vector.tensor_reduce(out=s2y[:, sl], in_=sq2v[:, sl, :],
                                            axis=AX.X, op=OP.add)

            # ---- LN2 ----
            g2, mg2 = _ln_finalize(nc, sc, s1y, s2y, epst, T)
            a2tm = big.tile([P, T, C], BF16, tag="scr2")
            a2norm_done = []
            for q8 in range(8):
                sl = slice(16 * q8, 16 * (q8 + 1))
                nc.vector.tensor_tensor(
                    out=a2tm[:, sl, :], in0=y_tm[:, sl, :],
                    in1=g2[:, sl, None].broadcast_to([P, 16, C]), op=OP.mult)
                nc.vector.tensor_tensor(
                    out=a2tm[:, sl, :], in0=a2tm[:, sl, :],
                    in1=mg2[:, sl, None].broadcast_to([P, 16, C]),
                    op=OP.subtract)
            # re-warm PE after the LN2 lull
            for wd in range(8):
                pw = psT.tile([128, 128], F32, tag="tp", name="pw")
                nc.tensor.matmul(out=pw, lhsT=ident, rhs=ident,
                                 start=True, stop=True)

            # ---- A2 guarded channel-major, doubled: rows 64:128 shifted by +1 ----
            a2g = big.tile([128, NG], BF16, tag="acm")
            nc.vector.memset(a2g[:, 0:PAD + RP], 0.0)
            nc.vector.memset(a2g[:, NG - PAD - RP:NG], 0.0)
            a2rows = a2g[0:C, PAD + RP:PAD + RP * (H + 1)].rearrange(
                "c (y w) -> c y w", w=RP)
            a2rowsB = a2g[C:128, PAD + RP:PAD + RP * (H + 1)].rearrange(
                "c (y w) -> c y w", w=RP)
            nc.vector.memset(a2rows[:, :, 0:1], 0.0)
            nc.vector.memset(a2rows[:, :, RP - 1:RP], 0.0)
            nc.vector.memset(a2rowsB[:, :, RP - 2:RP], 0.0)
            a2tm_v = a2tm.rearrange("p t c -> p (t c)")
            for j in range(16):
                pt = psT.tile([128, 4, 128], BF16, tag="tp")
                for k in range(4):
                    tt = 8 * j + 2 * k
                    nc.tensor.transpose(out=pt[:, k, :],
                                        in_=a2tm_v[:, 64 * tt:64 * (tt + 2)],
                                        identity=ident)
                ro = a2rows.rearrange("c (j a b) w -> c j a b w", a=4, b=2)
                nc.scalar.copy(out=ro[:, j, :, 0, 1:W + 1], in_=pt[0:C, :, :])
                nc.vector.tensor_copy(out=ro[:, j, :, 1, 1:W + 1],
                                      in_=pt[C:128, :, :])
                nc.vector.tensor_copy(out=a2rowsB[:, 8 * j:8 * (j + 1), 0:W],
                                      in_=a2rows[:, 8 * j:8 * (j + 1), 1:W + 1])

            # ---- MLP: fused fc1 (+) 3x3 depthwise conv, gelu, fc2 ----
            o2cm = big.tile([C, NG], BF16, tag="qt")
            n_mlp = 33
            for j in range(n_mlp):
                cb = PAD + RP + 512 * j
                size = min(512, PAD + RP * (H + 1) - cb)
                gch = []
                for g in range(2):
                    pG = psA.tile([128, 512], F32, tag="ps", name="ps")
                    for dy in (-1, 0, 1):
                        nc.tensor.matmul(
                            out=pG[:, 0:size], lhsT=wmp[:, 2 * (dy + 1) + g, :],
                            rhs=a2g[:, cb + RP * dy - 1:cb + RP * dy - 1 + size],
                            start=(dy == -1), stop=False)
                    for dy in (-1, 0, 1):
                        nc.tensor.matmul(
                            out=pG[:, 0:size], lhsT=wms[:, 2 * (dy + 1) + g, :],
                            rhs=a2g[0:C, cb + RP * dy + 1:cb + RP * dy + 1 + size],
                            start=False, stop=(dy == 1))
                    gc = ch.tile([128, 512], BF16, tag=f"gc{g}")
                    nc.scalar.activation(out=gc[:, 0:size], in_=pG[:, 0:size],
                                         func=AF.Gelu, bias=bg[:, g:g + 1],
                                         scale=1.0)
                    gch.append(gc)
                pF = psA.tile([128, 512], F32, tag="ps", name="ps")
                for g in range(2):
                    nc.tensor.matmul(out=pF[0:C, 0:size], lhsT=wf2[:, g, :],
                                     rhs=gch[g][:, 0:size],
                                     start=(g == 0), stop=(g == 1))
                nc.vector.tensor_scalar(out=o2cm[:, cb:cb + size],
                                        in0=pF[0:C, 0:size], scalar1=bf2,
                                        scalar2=None, op0=OP.add)

            # ---- MLP epilogue: transpose back, residual, store ----
            y2_tm = big.tile([P, T, C], F32, tag="xr")  # reuses x_tm slot
            out_v = out_d.rearrange("(t p) c -> p t c", p=P)
            for j in range(32):
                pt2 = psT.tile([128, 4, C], BF16, tag="tp")
                for k in range(4):
                    t = 4 * j + k
                    s = PAD + RP * (t + 1) + 1
                    nc.tensor.transpose(out=pt2[:, k, :],
                                        in_=o2cm[:, s:s + W],
                                        identity=ident[0:C, 0:C])
                nc.vector.tensor_tensor(out=y2_tm[:, 4 * j:4 * (j + 1), :],
                                        in0=pt2, in1=y_tm[:, 4 * j:4 * (j + 1), :],
                                        op=OP.add)
                if j % 4 == 3:
                    q8 = j // 4
                    nc.sync.dma_start(out=out_v[:, 16 * q8:16 * (q8 + 1), :],
                                       in_=y2_tm[:, 16 * q8:16 * (q8 + 1), :])

    _split_excess_waits(nc)
    return nc


@functools.cache
def _get_nc():
    return _build_nc()


def _prep_weights(inp):
    f = lambda v: np.asarray(v, np.float32)
    n1w, n1b = f(inp["n1_w"]), f(inp["n1_b"])
    q_w, q_b = f(inp["q_w"]), f(inp["q_b"])
    kv_w, kv_b = f(inp["kv_w"]), f(inp["kv_b"])
    sr_w, sr_b = f(inp["sr_w"]), f(inp["sr_b"])
    srnw, srnb = f(inp["srn_w"]), f(inp["srn_b"])
    pj_w, pj_b = f(inp["proj_w"]), f(inp["proj_b"])
    n2w, n2b = f(inp["n2_w"]), f(inp["n2_b"])
    f1w, f1b = f(inp["fc1_w"]), f(inp["fc1_b"])
    dww, dwb = f(inp["dw_w"]), f(inp["dw_b"])
    f2w, f2b = f(inp["fc2_w"]), f(inp["fc2_b"])

    scale = (C // 1) ** -0.5
    wq_l = (q_w * n1w[None, :]).T * scale
    bq_l = ((q_w @ n1b + q_b) * scale)[:, None]

    wsr_l = np.zeros((32, 128, C), np.float32)
    for pp in range(32):
        ky, kx = pp // 4, (pp % 4) * 2
        wsr_l[pp, :C, :] = (sr_w[:, :, ky, kx] * n1w[None, :]).T
        wsr_l[pp, C:, :] = (sr_w[:, :, ky, kx + 1] * n1w[None, :]).T
    wsr_l = wsr_l.transpose(1, 0, 2)
    bsr_l = (sr_w.sum((2, 3)) @ n1b + sr_b)[:, None]

    wkv_l = (kv_w * srnw[None, :]).T
    bkv_l = (kv_w @ srnb + kv_b)[:, None]

    wpj_l = np.zeros((C + 1, C + 1), np.float32)
    wpj_l[:C, :C] = pj_w.T
    wpj_l[C, :C] = pj_b
    wpj_l[C, C] = 1.0

    k9 = dww[:, 0, :, :].reshape(HID, 9)          # [256, 9]
    wmp_l = np.zeros((6, 128, 128), np.float32)
    wms_l = np.zeros((6, C, 128), np.float32)
    for dy in range(3):
        for g in range(2):
            Ma = (k9[:, dy * 3 + 0][:, None] * f1w * n2w[None, :])[128 * g:128 * (g + 1)]
            Mb = (k9[:, dy * 3 + 1][:, None] * f1w * n2w[None, :])[128 * g:128 * (g + 1)]
            Mc = (k9[:, dy * 3 + 2][:, None] * f1w * n2w[None, :])[128 * g:128 * (g + 1)]
            wmp_l[2 * dy + g, :C, :] = Ma.T
            wmp_l[2 * dy + g, C:, :] = Mb.T
            wms_l[2 * dy + g, :, :] = Mc.T
    wmp_l = wmp_l.transpose(1, 0, 2)
    wms_l = wms_l.transpose(1, 0, 2)
    bg_full = k9.sum(1) * (f1w @ n2b + f1b) + dwb  # [256]
    bg_l = np.ascontiguousarray(bg_full.reshape(2, 128).T)

    wf2_l = np.stack([f2w[:, :128].T, f2w[:, 128:].T], 0).transpose(1, 0, 2)
    bf2_l = f2b[:, None]

    bfc = lambda a: np.ascontiguousarray(a).astype(BF)
    return {
        "wq": bfc(wq_l), "bq": np.ascontiguousarray(bq_l),
        "wsr": bfc(wsr_l), "bsr": np.ascontiguousarray(bsr_l),
        "wkv": bfc(wkv_l), "bkv": np.ascontiguousarray(bkv_l),
        "wpj": bfc(wpj_l),
        "wmp": bfc(wmp_l), "wms": bfc(wms_l),
        "bg": np.ascontiguousarray(bg_l),
        "wf2": bfc(wf2_l), "bf2": np.ascontiguousarray(bf2_l),
    }


def kernel(trace=False, tmpdir=None, **inputs):
    nc = _get_nc()
    x = np.asarray(inputs["x"], np.float32)
    wts = _prep_weights(inputs)
    in_maps = [dict(wts, x=np.ascontiguousarray(x[b])) for b in range(B)]
    res = run_bass_kernel_spmd(nc, in_maps, core_ids=list(range(8)),
                               trace=trace, tmpdir=tmpdir)
    out = np.stack([res.results[b]["out"] for b in range(B)], 0)
    kernel.last_exec_time_ns = res.exec_time_ns
    return out


# revision 29
# speedup vs baseline: 1.4452x; 1.0511x over previous
"""Trainium2 Bass kernel for nn_Block_523986010339 (PVT-style transformer block).

Sharding: data-parallel over batch B=8 -> one batch element per NeuronCore.
Per-core layouts:
  - residual stream token-major fp32 [128p=token%128, 128t=token//128, 64c]
  - matmul operands channel-major bf16 [c, n], n = 128*y + x
  - LN mean folded into matmul weights via an extra "m*g" row; rsqrt scale
    applied token-major with broadcast APs
  - attention: S^T channel-major, exp without max-subtraction (tiny logits),
    denominator via fused ones-column in the V matmul, divided out after proj
  - MLP: fc1 and 3x3 depthwise conv fused into 9 accumulated matmuls over a
    zero-guarded channel-major layout (row pitch 130)
"""

import functools
import json

import numpy as np
import ml_dtypes

import concourse.bass as bass
import concourse.mybir as mybir
import concourse.tile as tile
from concourse.bass_utils import run_bass_kernel_spmd
from concourse.masks import make_identity

F32 = mybir.dt.float32
BF16 = mybir.dt.bfloat16
BF = ml_dtypes.bfloat16

B, N, C, H, W = 8, 16384, 64, 128, 128
SR, HID, NR = 8, 256, 256
P, T = 128, 128
RP = W + 2          # guarded row pitch
PAD = RP + 1        # head/tail pad so all tap offsets stay in-bounds
NG = PAD + RP * (H + 2) + PAD
AX = mybir.AxisListType
OP = mybir.AluOpType
AF = mybir.ActivationFunctionType


def _split_excess_waits(nc, max_waits=1):
    """walrus in this container rejects >1 sync wait per instruction; move
    excess waits onto injected Drain instructions just before the owner."""
    d = json.loads(mybir.module_to_json_string(nc.m))
    n_split = [0]

    def fix(insts):
        out = []
        for inst in insts:
            si = inst.get("sync_info") or {}
            waits = si.get("on_wait") or []
            if len(waits) > max_waits:
                extra = waits[:-max_waits]
                for i in range(0, len(extra), max_waits):
                    n_split[0] += 1
                    out.append({
                        "name": f"WSPLIT-{n_split[0]}",
                        "opcode": "NoOp",
                        "engine": inst["engine"],
                        "ins": [],
                        "outs": [],
                        "is_reset_sema": False,
                        "sync_info": {"on_update": [],
                                      "on_wait": extra[i:i + max_waits]},
                    })
                si["on_wait"] = waits[-max_waits:]
                inst["sync_info"] = si
            out.append(inst)
        return out

    for f in d.get("functions", []):
        for bb in f.get("blocks", []):
            bb["instructions"] = fix(bb["instructions"])
    nc.m = mybir.module_from_json_string(json.dumps(d))


def _ln_stats(nc, sc, big, x_tm, epst, nt):
    """Token-major LN stats: returns (g, mg) tiles [128, nt] fp32 given
    x_tm [128, nt, 64] fp32."""
    sq_scr = big.tile([P, nt * C], BF16, tag="scr2", name="sq")
    xsq_view = sq_scr.rearrange("p (t c) -> p t c", c=C)
    nc.scalar.square(out=sq_scr, in_=x_tm.rearrange("p t c -> p (t c)"))
    s1 = sc.tile([P, nt], F32, tag=f"s1_{nt}")
    s2 = sc.tile([P, nt], F32, tag=f"s2_{nt}")
    nc.vector.tensor_reduce(out=s1, in_=x_tm, axis=AX.X, op=OP.add)
    nc.vector.tensor_reduce(out=s2, in_=xsq_view, axis=AX.X, op=OP.add)
    return _ln_finalize(nc, sc, s1, s2, epst, nt)


def _ln_finalize(nc, sc, s1, s2, epst, nt):
    mean = sc.tile([P, nt], F32, tag=f"mean_{nt}")
    var = sc.tile([P, nt], F32, tag=f"var_{nt}")
    nc.vector.tensor_scalar_mul(out=mean, in0=s1, scalar1=1.0 / C)
    nc.vector.tensor_scalar_mul(out=var, in0=s2, scalar1=1.0 / C)
    mm = sc.tile([P, nt], F32, tag=f"mm_{nt}")
    nc.vector.tensor_tensor(out=mm, in0=mean, in1=mean, op=OP.mult)
    nc.vector.tensor_tensor(out=var, in0=var, in1=mm, op=OP.subtract)
    sd = sc.tile([P, nt], F32, tag=f"sd_{nt}")
    nc.scalar.activation(out=sd, in_=var, func=AF.Sqrt, bias=epst, scale=1.0)
    g = sc.tile([P, nt], F32, tag=f"g_{nt}")
    nc.vector.reciprocal(out=g, in_=sd)
    mg = sc.tile([P, nt], F32, tag=f"mg_{nt}")
    nc.vector.tensor_tensor(out=mg, in0=mean, in1=g, op=OP.mult)
    return g, mg


def _build_nc():
    nc = bass.Bass("TRN2")
    x_d = nc.dram_tensor("x", [N, C], F32, kind="ExternalInput")
    out_d = nc.dram_tensor("out", [N, C], F32, kind="ExternalOutput")
    wq_d = nc.dram_tensor("wq", [C, C], BF16, kind="ExternalInput")
    bq_d = nc.dram_tensor("bq", [C, 1], F32, kind="ExternalInput")
    wsr_d = nc.dram_tensor("wsr", [128, 32, C], BF16, kind="ExternalInput")
    bsr_d = nc.dram_tensor("bsr", [C, 1], F32, kind="ExternalInput")
    wkv_d = nc.dram_tensor("wkv", [C, 2 * C], BF16, kind="ExternalInput")
    bkv_d = nc.dram_tensor("bkv", [2 * C, 1], F32, kind="ExternalInput")
    wpj_d = nc.dram_tensor("wpj", [C + 1, C + 1], BF16, kind="ExternalInput")
    wmp_d = nc.dram_tensor("wmp", [128, 6, 128], BF16, kind="ExternalInput")
    wms_d = nc.dram_tensor("wms", [C, 6, 128], BF16, kind="ExternalInput")
    bg_d = nc.dram_tensor("bg", [128, 2], F32, kind="ExternalInput")
    wf2_d = nc.dram_tensor("wf2", [128, 2, C], BF16, kind="ExternalInput")
    bf2_d = nc.dram_tensor("bf2", [C, 1], F32, kind="ExternalInput")

    with tile.TileContext(nc) as tc:
        with (
            tc.tile_pool(name="consts", bufs=1) as consts,
            tc.tile_pool(name="big", bufs=1) as big,
            tc.tile_pool(name="sc", bufs=2) as sc,
            tc.tile_pool(name="ch", bufs=3) as ch,
            tc.tile_pool(name="psA", bufs=6, space="PSUM") as psA,
            tc.tile_pool(name="psT", bufs=2, space="PSUM") as psT,
        ):
            ident = consts.tile([128, 128], BF16)
            make_identity(nc, ident)
            wq = consts.tile([C, C], BF16)
            nc.gpsimd.dma_start(out=wq, in_=wq_d[:, :])
            wsr = consts.tile([128, 32, C], BF16)
            nc.gpsimd.dma_start(out=wsr, in_=wsr_d[:, :, :])
            wkv = consts.tile([C, 2 * C], BF16)
            nc.gpsimd.dma_start(out=wkv, in_=wkv_d[:, :])
            wpj = consts.tile([C + 1, C + 1], BF16)
            nc.gpsimd.dma_start(out=wpj, in_=wpj_d[:, :])
            wmp = consts.tile([128, 6, 128], BF16)
            nc.gpsimd.dma_start(out=wmp, in_=wmp_d[:, :, :])
            wms = consts.tile([C, 6, 128], BF16)
            nc.gpsimd.dma_start(out=wms, in_=wms_d[:, :, :])
            wf2 = consts.tile([128, 2, C], BF16)
            nc.gpsimd.dma_start(out=wf2, in_=wf2_d[:, :, :])
            bq = consts.tile([C, 1], F32)
            nc.gpsimd.dma_start(out=bq, in_=bq_d[:, :])
            bsr = consts.tile([C, 1], F32)
            nc.gpsimd.dma_start(out=bsr, in_=bsr_d[:, :])
            bkv = consts.tile([2 * C, 1], F32)
            nc.gpsimd.dma_start(out=bkv, in_=bkv_d[:, :])
            bg = consts.tile([128, 2], F32)
            nc.gpsimd.dma_start(out=bg, in_=bg_d[:, :])
            bf2 = consts.tile([C, 1], F32)
            nc.gpsimd.dma_start(out=bf2, in_=bf2_d[:, :])
            epst = consts.tile([P, 1], F32)
            nc.vector.memset(epst, 1e-5)

            # ---- load x (token-major), LN1 stats overlapped per slice ----
            x_tm = big.tile([P, T, C], F32, tag="xr")
            x_v = x_d.rearrange("(t p) c -> p t c", p=P)
            sq_scr = big.tile([P, T * C], BF16, tag="scr2", name="sq")
            sqv = sq_scr.rearrange("p (t c) -> p t c", c=C)
            s1 = sc.tile([P, T], F32, tag="s1")
            s2 = sc.tile([P, T], F32, tag="s2")
            for q8 in range(8):
                sl = slice(16 * q8, 16 * (q8 + 1))
                eng = nc.sync if q8 % 2 == 0 else nc.scalar
                eng.dma_start(out=x_tm[:, sl, :], in_=x_v[:, sl, :])
                nc.scalar.square(out=sqv[:, sl, :], in_=x_tm[:, sl, :])
                nc.vector.tensor_reduce(out=s1[:, sl], in_=x_tm[:, sl, :],
                                        axis=AX.X, op=OP.add)
                nc.vector.tensor_reduce(out=s2[:, sl], in_=sqv[:, sl, :],
                                        axis=AX.X, op=OP.add)
            g1, mg1 = _ln_finalize(nc, sc, s1, s2, epst, T)
            # warm up the PE so HAM is at 8/8 when real matmuls start
            for wd in range(15):
                pw = psT.tile([128, 128], F32, tag="tp", name="pw")
                nc.tensor.matmul(out=pw, lhsT=ident, rhs=ident,
                                 start=True, stop=True)
            a1tm = big.tile([P, T, C], BF16, tag="scr2")
            a1cm = big.tile([128, N], BF16, tag="acm")
            a1cm_v = a1cm[0:C, :].rearrange("c (j a b n) -> c j a b n", a=4, b=2, n=128)
            a1tm_v = a1tm.rearrange("p t c -> p (t c)")
            for q8 in range(8):
                sl = slice(16 * q8, 16 * (q8 + 1))
                nc.vector.tensor_tensor(
                    out=a1tm[:, sl, :], in0=x_tm[:, sl, :],
                    in1=g1[:, sl, None].broadcast_to([P, 16, C]), op=OP.mult)
                nc.vector.tensor_tensor(
                    out=a1tm[:, sl, :], in0=a1tm[:, sl, :],
                    in1=mg1[:, sl, None].broadcast_to([P, 16, C]),
                    op=OP.subtract)
                for j in (2 * q8, 2 * q8 + 1):
                    pt = psT.tile([128, 4, 128], BF16, tag="tp")
                    for k in range(4):
                        tt = 8 * j + 2 * k
                        nc.tensor.transpose(out=pt[:, k, :],
                                            in_=a1tm_v[:, 64 * tt:64 * (tt + 2)],
                                            identity=ident)
                    nc.scalar.copy(out=a1cm_v[:, j, :, 0, :], in_=pt[0:C, :, :])
                    nc.vector.tensor_copy(out=a1cm_v[:, j, :, 1, :],
                                          in_=pt[C:128, :, :])
                    nc.vector.tensor_copy(
                        out=a1cm[C:128, 1024 * j:1024 * (j + 1) - 1],
                        in_=a1cm[0:C, 1024 * j + 1:1024 * (j + 1)])
                    if j > 0:
                        nc.gpsimd.tensor_copy(
                            out=a1cm[C:128, 1024 * j - 1:1024 * j],
                            in_=a1cm[0:C, 1024 * j:1024 * j + 1])

            # ---- spatial reduction conv (8x8 stride 8) ----
            a1sr = a1cm.rearrange("c (Y ky X kx) -> c ky kx Y X", ky=SR, kx=SR, X=16)
            psr = psA.tile([128, 512], F32, tag="ps", name="ps").rearrange("c (a y x) -> c a y x", a=2, y=16)[0:C, 0, :, :]
            for pp in range(32):
                ky, kx = pp // 4, (pp % 4) * 2
                nc.tensor.matmul(out=psr, lhsT=wsr[:, pp, :],
                                 rhs=a1sr[:, ky, kx, :, :],
                                 start=(pp == 0), stop=(pp == 31))
            xrcm = consts.tile([C, NR], BF16)
            nc.scalar.activation(out=xrcm.rearrange("c (y x) -> c y x", x=16),
                                 in_=psr, func=AF.Identity,
                                 bias=bsr, scale=1.0)

            # ---- LN on reduced tokens (srn), token-major ----
            xr_tm = consts.tile([P, 2, C], F32)
            for hh in range(2):
                pv = psT.tile([128, C], BF16, tag="tp")
                nc.tensor.transpose(out=pv, in_=xrcm[:, 128 * hh:128 * (hh + 1)],
                                    identity=ident[0:C, 0:C])
                nc.vector.tensor_copy(out=xr_tm[:, hh, :], in_=pv)
            g_r, mg_r = _ln_stats(nc, sc, consts, xr_tm, epst, 2)
            ar_tm = consts.tile([P, 2, C], BF16)
            nc.vector.tensor_tensor(
                out=ar_tm, in0=xr_tm,
                in1=g_r[:, :, None].broadcast_to([P, 2, C]), op=OP.mult)
            mgb = sc.tile([P, 2, C], BF16, tag="mgb")
            nc.vector.tensor_tensor(
                out=mgb, in0=mg_r[:, :, None].broadcast_to([P, 2, C]),
                in1=g_r[:, :, None].broadcast_to([P, 2, C]), op=OP.bypass)
            nc.vector.tensor_tensor(out=ar_tm, in0=ar_tm, in1=mgb, op=OP.subtract)
            arcm = consts.tile([C, NR], BF16)
            for hh in range(2):
                pv = psT.tile([C, 128], BF16, tag="tp")
                nc.tensor.transpose(out=pv, in_=ar_tm[:, hh, :], identity=ident)
                nc.vector.tensor_copy(out=arcm[:, 128 * hh:128 * (hh + 1)], in_=pv)

            # ---- KV ----
            pkv = psA.tile([128, 512], F32, tag="ps", name="ps")[:, 0:NR]
            nc.tensor.matmul(out=pkv, lhsT=wkv, rhs=arcm, start=True, stop=True)
            kvcm = consts.tile([2 * C, NR], BF16)
            nc.scalar.activation(out=kvcm, in_=pkv, func=AF.Identity,
                                 bias=bkv, scale=1.0)
            # fold q-projection into K:  S^T = (K @ Wq) @ A1
            bqb = consts.tile([C, 1], BF16)
            nc.vector.tensor_copy(out=bqb, in_=bq)
            pkw = psT.tile([C, NR], F32, tag="tp", name="pkw")
            nc.tensor.matmul(out=pkw, lhsT=wq, rhs=kvcm[0:C, :],
                             start=True, stop=True)
            kwt = consts.tile([C, NR], BF16)
            nc.scalar.copy(out=kwt, in_=pkw)
            sbias = consts.tile([128, 2], F32)
            for hh in range(2):
                pb = psT.tile([128, 1], F32, tag="tp", name="pb")
                nc.tensor.matmul(out=pb,
                                 lhsT=kvcm[0:C, 128 * hh:128 * (hh + 1)],
                                 rhs=bqb, start=True, stop=True)
                nc.vector.tensor_copy(out=sbias[:, hh:hh + 1], in_=pb)
            vp = consts.tile([128, 2, C + 1], BF16)
            nc.vector.memset(vp[:, :, C:C + 1], 1.0)
            for hh in range(2):
                pv = psT.tile([128, C], BF16, tag="tp")
                nc.tensor.transpose(out=pv,
                                    in_=kvcm[C:2 * C, 128 * hh:128 * (hh + 1)],
                                    identity=ident[C:2 * C, C:2 * C])
                nc.vector.tensor_copy(out=vp[:, hh, 0:C], in_=pv)

            # ---- attention, streamed in 512-column chunks ----
            y_tm = big.tile([P, T, C], F32, tag="y")
            sq2 = big.tile([P, T * C], BF16, tag="scr2", name="sq2")
            sq2v = sq2.rearrange("p (t c) -> p t c", c=C)
            s1y = sc.tile([P, T], F32, tag="s1y")
            s2y = sc.tile([P, T], F32, tag="s2y")
            for i in range(32):
                ech = ch.tile([128, 2, 512], BF16, tag="e")
                for hh in range(2):
                    pS = psA.tile([128, 512], F32, tag="ps", name="ps")
                    nc.tensor.matmul(out=pS,
                                     lhsT=kwt[:, 128 * hh:128 * (hh + 1)],
                                     rhs=a1cm[0:C, 512 * i:512 * (i + 1)],
                                     start=True, stop=True)
                    nc.scalar.activation(out=ech[:, hh, :], in_=pS, func=AF.Exp,
                                         bias=sbias[:, hh:hh + 1], scale=1.0)
                pO = psA.tile([128, 512], F32, tag="ps", name="ps")[0:C + 1, :]
                for hh in range(2):
                    nc.tensor.matmul(out=pO, lhsT=vp[:, hh, :],
                                     rhs=ech[:, hh, :],
                                     start=(hh == 0), stop=(hh == 1))
                pod = ch.tile([C + 1, 512], BF16, tag="pod")
                nc.vector.tensor_copy(out=pod, in_=pO)
                ptr = psT.tile([128, 4, C + 1], F32, tag="tp")
                for k in range(4):
                    nc.tensor.matmul(out=ptr[:, k, :],
                                     lhsT=pod[:, 128 * k:128 * (k + 1)],
                                     rhs=wpj, start=True, stop=True)
                rt = sc.tile([P, 4, 1], F32, tag="rt")
                nc.vector.reciprocal(out=rt, in_=ptr[:, :, C:C + 1])
                tmp = ch.tile([P, 4, C], F32, tag="tmp")
                nc.vector.tensor_tensor(out=tmp, in0=ptr[:, :, 0:C],
                                        in1=rt.broadcast_to([P, 4, C]),
                                        op=OP.mult)
                nc.vector.tensor_tensor(out=y_tm[:, 4 * i:4 * (i + 1), :],
                                        in0=tmp, in1=x_tm[:, 4 * i:4 * (i + 1), :],
                                        op=OP.add)
                if i % 4 == 3:
                    sl = slice(16 * (i // 4), 16 * (i // 4 + 1))
                    nc.scalar.square(out=sq2v[:, sl, :], in_=y_tm[:, sl, :])
                    nc.vector.tensor_reduce(out=s1y[:, sl], in_=y_tm[:, sl, :],
                                            axis=AX.X, op=OP.add)
                    nc.vector.tensor_reduce(out=s2y[:, sl], in_=sq2v[:, sl, :],
                                            axis=AX.X, op=OP.add)

            # ---- LN2 ----
            g2, mg2 = _ln_finalize(nc, sc, s1y, s2y, epst, T)
            a2tm = big.tile([P, T, C], BF16, tag="scr2")
            # ---- A2 guarded channel-major, doubled: rows 64:128 shifted by +1 ----
            a2g = big.tile([128, NG], BF16, tag="acm")
            nc.vector.memset(a2g[:, 0:PAD + RP], 0.0)
            nc.vector.memset(a2g[:, NG - PAD - RP:NG], 0.0)
            a2rows = a2g[0:C, PAD + RP:PAD + RP * (H + 1)].rearrange(
                "c (y w) -> c y w", w=RP)
            a2rowsB = a2g[C:128, PAD + RP:PAD + RP * (H + 1)].rearrange(
                "c (y w) -> c y w", w=RP)
            nc.vector.memset(a2rows[:, :, 0:1], 0.0)
            nc.vector.memset(a2rows[:, :, RP - 1:RP], 0.0)
            nc.vector.memset(a2rowsB[:, :, RP - 2:RP], 0.0)
            a2tm_v = a2tm.rearrange("p t c -> p (t c)")
            ro = a2rows.rearrange("c (j a b) w -> c j a b w", a=4, b=2)
            for q8 in range(8):
                sl = slice(16 * q8, 16 * (q8 + 1))
                nc.vector.tensor_tensor(
                    out=a2tm[:, sl, :], in0=y_tm[:, sl, :],
                    in1=g2[:, sl, None].broadcast_to([P, 16, C]), op=OP.mult)
                nc.vector.tensor_tensor(
                    out=a2tm[:, sl, :], in0=a2tm[:, sl, :],
                    in1=mg2[:, sl, None].broadcast_to([P, 16, C]),
                    op=OP.subtract)
                for j in (2 * q8, 2 * q8 + 1):
                    pt = psT.tile([128, 4, 128], BF16, tag="tp")
                    for k in range(4):
                        tt = 8 * j + 2 * k
                        nc.tensor.transpose(out=pt[:, k, :],
                                            in_=a2tm_v[:, 64 * tt:64 * (tt + 2)],
                                            identity=ident)
                    nc.scalar.copy(out=ro[:, j, :, 0, 1:W + 1], in_=pt[0:C, :, :])
                    nc.vector.tensor_copy(out=ro[:, j, :, 1, 1:W + 1],
                                          in_=pt[C:128, :, :])
                    nc.vector.tensor_copy(
                        out=a2rowsB[:, 8 * j:8 * (j + 1), 0:W],
                        in_=a2rows[:, 8 * j:8 * (j + 1), 1:W + 1])

            # ---- MLP: fused fc1 (+) 3x3 depthwise conv, gelu, fc2 ----
            o2cm = big.tile([C, NG], BF16, tag="qt")
            n_mlp = 33
            for j in range(n_mlp):
                cb = PAD + RP + 512 * j
                size = min(512, PAD + RP * (H + 1) - cb)
                gch = []
                for g in range(2):
                    pG = psA.tile([128, 512], F32, tag="ps", name="ps")
                    for dy in (-1, 0, 1):
                        nc.tensor.matmul(
                            out=pG[:, 0:size], lhsT=wmp[:, 2 * (dy + 1) + g, :],
                            rhs=a2g[:, cb + RP * dy - 1:cb + RP * dy - 1 + size],
                            start=(dy == -1), stop=False)
                    for dy in (-1, 0, 1):
                        nc.tensor.matmul(
                            out=pG[:, 0:size], lhsT=wms[:, 2 * (dy + 1) + g, :],
                            rhs=a2g[0:C, cb + RP * dy + 1:cb + RP * dy + 1 + size],
                            start=False, stop=(dy == 1))
                    gc = ch.tile([128, 512], BF16, tag=f"gc{g}")
                    nc.scalar.activation(out=gc[:, 0:size], in_=pG[:, 0:size],
                                         func=AF.Gelu, bias=bg[:, g:g + 1],
                                         scale=1.0)
                    gch.append(gc)
                pF = psA.tile([128, 512], F32, tag="ps", name="ps")
                for g in range(2):
                    nc.tensor.matmul(out=pF[0:C, 0:size], lhsT=wf2[:, g, :],
                                     rhs=gch[g][:, 0:size],
                                     start=(g == 0), stop=(g == 1))
                nc.vector.tensor_scalar(out=o2cm[:, cb:cb + size],
                                        in0=pF[0:C, 0:size], scalar1=bf2,
                                        scalar2=None, op0=OP.add)

            # ---- MLP epilogue: transpose back, residual, store ----
            y2_tm = big.tile([P, T, C], F32, tag="xr")  # reuses x_tm slot
            out_v = out_d.rearrange("(t p) c -> p t c", p=P)
            for j in range(32):
                pt2 = psT.tile([128, 4, C], BF16, tag="tp")
                for k in range(4):
                    t = 4 * j + k
                    s = PAD + RP * (t + 1) + 1
                    nc.tensor.transpose(out=pt2[:, k, :],
                                        in_=o2cm[:, s:s + W],
                                        identity=ident[0:C, 0:C])
                nc.vector.tensor_tensor(out=y2_tm[:, 4 * j:4 * (j + 1), :],
                                        in0=pt2, in1=y_tm[:, 4 * j:4 * (j + 1), :],
                                        op=OP.add)
                if j % 4 == 3:
                    q8 = j // 4
                    nc.sync.dma_start(out=out_v[:, 16 * q8:16 * (q8 + 1), :],
                                       in_=y2_tm[:, 16 * q8:16 * (q8 + 1), :])

    _split_excess_waits(nc)
    return nc


@functools.cache
def _get_nc():
    return _build_nc()


def _prep_weights(inp):
    f = lambda v: np.asarray(v, np.float32)
    n1w, n1b = f(inp["n1_w"]), f(inp["n1_b"])
    q_w, q_b = f(inp["q_w"]), f(inp["q_b"])
    kv_w, kv_b = f(inp["kv_w"]), f(inp["kv_b"])
    sr_w, sr_b = f(inp["sr_w"]), f(inp["sr_b"])
    srnw, srnb = f(inp["srn_w"]), f(inp["srn_b"])
    pj_w, pj_b = f(inp["proj_w"]), f(inp["proj_b"])
    n2w, n2b = f(inp["n2_w"]), f(inp["n2_b"])
    f1w, f1b = f(inp["fc1_w"]), f(inp["fc1_b"])
    dww, dwb = f(inp["dw_w"]), f(inp["dw_b"])
    f2w, f2b = f(inp["fc2_w"]), f(inp["fc2_b"])

    scale = (C // 1) ** -0.5
    wq_l = (q_w * n1w[None, :]).T * scale
    bq_l = ((q_w @ n1b + q_b) * scale)[:, None]

    wsr_l = np.zeros((32, 128, C), np.float32)
    for pp in range(32):
        ky, kx = pp // 4, (pp % 4) * 2
        wsr_l[pp, :C, :] = (sr_w[:, :, ky, kx] * n1w[None, :]).T
        wsr_l[pp, C:, :] = (sr_w[:, :, ky, kx + 1] * n1w[None, :]).T
    wsr_l = wsr_l.transpose(1, 0, 2)
    bsr_l = (sr_w.sum((2, 3)) @ n1b + sr_b)[:, None]

    wkv_l = (kv_w * srnw[None, :]).T
    bkv_l = (kv_w @ srnb + kv_b)[:, None]

    wpj_l = np.zeros((C + 1, C + 1), np.float32)
    wpj_l[:C, :C] = pj_w.T
    wpj_l[C, :C] = pj_b
    wpj_l[C, C] = 1.0

    k9 = dww[:, 0, :, :].reshape(HID, 9)          # [256, 9]
    wmp_l = np.zeros((6, 128, 128), np.float32)
    wms_l = np.zeros((6, C, 128), np.float32)
    for dy in range(3):
        for g in range(2):
            Ma = (k9[:, dy * 3 + 0][:, None] * f1w * n2w[None, :])[128 * g:128 * (g + 1)]
            Mb = (k9[:, dy * 3 + 1][:, None] * f1w * n2w[None, :])[128 * g:128 * (g + 1)]
            Mc = (k9[:, dy * 3 + 2][:, None] * f1w * n2w[None, :])[128 * g:128 * (g + 1)]
            wmp_l[2 * dy + g, :C, :] = Ma.T
            wmp_l[2 * dy + g, C:, :] = Mb.T
            wms_l[2 * dy + g, :, :] = Mc.T
    wmp_l = wmp_l.transpose(1, 0, 2)
    wms_l = wms_l.transpose(1, 0, 2)
    bg_full = k9.sum(1) * (f1w @ n2b + f1b) + dwb  # [256]
    bg_l = np.ascontiguousarray(bg_full.reshape(2, 128).T)

    wf2_l = np.stack([f2w[:, :128].T, f2w[:, 128:].T], 0).transpose(1, 0, 2)
    bf2_l = f2b[:, None]

    bfc = lambda a: np.ascontiguousarray(a).astype(BF)
    return {
        "wq": bfc(wq_l), "bq": np.ascontiguousarray(bq_l),
        "wsr": bfc(wsr_l), "bsr": np.ascontiguousarray(bsr_l),
        "wkv": bfc(wkv_l), "bkv": np.ascontiguousarray(bkv_l),
        "wpj": bfc(wpj_l),
        "wmp": bfc(wmp_l), "wms": bfc(wms_l),
        "bg": np.ascontiguousarray(bg_l),
        "wf2": bfc(wf2_l), "bf2": np.ascontiguousarray(bf2_l),
    }


def kernel(trace=False, tmpdir=None, **inputs):
    nc = _get_nc()
    x = np.asarray(inputs["x"], np.float32)
    wts = _prep_weights(inputs)
    in_maps = [dict(wts, x=np.ascontiguousarray(x[b])) for b in range(B)]
    res = run_bass_kernel_spmd(nc, in_maps, core_ids=list(range(8)),
                               trace=trace, tmpdir=tmpdir)
    out = np.stack([res.results[b]["out"] for b in range(B)], 0)
    kernel.last_exec_time_ns = res.exec_time_ns
    return out


# revision 30
# speedup vs baseline: 1.4660x; 1.0144x over previous
"""Trainium2 Bass kernel for nn_Block_523986010339 (PVT-style transformer block).

Sharding: data-parallel over batch B=8 -> one batch element per NeuronCore.
Per-core layouts:
  - residual stream token-major fp32 [128p=token%128, 128t=token//128, 64c]
  - matmul operands channel-major bf16 [c, n], n = 128*y + x
  - LN mean folded into matmul weights via an extra "m*g" row; rsqrt scale
    applied token-major with broadcast APs
  - attention: S^T channel-major, exp without max-subtraction (tiny logits),
    denominator via fused ones-column in the V matmul, divided out after proj
  - MLP: fc1 and 3x3 depthwise conv fused into 9 accumulated matmuls over a
    zero-guarded channel-major layout (row pitch 130)
"""

import functools
import json

import numpy as np
import ml_dtypes

import concourse.bass as bass
import concourse.mybir as mybir
import concourse.tile as tile
from concourse.bass_utils import run_bass_kernel_spmd
from concourse.masks import make_identity

F32 = mybir.dt.float32
BF16 = mybir.dt.bfloat16
BF = ml_dtypes.bfloat16

B, N, C, H, W = 8, 16384, 64, 128, 128
SR, HID, NR = 8, 256, 256
P, T = 128, 128
RP = W + 2          # guarded row pitch
PAD = RP + 1        # head/tail pad so all tap offsets stay in-bounds
NG = PAD + RP * (H + 2) + PAD
AX = mybir.AxisListType
OP = mybir.AluOpType
AF = mybir.ActivationFunctionType


def _split_excess_waits(nc, max_waits=1):
    """walrus in this container rejects >1 sync wait per instruction; move
    excess waits onto injected Drain instructions just before the owner."""
    d = json.loads(mybir.module_to_json_string(nc.m))
    n_split = [0]

    def fix(insts):
        out = []
        for inst in insts:
            si = inst.get("sync_info") or {}
            waits = si.get("on_wait") or []
            if len(waits) > max_waits:
                extra = waits[:-max_waits]
                for i in range(0, len(extra), max_waits):
                    n_split[0] += 1
                    out.append({
                        "name": f"WSPLIT-{n_split[0]}",
                        "opcode": "NoOp",
                        "engine": inst["engine"],
                        "ins": [],
                        "outs": [],
                        "is_reset_sema": False,
                        "sync_info": {"on_update": [],
                                      "on_wait": extra[i:i + max_waits]},
                    })
                si["on_wait"] = waits[-max_waits:]
                inst["sync_info"] = si
            out.append(inst)
        return out

    for f in d.get("functions", []):
        for bb in f.get("blocks", []):
            bb["instructions"] = fix(bb["instructions"])
    nc.m = mybir.module_from_json_string(json.dumps(d))


def _ln_stats(nc, sc, big, x_tm, epst, nt):
    """Token-major LN stats: returns (g, mg) tiles [128, nt] fp32 given
    x_tm [128, nt, 64] fp32."""
    sq_scr = big.tile([P, nt * C], BF16, tag="scr2", name="sq")
    xsq_view = sq_scr.rearrange("p (t c) -> p t c", c=C)
    nc.scalar.square(out=sq_scr, in_=x_tm.rearrange("p t c -> p (t c)"))
    s1 = sc.tile([P, nt], F32, tag=f"s1_{nt}")
    s2 = sc.tile([P, nt], F32, tag=f"s2_{nt}")
    nc.vector.tensor_reduce(out=s1, in_=x_tm, axis=AX.X, op=OP.add)
    nc.vector.tensor_reduce(out=s2, in_=xsq_view, axis=AX.X, op=OP.add)
    return _ln_finalize(nc, sc, s1, s2, epst, nt)


def _ln_finalize(nc, sc, s1, s2, epst, nt):
    mean = sc.tile([P, nt], F32, tag=f"mean_{nt}")
    var = sc.tile([P, nt], F32, tag=f"var_{nt}")
    nc.vector.tensor_scalar_mul(out=mean, in0=s1, scalar1=1.0 / C)
    nc.vector.tensor_scalar_mul(out=var, in0=s2, scalar1=1.0 / C)
    mm = sc.tile([P, nt], F32, tag=f"mm_{nt}")
    nc.vector.tensor_tensor(out=mm, in0=mean, in1=mean, op=OP.mult)
    nc.vector.tensor_tensor(out=var, in0=var, in1=mm, op=OP.subtract)
    sd = sc.tile([P, nt], F32, tag=f"sd_{nt}")
    nc.scalar.activation(out=sd, in_=var, func=AF.Sqrt, bias=epst, scale=1.0)
    g = sc.tile([P, nt], F32, tag=f"g_{nt}")
    nc.vector.reciprocal(out=g, in_=sd)
    mg = sc.tile([P, nt], F32, tag=f"mg_{nt}")
    nc.vector.tensor_tensor(out=mg, in0=mean, in1=g, op=OP.mult)
    return g, mg


def _build_nc():
    nc = bass.Bass("TRN2")
    x_d = nc.dram_tensor("x", [N, C], F32, kind="ExternalInput")
    out_d = nc.dram_tensor("out", [N, C], F32, kind="ExternalOutput")
    wq_d = nc.dram_tensor("wq", [C, C], BF16, kind="ExternalInput")
    bq_d = nc.dram_tensor("bq", [C, 1], F32, kind="ExternalInput")
    wsr_d = nc.dram_tensor("wsr", [128, 32, C], BF16, kind="ExternalInput")
    bsr_d = nc.dram_tensor("bsr", [C, 1], F32, kind="ExternalInput")
    wkv_d = nc.dram_tensor("wkv", [C, 2 * C], BF16, kind="ExternalInput")
    bkv_d = nc.dram_tensor("bkv", [2 * C, 1], F32, kind="ExternalInput")
    wpj_d = nc.dram_tensor("wpj", [C + 1, C + 1], BF16, kind="ExternalInput")
    wmp_d = nc.dram_tensor("wmp", [128, 6, 128], BF16, kind="ExternalInput")
    wms_d = nc.dram_tensor("wms", [C, 6, 128], BF16, kind="ExternalInput")
    bg_d = nc.dram_tensor("bg", [128, 2], F32, kind="ExternalInput")
    wf2_d = nc.dram_tensor("wf2", [128, 2, C], BF16, kind="ExternalInput")
    bf2_d = nc.dram_tensor("bf2", [C, 1], F32, kind="ExternalInput")

    with tile.TileContext(nc) as tc:
        with (
            tc.tile_pool(name="consts", bufs=1) as consts,
            tc.tile_pool(name="big", bufs=1) as big,
            tc.tile_pool(name="sc", bufs=2) as sc,
            tc.tile_pool(name="ch", bufs=3) as ch,
            tc.tile_pool(name="psA", bufs=6, space="PSUM") as psA,
            tc.tile_pool(name="psT", bufs=2, space="PSUM") as psT,
        ):
            ident = consts.tile([128, 128], BF16)
            make_identity(nc, ident)
            wq = consts.tile([C, C], BF16)
            nc.gpsimd.dma_start(out=wq, in_=wq_d[:, :])
            wsr = consts.tile([128, 32, C], BF16)
            nc.gpsimd.dma_start(out=wsr, in_=wsr_d[:, :, :])
            wkv = consts.tile([C, 2 * C], BF16)
            nc.gpsimd.dma_start(out=wkv, in_=wkv_d[:, :])
            wpj = consts.tile([C + 1, C + 1], BF16)
            nc.gpsimd.dma_start(out=wpj, in_=wpj_d[:, :])
            wmp = consts.tile([128, 6, 128], BF16)
            nc.gpsimd.dma_start(out=wmp, in_=wmp_d[:, :, :])
            wms = consts.tile([C, 6, 128], BF16)
            nc.gpsimd.dma_start(out=wms, in_=wms_d[:, :, :])
            wf2 = consts.tile([128, 2, C], BF16)
            nc.gpsimd.dma_start(out=wf2, in_=wf2_d[:, :, :])
            bq = consts.tile([C, 1], F32)
            nc.gpsimd.dma_start(out=bq, in_=bq_d[:, :])
            bsr = consts.tile([C, 1], F32)
            nc.gpsimd.dma_start(out=bsr, in_=bsr_d[:, :])
            bkv = consts.tile([2 * C, 1], F32)
            nc.gpsimd.dma_start(out=bkv, in_=bkv_d[:, :])
            bg = consts.tile([128, 2], F32)
            nc.gpsimd.dma_start(out=bg, in_=bg_d[:, :])
            bf2 = consts.tile([C, 1], F32)
            nc.gpsimd.dma_start(out=bf2, in_=bf2_d[:, :])
            epst = consts.tile([P, 1], F32)
            nc.vector.memset(epst, 1e-5)

            # ---- load x (token-major), LN1 stats overlapped per slice ----
            x_tm = big.tile([P, T, C], F32, tag="xr")
            x_v = x_d.rearrange("(t p) c -> p t c", p=P)
            sq_scr = big.tile([P, T * C], BF16, tag="scr2", name="sq")
            sqv = sq_scr.rearrange("p (t c) -> p t c", c=C)
            s1 = sc.tile([P, T], F32, tag="s1")
            s2 = sc.tile([P, T], F32, tag="s2")
            for q8 in range(8):
                sl = slice(16 * q8, 16 * (q8 + 1))
                eng = nc.sync if q8 % 2 == 0 else nc.scalar
                eng.dma_start(out=x_tm[:, sl, :], in_=x_v[:, sl, :])
                nc.scalar.square(out=sqv[:, sl, :], in_=x_tm[:, sl, :])
                nc.vector.tensor_reduce(out=s1[:, sl], in_=x_tm[:, sl, :],
                                        axis=AX.X, op=OP.add)
                nc.vector.tensor_reduce(out=s2[:, sl], in_=sqv[:, sl, :],
                                        axis=AX.X, op=OP.add)
            g1, mg1 = _ln_finalize(nc, sc, s1, s2, epst, T)
            # warm up the PE so HAM is at 8/8 when real matmuls start
            for wd in range(15):
                pw = psT.tile([128, 128], F32, tag="tp", name="pw")
                nc.tensor.matmul(out=pw, lhsT=ident, rhs=ident,
                                 start=True, stop=True)
            a1tm = big.tile([P, T, C], BF16, tag="scr2")
            a1cm = big.tile([128, N], BF16, tag="acm")
            a1cm_v = a1cm[0:C, :].rearrange("c (j a b n) -> c j a b n", a=4, b=2, n=128)
            a1tm_v = a1tm.rearrange("p t c -> p (t c)")
            for q8 in range(8):
                sl = slice(16 * q8, 16 * (q8 + 1))
                nc.vector.tensor_tensor(
                    out=a1tm[:, sl, :], in0=x_tm[:, sl, :],
                    in1=g1[:, sl, None].broadcast_to([P, 16, C]), op=OP.mult)
                nc.vector.tensor_tensor(
                    out=a1tm[:, sl, :], in0=a1tm[:, sl, :],
                    in1=mg1[:, sl, None].broadcast_to([P, 16, C]),
                    op=OP.subtract)
                for j in (2 * q8, 2 * q8 + 1):
                    pt = psT.tile([128, 4, 128], BF16, tag="tp")
                    for k in range(4):
                        tt = 8 * j + 2 * k
                        nc.tensor.transpose(out=pt[:, k, :],
                                            in_=a1tm_v[:, 64 * tt:64 * (tt + 2)],
                                            identity=ident)
                    nc.scalar.copy(out=a1cm_v[:, j, :, 0, :], in_=pt[0:C, :, :])
                    nc.vector.tensor_copy(out=a1cm_v[:, j, :, 1, :],
                                          in_=pt[C:128, :, :])
                    nc.vector.tensor_copy(
                        out=a1cm[C:128, 1024 * j:1024 * (j + 1) - 1],
                        in_=a1cm[0:C, 1024 * j + 1:1024 * (j + 1)])
                    if j > 0:
                        nc.gpsimd.tensor_copy(
                            out=a1cm[C:128, 1024 * j - 1:1024 * j],
                            in_=a1cm[0:C, 1024 * j:1024 * j + 1])

            # ---- spatial reduction conv (8x8 stride 8) ----
            a1sr = a1cm.rearrange("c (Y ky X kx) -> c ky kx Y X", ky=SR, kx=SR, X=16)
            psr = psA.tile([128, 512], F32, tag="ps", name="ps").rearrange("c (a y x) -> c a y x", a=2, y=16)[0:C, 0, :, :]
            for pp in range(32):
                ky, kx = pp // 4, (pp % 4) * 2
                nc.tensor.matmul(out=psr, lhsT=wsr[:, pp, :],
                                 rhs=a1sr[:, ky, kx, :, :],
                                 start=(pp == 0), stop=(pp == 31))
            xrcm = consts.tile([C, NR], BF16)
            nc.scalar.activation(out=xrcm.rearrange("c (y x) -> c y x", x=16),
                                 in_=psr, func=AF.Identity,
                                 bias=bsr, scale=1.0)

            # ---- LN on reduced tokens (srn), token-major ----
            xr_tm = consts.tile([P, 2, C], F32)
            for hh in range(2):
                pv = psT.tile([128, C], BF16, tag="tp")
                nc.tensor.transpose(out=pv, in_=xrcm[:, 128 * hh:128 * (hh + 1)],
                                    identity=ident[0:C, 0:C])
                nc.vector.tensor_copy(out=xr_tm[:, hh, :], in_=pv)
            g_r, mg_r = _ln_stats(nc, sc, consts, xr_tm, epst, 2)
            ar_tm = consts.tile([P, 2, C], BF16)
            nc.vector.tensor_tensor(
                out=ar_tm, in0=xr_tm,
                in1=g_r[:, :, None].broadcast_to([P, 2, C]), op=OP.mult)
            mgb = sc.tile([P, 2, C], BF16, tag="mgb")
            nc.vector.tensor_tensor(
                out=mgb, in0=mg_r[:, :, None].broadcast_to([P, 2, C]),
                in1=g_r[:, :, None].broadcast_to([P, 2, C]), op=OP.bypass)
            nc.vector.tensor_tensor(out=ar_tm, in0=ar_tm, in1=mgb, op=OP.subtract)
            arcm = consts.tile([C, NR], BF16)
            for hh in range(2):
                pv = psT.tile([C, 128], BF16, tag="tp")
                nc.tensor.transpose(out=pv, in_=ar_tm[:, hh, :], identity=ident)
                nc.vector.tensor_copy(out=arcm[:, 128 * hh:128 * (hh + 1)], in_=pv)

            # ---- KV ----
            pkv = psA.tile([128, 512], F32, tag="ps", name="ps")[:, 0:NR]
            nc.tensor.matmul(out=pkv, lhsT=wkv, rhs=arcm, start=True, stop=True)
            kvcm = consts.tile([2 * C, NR], BF16)
            nc.scalar.activation(out=kvcm, in_=pkv, func=AF.Identity,
                                 bias=bkv, scale=1.0)
            # fold q-projection into K:  S^T = (K @ Wq) @ A1
            bqb = consts.tile([C, 1], BF16)
            nc.vector.tensor_copy(out=bqb, in_=bq)
            pkw = psT.tile([C, NR], F32, tag="tp", name="pkw")
            nc.tensor.matmul(out=pkw, lhsT=wq, rhs=kvcm[0:C, :],
                             start=True, stop=True)
            kwt = consts.tile([C, NR], BF16)
            nc.scalar.copy(out=kwt, in_=pkw)
            sbias = consts.tile([128, 2], F32)
            for hh in range(2):
                pb = psT.tile([128, 1], F32, tag="tp", name="pb")
                nc.tensor.matmul(out=pb,
                                 lhsT=kvcm[0:C, 128 * hh:128 * (hh + 1)],
                                 rhs=bqb, start=True, stop=True)
                nc.vector.tensor_copy(out=sbias[:, hh:hh + 1], in_=pb)
            vp = consts.tile([128, 2, C + 1], BF16)
            nc.vector.memset(vp[:, :, C:C + 1], 1.0)
            for hh in range(2):
                pv = psT.tile([128, C], BF16, tag="tp")
                nc.tensor.transpose(out=pv,
                                    in_=kvcm[C:2 * C, 128 * hh:128 * (hh + 1)],
                                    identity=ident[C:2 * C, C:2 * C])
                nc.vector.tensor_copy(out=vp[:, hh, 0:C], in_=pv)

            # ---- attention, streamed in 512-column chunks ----
            y_tm = big.tile([P, T, C], F32, tag="y")
            sq2 = big.tile([P, T * C], BF16, tag="scr2", name="sq2")
            sq2v = sq2.rearrange("p (t c) -> p t c", c=C)
            s1y = sc.tile([P, T], F32, tag="s1y")
            s2y = sc.tile([P, T], F32, tag="s2y")
            for i in range(32):
                ech = ch.tile([128, 2, 512], BF16, tag="e")
                for hh in range(2):
                    pS = psA.tile([128, 512], F32, tag="ps", name="ps")
                    nc.tensor.matmul(out=pS,
                                     lhsT=kwt[:, 128 * hh:128 * (hh + 1)],
                                     rhs=a1cm[0:C, 512 * i:512 * (i + 1)],
                                     start=True, stop=True)
                    nc.scalar.activation(out=ech[:, hh, :], in_=pS, func=AF.Exp,
                                         bias=sbias[:, hh:hh + 1], scale=1.0)
                pO = psA.tile([128, 512], F32, tag="ps", name="ps")[0:C + 1, :]
                for hh in range(2):
                    nc.tensor.matmul(out=pO, lhsT=vp[:, hh, :],
                                     rhs=ech[:, hh, :],
                                     start=(hh == 0), stop=(hh == 1))
                pod = ch.tile([C + 1, 512], BF16, tag="pod")
                nc.vector.tensor_copy(out=pod, in_=pO)
                ptr = psT.tile([128, 4, C + 1], F32, tag="tp")
                for k in range(4):
                    nc.tensor.matmul(out=ptr[:, k, :],
                                     lhsT=pod[:, 128 * k:128 * (k + 1)],
                                     rhs=wpj, start=True, stop=True)
                rt = sc.tile([P, 4, 1], F32, tag="rt")
                nc.vector.reciprocal(out=rt, in_=ptr[:, :, C:C + 1])
                tmp = ch.tile([P, 4, C], F32, tag="tmp")
                nc.vector.tensor_tensor(out=tmp, in0=ptr[:, :, 0:C],
                                        in1=rt.broadcast_to([P, 4, C]),
                                        op=OP.mult)
                nc.vector.tensor_tensor(out=y_tm[:, 4 * i:4 * (i + 1), :],
                                        in0=tmp, in1=x_tm[:, 4 * i:4 * (i + 1), :],
                                        op=OP.add)
                if i % 4 == 3:
                    sl = slice(16 * (i // 4), 16 * (i // 4 + 1))
                    nc.scalar.square(out=sq2v[:, sl, :], in_=y_tm[:, sl, :])
                    nc.vector.tensor_reduce(out=s1y[:, sl], in_=y_tm[:, sl, :],
                                            axis=AX.X, op=OP.add)
                    nc.vector.tensor_reduce(out=s2y[:, sl], in_=sq2v[:, sl, :],
                                            axis=AX.X, op=OP.add)

            # ---- LN2 ----
            g2, mg2 = _ln_finalize(nc, sc, s1y, s2y, epst, T)
            a2tm = big.tile([P, T, C], BF16, tag="scr2")
            # ---- A2 guarded channel-major, doubled: rows 64:128 shifted by +1 ----
            a2g = big.tile([128, NG], BF16, tag="acm")
            nc.vector.memset(a2g[:, 0:PAD + RP], 0.0)
            nc.vector.memset(a2g[:, NG - PAD - RP:NG], 0.0)
            a2rows = a2g[0:C, PAD + RP:PAD + RP * (H + 1)].rearrange(
                "c (y w) -> c y w", w=RP)
            a2rowsB = a2g[C:128, PAD + RP:PAD + RP * (H + 1)].rearrange(
                "c (y w) -> c y w", w=RP)
            nc.vector.memset(a2rows[:, :, 0:1], 0.0)
            nc.vector.memset(a2rows[:, :, RP - 1:RP], 0.0)
            nc.vector.memset(a2rowsB[:, :, RP - 2:RP], 0.0)
            a2tm_v = a2tm.rearrange("p t c -> p (t c)")
            ro = a2rows.rearrange("c (j a b) w -> c j a b w", a=4, b=2)
            for q8 in range(8):
                sl = slice(16 * q8, 16 * (q8 + 1))
                nc.vector.tensor_tensor(
                    out=a2tm[:, sl, :], in0=y_tm[:, sl, :],
                    in1=g2[:, sl, None].broadcast_to([P, 16, C]), op=OP.mult)
                nc.vector.tensor_tensor(
                    out=a2tm[:, sl, :], in0=a2tm[:, sl, :],
                    in1=mg2[:, sl, None].broadcast_to([P, 16, C]),
                    op=OP.subtract)
                for j in (2 * q8, 2 * q8 + 1):
                    pt = psT.tile([128, 4, 128], BF16, tag="tp")
                    for k in range(4):
                        tt = 8 * j + 2 * k
                        nc.tensor.transpose(out=pt[:, k, :],
                                            in_=a2tm_v[:, 64 * tt:64 * (tt + 2)],
                                            identity=ident)
                    nc.scalar.copy(out=ro[:, j, :, 0, 1:W + 1], in_=pt[0:C, :, :])
                    nc.vector.tensor_copy(out=ro[:, j, :, 1, 1:W + 1],
                                          in_=pt[C:128, :, :])
                    nc.vector.tensor_copy(
                        out=a2rowsB[:, 8 * j:8 * (j + 1), 0:W],
                        in_=a2rows[:, 8 * j:8 * (j + 1), 1:W + 1])

            # ---- MLP: fused fc1 (+) 3x3 depthwise conv, gelu, fc2 ----
            o2cm = big.tile([C, NG], BF16, tag="qt")
            y2_tm = big.tile([P, T, C], F32, tag="xr")  # reuses x_tm slot
            out_v = out_d.rearrange("(t p) c -> p t c", p=P)
            next_e = 0

            def emit_epi(e):
                pt2 = psT.tile([128, 4, C], BF16, tag="tp", name="pt2")
                for k in range(4):
                    t = 4 * e + k
                    s = PAD + RP * (t + 1) + 1
                    nc.tensor.transpose(out=pt2[:, k, :],
                                        in_=o2cm[:, s:s + W],
                                        identity=ident[0:C, 0:C])
                nc.vector.tensor_tensor(out=y2_tm[:, 4 * e:4 * (e + 1), :],
                                        in0=pt2, in1=y_tm[:, 4 * e:4 * (e + 1), :],
                                        op=OP.add)
                if e % 4 == 3:
                    q8 = e // 4
                    nc.sync.dma_start(out=out_v[:, 16 * q8:16 * (q8 + 1), :],
                                      in_=y2_tm[:, 16 * q8:16 * (q8 + 1), :])
            n_mlp = 33
            for j in range(n_mlp):
                cb = PAD + RP + 512 * j
                size = min(512, PAD + RP * (H + 1) - cb)
                gch = []
                for g in range(2):
                    pG = psA.tile([128, 512], F32, tag="ps", name="ps")
                    for dy in (-1, 0, 1):
                        nc.tensor.matmul(
                            out=pG[:, 0:size], lhsT=wmp[:, 2 * (dy + 1) + g, :],
                            rhs=a2g[:, cb + RP * dy - 1:cb + RP * dy - 1 + size],
                            start=(dy == -1), stop=False)
                    for dy in (-1, 0, 1):
                        nc.tensor.matmul(
                            out=pG[:, 0:size], lhsT=wms[:, 2 * (dy + 1) + g, :],
                            rhs=a2g[0:C, cb + RP * dy + 1:cb + RP * dy + 1 + size],
                            start=False, stop=(dy == 1))
                    gc = ch.tile([128, 512], BF16, tag=f"gc{g}")
                    nc.scalar.activation(out=gc[:, 0:size], in_=pG[:, 0:size],
                                         func=AF.Gelu, bias=bg[:, g:g + 1],
                                         scale=1.0)
                    gch.append(gc)
                pF = psA.tile([128, 512], F32, tag="ps", name="ps")
                for g in range(2):
                    nc.tensor.matmul(out=pF[0:C, 0:size], lhsT=wf2[:, g, :],
                                     rhs=gch[g][:, 0:size],
                                     start=(g == 0), stop=(g == 1))
                nc.vector.tensor_scalar(out=o2cm[:, cb:cb + size],
                                        in0=pF[0:C, 0:size], scalar1=bf2,
                                        scalar2=None, op0=OP.add)
                e_max = (512 * j - 6) // (4 * RP) if j >= 1 else -1
                while next_e <= min(e_max, 31):
                    emit_epi(next_e)
                    next_e += 1
            while next_e < 32:
                emit_epi(next_e)
                next_e += 1

    _split_excess_waits(nc)
    return nc


@functools.cache
def _get_nc():
    return _build_nc()


def _prep_weights(inp):
    f = lambda v: np.asarray(v, np.float32)
    n1w, n1b = f(inp["n1_w"]), f(inp["n1_b"])
    q_w, q_b = f(inp["q_w"]), f(inp["q_b"])
    kv_w, kv_b = f(inp["kv_w"]), f(inp["kv_b"])
    sr_w, sr_b = f(inp["sr_w"]), f(inp["sr_b"])
    srnw, srnb = f(inp["srn_w"]), f(inp["srn_b"])
    pj_w, pj_b = f(inp["proj_w"]), f(inp["proj_b"])
    n2w, n2b = f(inp["n2_w"]), f(inp["n2_b"])
    f1w, f1b = f(inp["fc1_w"]), f(inp["fc1_b"])
    dww, dwb = f(inp["dw_w"]), f(inp["dw_b"])
    f2w, f2b = f(inp["fc2_w"]), f(inp["fc2_b"])

    scale = (C // 1) ** -0.5
    wq_l = (q_w * n1w[None, :]).T * scale
    bq_l = ((q_w @ n1b + q_b) * scale)[:, None]

    wsr_l = np.zeros((32, 128, C), np.float32)
    for pp in range(32):
        ky, kx = pp // 4, (pp % 4) * 2
        wsr_l[pp, :C, :] = (sr_w[:, :, ky, kx] * n1w[None, :]).T
        wsr_l[pp, C:, :] = (sr_w[:, :, ky, kx + 1] * n1w[None, :]).T
    wsr_l = wsr_l.transpose(1, 0, 2)
    bsr_l = (sr_w.sum((2, 3)) @ n1b + sr_b)[:, None]

    wkv_l = (kv_w * srnw[None, :]).T
    bkv_l = (kv_w @ srnb + kv_b)[:, None]

    wpj_l = np.zeros((C + 1, C + 1), np.float32)
    wpj_l[:C, :C] = pj_w.T
    wpj_l[C, :C] = pj_b
    wpj_l[C, C] = 1.0

    k9 = dww[:, 0, :, :].reshape(HID, 9)          # [256, 9]
    wmp_l = np.zeros((6, 128, 128), np.float32)
    wms_l = np.zeros((6, C, 128), np.float32)
    for dy in range(3):
        for g in range(2):
            Ma = (k9[:, dy * 3 + 0][:, None] * f1w * n2w[None, :])[128 * g:128 * (g + 1)]
            Mb = (k9[:, dy * 3 + 1][:, None] * f1w * n2w[None, :])[128 * g:128 * (g + 1)]
            Mc = (k9[:, dy * 3 + 2][:, None] * f1w * n2w[None, :])[128 * g:128 * (g + 1)]
            wmp_l[2 * dy + g, :C, :] = Ma.T
            wmp_l[2 * dy + g, C:, :] = Mb.T
            wms_l[2 * dy + g, :, :] = Mc.T
    wmp_l = wmp_l.transpose(1, 0, 2)
    wms_l = wms_l.transpose(1, 0, 2)
    bg_full = k9.sum(1) * (f1w @ n2b + f1b) + dwb  # [256]
    bg_l = np.ascontiguousarray(bg_full.reshape(2, 128).T)

    wf2_l = np.stack([f2w[:, :128].T, f2w[:, 128:].T], 0).transpose(1, 0, 2)
    bf2_l = f2b[:, None]

    bfc = lambda a: np.ascontiguousarray(a).astype(BF)
    return {
        "wq": bfc(wq_l), "bq": np.ascontiguousarray(bq_l),
        "wsr": bfc(wsr_l), "bsr": np.ascontiguousarray(bsr_l),
        "wkv": bfc(wkv_l), "bkv": np.ascontiguousarray(bkv_l),
        "wpj": bfc(wpj_l),
        "wmp": bfc(wmp_l), "wms": bfc(wms_l),
        "bg": np.ascontiguousarray(bg_l),
        "wf2": bfc(wf2_l), "bf2": np.ascontiguousarray(bf2_l),
    }


def kernel(trace=False, tmpdir=None, **inputs):
    nc = _get_nc()
    x = np.asarray(inputs["x"], np.float32)
    wts = _prep_weights(inputs)
    in_maps = [dict(wts, x=np.ascontiguousarray(x[b])) for b in range(B)]
    res = run_bass_kernel_spmd(nc, in_maps, core_ids=list(range(8)),
                               trace=trace, tmpdir=tmpdir)
    out = np.stack([res.results[b]["out"] for b in range(B)], 0)
    kernel.last_exec_time_ns = res.exec_time_ns
    return out
